# revision 9
# baseline (speedup 1.0000x reference)
"""Trainium2 Bass kernel v2 for nn_DetectionLoss — fp16 grid pipeline.

Data-parallel: 16 images over 8 cores (2 images/core). Per image, the
[A=65536, G=32] match grid is computed in fp16 (coords pre-scaled by 1/64 —
the loss is scale-invariant) in supertile-(g,u) layout so every DVE op is
packed-innermost and runs in 2x/4x mode. Division uses the Act engine's
table Reciprocal (~0.5% rel err — only feeds match *selection*, fine at the
2e-2 gate). Matched-gt coords go through fp16 PE transposes of the one-hot
grid + 16-wide block-diag matmuls. DIoU runs in fp16 on coord planes; focal
+ hard-negative mining keep the baseline f32/fp16 structure. Host combines
per-image scalars exactly like the reference."""
import sys

sys.path.insert(0, '/opt/trn_rl_repo')

import numpy as np
import concourse.bass as bass
import concourse.bacc as bacc
import concourse.mybir as mybir
from concourse.tile import TileContext
from concourse.bass_utils import run_bass_kernel_spmd
from concourse.masks import make_identity
from contextlib import ExitStack

Alu = mybir.AluOpType
Act = mybir.ActivationFunctionType
Ax = mybir.AxisListType
F32 = mybir.dt.float32
FP16 = mybir.dt.float16

P = 128
A = 65536
G = 32
IMG = 2
NCORE = 8
COLS = A // P       # 512
U = 32
W = G * U           # 1024 els per supertile block
NSUP = COLS // U    # 16
NQ = 4
QSUP = NSUP // NQ   # 4
QW = QSUP * W       # 4096
SC = 1.0 / 64.0
POS_THR = 1.0 / 3.0
NBIN = 16
NLEV = 2
NEG_POS_RATIO = 3.0


def _act_recip(nc, out, in_):
    """Raw Act-engine Reciprocal (table approx, ~0.5% rel err)."""
    sc = nc.scalar
    ins = [sc.lower_ap(in_)]
    for argv in (0.0, 1.0, 0.0):
        ins.append(mybir.ImmediateValue(dtype=mybir.dt.float32, value=argv))
    return sc.add_instruction(
        mybir.InstActivation(name=nc.get_next_instruction_name(),
                             func=Act.Reciprocal, ins=ins,
                             outs=[sc.lower_ap(out)]))


def _build_nc():
    nc = bacc.Bacc("TRN2", target_bir_lowering=False, debug=False)
    anch_d = nc.dram_tensor("anch", [P, COLS * 4], F32, kind="ExternalInput")
    bbox_d = nc.dram_tensor("bbox", [IMG, P, COLS * 4], F32, kind="ExternalInput")
    conf_d = nc.dram_tensor("conf", [IMG, P, COLS], F32, kind="ExternalInput")
    gt_d = nc.dram_tensor("gtb", [IMG, 1, G * 4], F32, kind="ExternalInput")
    res_d = nc.dram_tensor("res", [IMG, 1, 8], F32, kind="ExternalOutput")

    v = nc.vector
    sc = nc.scalar
    pe = nc.tensor

    with TileContext(nc) as tc, ExitStack() as ctx, \
            nc.allow_low_precision(reason="fp16 grid; host checks rel err"):
        pool = ctx.enter_context(tc.tile_pool(name="main", bufs=1))
        pspool = ctx.enter_context(tc.tile_pool(name="ps", bufs=1, space="PSUM"))

        def T(name, cols, parts=P, dt=F32):
            return pool.tile([parts, cols], dt, name=name)

        def T16(name, cols, parts=P):
            return pool.tile([parts, cols], FP16, name=name)

        # ---------------- per-core constants ----------------
        anch_sb = T("anch_sb", COLS * 4)
        nc.sync.dma_start(anch_sb[:, 0:COLS * 2], anch_d[:, 0:COLS * 2])
        nc.sync.dma_start(anch_sb[:, COLS * 2:], anch_d[:, COLS * 2:])
        anch3 = anch_sb[:].rearrange("p (n c) -> p n c", c=4)

        ax2h = T16("ax2h", COLS)
        ay2h = T16("ay2h", COLS)
        nax1h = T16("nax1h", COLS)
        nay1h = T16("nay1h", COLS)
        areaAh = T16("areaAh", COLS)
        f0 = T("f0", COLS)
        f1 = T("f1", COLS)
        sc.activation(ax2h[:], anch3[:, :, 2:3].squeeze(2), Act.Copy, scale=SC)
        sc.activation(ay2h[:], anch3[:, :, 3:4].squeeze(2), Act.Copy, scale=SC)
        sc.activation(nax1h[:], anch3[:, :, 0:1].squeeze(2), Act.Copy, scale=-SC)
        sc.activation(nay1h[:], anch3[:, :, 1:2].squeeze(2), Act.Copy, scale=-SC)
        v.tensor_tensor(out=f0[:], in0=anch3[:, :, 2:3].squeeze(2),
                        in1=anch3[:, :, 0:1].squeeze(2), op=Alu.subtract)
        v.tensor_tensor(out=f1[:], in0=anch3[:, :, 3:4].squeeze(2),
                        in1=anch3[:, :, 1:2].squeeze(2), op=Alu.subtract)
        v.tensor_tensor(out=f0[:], in0=f0[:], in1=f1[:], op=Alu.mult)
        sc.activation(areaAh[:], f0[:], Act.Copy, scale=SC * SC)

        ident = T16("ident", P)
        make_identity(nc, ident[:])
        identf = T("identf", P)
        make_identity(nc, identf[:])
        ones_col = T("ones_col", 1)
        ones_row = T("ones_row", P, parts=1)
        ones_row16 = T16("ones_row16", P, parts=1)
        v.memset(ones_col[:], 1.0)
        v.memset(ones_row[:], 1.0)
        v.memset(ones_row16[:], 1.0)
        iota_i = pool.tile([P, NBIN], mybir.dt.int32, name="iota_i")
        nc.gpsimd.iota(iota_i[:], pattern=[[1, NBIN]], base=0, channel_multiplier=0)
        iota_f = T("iota_f", NBIN)
        v.tensor_copy(iota_f[:], iota_i[:])

        def pbcast(dst, src_row):
            n = src_row.shape[-1]
            bc_ps = pspool.tile([P, G], F32, name="bc_ps", tag="pss")
            nc.tensor.matmul(bc_ps[:, 0:n], ones_row[:], src_row)
            v.tensor_copy(dst, bc_ps[:, 0:n])

        # ---------------- shared big tiles ----------------
        grid = T16("grid", NSUP * W)     # r values, (s, g, u) blocks
        tgrid = T16("tgrid", NSUP * W)   # one-hot, (s, u, g) blocks
        sA_l = [T16(f"sA{k}", QW) for k in range(2)]
        sB_l = [T16(f"sB{k}", QW) for k in range(2)]
        sCt_l = [T16("sCt0", QW)] * 2
        rm = T16("rm", COLS)
        colT = T16("colT", NSUP * G)
        cpa = T16("cpa", G)
        forced = T16("forced", COLS)
        pos16 = T16("pos16", COLS)
        posf = T("posf", COLS)
        cmb = T16("cmb", G)
        cmaxpl = T16("cmaxpl", W)

        gtrow_l = [T(f"gtrow{b}", G * 4, parts=1) for b in range(IMG)]
        gtall_l = [T(f"gtall{b}", G * 4) for b in range(IMG)]
        gsc = [T(f"gsc{c}", G) for c in range(4)]
        sGf = T("sGf", G)
        tG = T("tG", G)
        ngx1p = T16("ngx1p", W)
        ngy1p = T16("ngy1p", W)
        gx2p = T16("gx2p", W)
        gy2p = T16("gy2p", W)
        sGp = T16("sGp", W)
        gtmatf_l = [T(f"gtmatf{b}", 16) for b in range(IMG)]
        gtmath_l = [T16(f"gtmath{b}", 16) for b in range(IMG)]

        tsb = T16("tsb", W)
        matched = T16("matched", 4 * COLS)   # (s, u, c)
        mch = [T16(f"mch{c}", COLS) for c in range(4)]
        bxh = [T16(f"bxh{c}", COLS) for c in range(4)]
        areaPh = T16("areaPh", COLS)
        bbox_sb_l = [T(f"bbox_sb{b}", COLS * 4) for b in range(IMG)]
        conf_sb_l = [T(f"conf_sb{b}", COLS) for b in range(IMG)]

        d0 = T16("d0", COLS)
        d1 = T16("d1", COLS)
        d2 = T16("d2", COLS)
        d3 = T16("d3", COLS)
        d4 = T16("d4", COLS)

        s0 = T("s0", COLS)
        s1 = T("s1", COLS)
        s2 = T("s2", COLS)
        s3 = T("s3", COLS)
        s4 = T("s4", COLS)
        s5 = T("s5", COLS)
        cl = T("cl", COLS)
        nv = T("nv", COLS)
        nv16 = T16("nv16", COLS)
        sink16 = T16("sink16", COLS)

        npp = T("npp", 1)
        locsum_pp = T("locsum_pp", 1)
        possum_pp = T("possum_pp", 1)
        cnt_pp = T("cnt_pp", 1)
        sum_pp = T("sum_pp", 1)
        maxv_pp = T("maxv_pp", 1)
        maxvb = T("maxvb", 1)
        w1c = T("w1c", 1)
        tau_b = T("tau_b", 1)
        stack = T("stack", 4)
        thr = T("thr", NBIN)
        nthr = T("nthr", NBIN)
        cge = T("cge", NBIN)
        wl = [T(f"wl{l}", 1) for l in range(NLEV)]
        lo_b = [T(f"lo_b{l}", 1) for l in range(NLEV)]
        cget = T("cget", NBIN, parts=1)
        gek = T("gek", NBIN, parts=1)
        scnt = T("scnt", 1, parts=1)
        lo_new = T("lo_new", 1, parts=1)
        tau = [T(f"tau{l}", 1, parts=1) for l in range(NLEV)]
        maxv1 = T("maxv1", 1, parts=1)
        npos1 = T("npos1", 1, parts=1)
        k1 = T("k1", 1, parts=1)
        k2 = T("k2", 1, parts=1)
        kk = T("kk", 1, parts=1)
        mx_row = T("mx_row", P, parts=1)
        ctt = T16("ctt", P, parts=G)
        cmax_col = T16("cmax_col", 1, parts=G)
        cm_row = T16("cm_row", G, parts=1)
        res_sb = T("res_sb", 8, parts=1)

        # 4D view helpers
        def q4gu(t, q):  # (s, g, u) packed quarter
            return t[:, q*QW:(q+1)*QW].rearrange("p (s g u) -> p s g u", g=G, u=U)

        def aview(t, q):  # anchor [P, COLS] -> [p, s, g(b), u]
            return (t[:, q*QSUP*U:(q+1)*QSUP*U]
                    .rearrange("p (s u) -> p s u", u=U)
                    .unsqueeze(2).to_broadcast([P, QSUP, G, U]))

        def gview(t):     # gt plane [P, W] -> [p, s(b), g, u]
            return (t[:].rearrange("p (g u) -> p g u", u=U)
                    .unsqueeze(1).to_broadcast([P, QSUP, G, U]))

        for b in range(IMG):
            bbox_sb = bbox_sb_l[b]
            conf_sb = conf_sb_l[b]
            gtrow = gtrow_l[b]
            gtall = gtall_l[b]
            gtmatf = gtmatf_l[b]
            gtmath = gtmath_l[b]
            # ---------------- loads ----------------
            nc.sync.dma_start(bbox_sb[:, 0:COLS * 2], bbox_d[b][:, 0:COLS * 2])
            nc.sync.dma_start(bbox_sb[:, COLS * 2:], bbox_d[b][:, COLS * 2:])
            nc.sync.dma_start(conf_sb[:], conf_d[b])
            nc.scalar.dma_start(gtrow[:], gt_d[b])
            gt_ps = pspool.tile([P, G * 4], F32, name="gt_ps", tag="gtp")
            nc.tensor.matmul(gt_ps[:], ones_row[:], gtrow[:])
            v.tensor_copy(gtall[:], gt_ps[:])
            gt3 = gtall[:].rearrange("p (g c) -> p g c", c=4)
            gt2d = gt_d[b].rearrange("q (g c) -> (q g) c", c=4)
            v.memset(gtmatf[:], 0.0)
            for uu in range(4):
                nc.scalar.dma_start(gtmatf[uu*G:(uu+1)*G, uu*4:(uu+1)*4], gt2d)
            sc.activation(gtmath[:], gtmatf[:], Act.Copy, scale=SC)

            # ---------------- gt prep ----------------
            v.tensor_scalar(gsc[0][:], gt3[:, :, 0:1].squeeze(2), -SC, None, Alu.mult)
            v.tensor_scalar(gsc[1][:], gt3[:, :, 1:2].squeeze(2), -SC, None, Alu.mult)
            v.tensor_scalar(gsc[2][:], gt3[:, :, 2:3].squeeze(2), SC, None, Alu.mult)
            v.tensor_scalar(gsc[3][:], gt3[:, :, 3:4].squeeze(2), SC, None, Alu.mult)
            v.tensor_tensor(out=sGf[:], in0=gt3[:, :, 2:3].squeeze(2),
                            in1=gt3[:, :, 0:1].squeeze(2), op=Alu.subtract)
            v.tensor_tensor(out=tG[:], in0=gt3[:, :, 3:4].squeeze(2),
                            in1=gt3[:, :, 1:2].squeeze(2), op=Alu.subtract)
            v.tensor_tensor(out=sGf[:], in0=sGf[:], in1=tG[:], op=Alu.mult)
            v.tensor_scalar(sGf[:], sGf[:], SC * SC, None, Alu.mult)
            for pl, src in ((ngx1p, gsc[0]), (ngy1p, gsc[1]), (gx2p, gsc[2]),
                            (gy2p, gsc[3]), (sGp, sGf)):
                sc.activation(pl[:].rearrange("p (g u) -> p g u", u=U),
                              src[:].unsqueeze(2).to_broadcast([P, G, U]), Act.Copy)

            # ---------------- pass 1: grid ----------------
            for q in range(NQ):
                sA = sA_l[q % 2]
                sB = sB_l[q % 2]
                sCt = sCt_l[q % 2]
                gq = q4gu(grid, q)
                a4 = q4gu(sA, 0)
                b4 = q4gu(sB, 0)
                c4 = q4gu(sCt, 0)
                v.tensor_tensor(out=a4, in0=aview(nax1h, q), in1=gview(ngx1p), op=Alu.min)
                v.tensor_tensor(out=b4, in0=aview(ax2h, q), in1=gview(gx2p), op=Alu.min)
                v.tensor_tensor(out=a4, in0=b4, in1=a4, op=Alu.add)
                v.tensor_tensor(out=b4, in0=aview(nay1h, q), in1=gview(ngy1p), op=Alu.min)
                v.tensor_tensor(out=c4, in0=aview(ay2h, q), in1=gview(gy2p), op=Alu.min)
                v.tensor_tensor(out=b4, in0=c4, in1=b4, op=Alu.add)
                v.tensor_scalar(sA[:], sA[:], 0.0, None, Alu.max)
                v.tensor_scalar(sB[:], sB[:], 0.0, None, Alu.max)
                v.tensor_tensor(out=a4, in0=a4, in1=b4, op=Alu.mult)      # inter
                v.tensor_tensor(out=b4, in0=aview(areaAh, q), in1=gview(sGp),
                                op=Alu.add)                               # S
                _act_recip(nc, sB[:], sB[:])
                v.tensor_tensor(out=gq, in0=a4, in1=b4, op=Alu.mult)      # r

                cur = gq
                width = G
                while width > 1:
                    h = width // 2
                    if h == 1:
                        dst = (rm[:, q*QSUP*U:(q+1)*QSUP*U]
                               .rearrange("p (s u) -> p s u", u=U).unsqueeze(2))
                    else:
                        dst = q4gu(sCt, 0)[:, :, 0:h, :]
                    v.tensor_tensor(out=dst, in0=cur[:, :, 0:h, :],
                                    in1=cur[:, :, h:width, :], op=Alu.max)
                    cur = dst
                    width = h
                curc = gq
                widc = U
                while widc > 1:
                    hc = widc // 2
                    if hc == 1:
                        dstc = (colT[:, q*QSUP*G:(q+1)*QSUP*G]
                                .rearrange("p (s g) -> p s g", g=G).unsqueeze(3))
                    else:
                        dstc = q4gu(sB, 0)[:, :, :, 0:hc]
                    v.tensor_tensor(out=dstc, in0=curc[:, :, :, 0:hc],
                                    in1=curc[:, :, :, hc:widc], op=Alu.max)
                    curc = dstc
                    widc = hc

            # ---------------- cmax finalize ----------------
            v.tensor_reduce(out=cpa[:], in_=colT[:].rearrange("p (s g) -> p g s", g=G),
                            axis=Ax.X, op=Alu.max)
            ct_ps = pspool.tile([G, P], FP16, name="ct_ps", tag="pst")
            pe.transpose(ct_ps[:], cpa[:], ident[:])
            v.tensor_copy(ctt[:], ct_ps[:])
            v.tensor_reduce(out=cmax_col[:], in_=ctt[:], axis=Ax.X, op=Alu.max)
            cm_ps = pspool.tile([1, G], FP16, name="cm_ps", tag="pst")
            pe.transpose(cm_ps[:], cmax_col[:], ident[:G, :G])
            v.tensor_copy(cm_row[:], cm_ps[:])
            bc_ps = pspool.tile([P, G], F32, name="bc_ps", tag="pss")
            nc.tensor.matmul(bc_ps[:], ones_row16[:], cm_row[:])
            v.tensor_copy(cmb[:], bc_ps[:])
            sc.activation(cmaxpl[:].rearrange("p (g u) -> p g u", u=U),
                          cmb[:].unsqueeze(2).to_broadcast([P, G, U]), Act.Copy)

            # ---------------- forced + pos ----------------
            for q in range(NQ):
                sA = sA_l[q % 2]
                gq = q4gu(grid, q)
                v.tensor_tensor(out=q4gu(sA, 0), in0=gq, in1=gview(cmaxpl),
                                op=Alu.is_equal)
                cur = q4gu(sA, 0)
                width = G
                while width > 1:
                    h = width // 2
                    if h == 1:
                        dst = (forced[:, q*QSUP*U:(q+1)*QSUP*U]
                               .rearrange("p (s u) -> p s u", u=U).unsqueeze(2))
                    else:
                        dst = cur[:, :, 0:h, :]
                    v.tensor_tensor(out=dst, in0=cur[:, :, 0:h, :],
                                    in1=cur[:, :, h:width, :], op=Alu.max)
                    cur = dst
                    width = h
            v.tensor_scalar(pos16[:], rm[:], POS_THR, None, Alu.is_gt)
            v.tensor_tensor(out=pos16[:], in0=pos16[:], in1=forced[:], op=Alu.max)
            v.tensor_scalar(sink16[:], pos16[:], 1.0, 0.0, Alu.mult, Alu.add,
                            accum_out=npp[:])
            v.tensor_copy(posf[:], pos16[:])

            # ---------------- ismax -> tgrid (s, u, g) ----------------
            for q in range(NQ):
                gin = grid[:, q*QW:(q+1)*QW].rearrange(
                    "p (s g u) -> p s u g", g=G, u=U)
                rmv = (rm[:, q*QSUP*U:(q+1)*QSUP*U]
                       .rearrange("p (s u) -> p s u", u=U)
                       .unsqueeze(3).to_broadcast([P, QSUP, U, G]))
                tout = tgrid[:, q*QW:(q+1)*QW].rearrange(
                    "p (s u g) -> p s u g", u=U, g=G)
                v.tensor_tensor(out=tout, in0=gin, in1=rmv, op=Alu.is_equal)

            # ---------------- matched coords (PE) ----------------
            for s in range(NSUP):
                mout = pspool.tile([P, P], F32, name=f"mo{s % 2}", tag=f"mo{s % 2}")
                for j in range(8):
                    tp = pspool.tile([P, P], FP16, name=f"tp{j % 2}", tag=f"tp{j % 2}")
                    pe.transpose(tp[:], tgrid[:, s*W + j*P: s*W + (j+1)*P], ident[:])
                    sc.copy(tsb[:, j*P:(j+1)*P], tp[:])
                    nc.tensor.matmul(mout[:, j*16:(j+1)*16], tsb[:, j*P:(j+1)*P],
                                     gtmath[:])
                sc.copy(matched[:, s*P:(s+1)*P], mout[:])

            m4 = matched[:].rearrange("p (n c) -> p n c", c=4)
            for c in range(4):
                sc.copy(mch[c][:], m4[:, :, c:c+1].squeeze(2))

            # ---------------- bbox planes + diou ----------------
            bb3 = bbox_sb[:].rearrange("p (n c) -> p n c", c=4)
            for c in range(4):
                sc.activation(bxh[c][:], bb3[:, :, c:c+1].squeeze(2), Act.Copy,
                              scale=SC)
            v.tensor_tensor(out=d0[:], in0=bxh[2][:], in1=bxh[0][:], op=Alu.subtract)
            v.tensor_tensor(out=d1[:], in0=bxh[3][:], in1=bxh[1][:], op=Alu.subtract)
            v.tensor_tensor(out=areaPh[:], in0=d0[:], in1=d1[:], op=Alu.mult)

            # inter
            v.tensor_tensor(out=d0[:], in0=bxh[0][:], in1=mch[0][:], op=Alu.max)
            v.tensor_tensor(out=d1[:], in0=bxh[2][:], in1=mch[2][:], op=Alu.min)
            v.tensor_tensor(out=d0[:], in0=d1[:], in1=d0[:], op=Alu.subtract)
            v.tensor_scalar(d0[:], d0[:], 0.0, None, Alu.max)
            v.tensor_tensor(out=d1[:], in0=bxh[1][:], in1=mch[1][:], op=Alu.max)
            v.tensor_tensor(out=d2[:], in0=bxh[3][:], in1=mch[3][:], op=Alu.min)
            v.tensor_tensor(out=d1[:], in0=d2[:], in1=d1[:], op=Alu.subtract)
            v.tensor_scalar(d1[:], d1[:], 0.0, None, Alu.max)
            v.tensor_tensor(out=d0[:], in0=d0[:], in1=d1[:], op=Alu.mult)  # inter
            # matched area
            v.tensor_tensor(out=d1[:], in0=mch[2][:], in1=mch[0][:], op=Alu.subtract)
            v.tensor_tensor(out=d2[:], in0=mch[3][:], in1=mch[1][:], op=Alu.subtract)
            v.tensor_tensor(out=d1[:], in0=d1[:], in1=d2[:], op=Alu.mult)
            # union, iou
            v.tensor_tensor(out=d1[:], in0=d1[:], in1=areaPh[:], op=Alu.add)
            v.tensor_tensor(out=d1[:], in0=d1[:], in1=d0[:], op=Alu.subtract)
            _act_recip(nc, d1[:], d1[:])
            v.tensor_tensor(out=d0[:], in0=d0[:], in1=d1[:], op=Alu.mult)  # iou
            # enclosing c2
            v.tensor_tensor(out=d1[:], in0=bxh[0][:], in1=mch[0][:], op=Alu.min)
            v.tensor_tensor(out=d2[:], in0=bxh[2][:], in1=mch[2][:], op=Alu.max)
            v.tensor_tensor(out=d1[:], in0=d2[:], in1=d1[:], op=Alu.subtract)
            v.tensor_tensor(out=d1[:], in0=d1[:], in1=d1[:], op=Alu.mult)
            v.tensor_tensor(out=d2[:], in0=bxh[1][:], in1=mch[1][:], op=Alu.min)
            v.tensor_tensor(out=d3[:], in0=bxh[3][:], in1=mch[3][:], op=Alu.max)
            v.tensor_tensor(out=d2[:], in0=d3[:], in1=d2[:], op=Alu.subtract)
            v.tensor_tensor(out=d2[:], in0=d2[:], in1=d2[:], op=Alu.mult)
            v.tensor_tensor(out=d1[:], in0=d1[:], in1=d2[:], op=Alu.add)   # c2
            _act_recip(nc, d1[:], d1[:])
            # center dist d2 (quarter-scaled: absorbed by c2 ratio using same /2)
            v.tensor_tensor(out=d2[:], in0=bxh[0][:], in1=bxh[2][:], op=Alu.add)
            v.tensor_tensor(out=d3[:], in0=mch[0][:], in1=mch[2][:], op=Alu.add)
            v.tensor_tensor(out=d2[:], in0=d2[:], in1=d3[:], op=Alu.subtract)
            v.tensor_tensor(out=d2[:], in0=d2[:], in1=d2[:], op=Alu.mult)
            v.tensor_tensor(out=d3[:], in0=bxh[1][:], in1=bxh[3][:], op=Alu.add)
            v.tensor_tensor(out=d4[:], in0=mch[1][:], in1=mch[3][:], op=Alu.add)
            v.tensor_tensor(out=d3[:], in0=d3[:], in1=d4[:], op=Alu.subtract)
            v.tensor_tensor(out=d3[:], in0=d3[:], in1=d3[:], op=Alu.mult)
            v.tensor_tensor(out=d2[:], in0=d2[:], in1=d3[:], op=Alu.add)   # 4*d2
            v.tensor_tensor(out=d2[:], in0=d2[:], in1=d1[:], op=Alu.mult)
            v.tensor_scalar(d2[:], d2[:], 0.25, None, Alu.mult)            # d2/c2
            v.tensor_scalar(d0[:], d0[:], -1.0, 1.0, Alu.mult, Alu.add)    # 1-iou
            v.tensor_tensor(out=d2[:], in0=d2[:], in1=d0[:], op=Alu.add)
            v.tensor_scalar(d2[:], d2[:], 100.0, None, Alu.min)
            v.tensor_tensor(out=d2[:], in0=d2[:], in1=pos16[:], op=Alu.mult)
            v.tensor_scalar(sink16[:], d2[:], 1.0, 0.0, Alu.mult, Alu.add,
                            accum_out=locsum_pp[:])

            # ---------------- focal conf loss (f32, baseline) ----------------
            sc.activation(s0[:], conf_sb[:], Act.Sigmoid)
            sc.activation(s1[:], conf_sb[:], Act.Exp)
            sc.activation(s1[:], s1[:], Act.Ln, bias=1.0)
            v.tensor_tensor(out=s2[:], in0=conf_sb[:], in1=posf[:], op=Alu.mult)
            v.tensor_tensor(out=s2[:], in0=s1[:], in1=s2[:], op=Alu.subtract)
            v.tensor_scalar(s3[:], posf[:], -2.0, 1.0, Alu.mult, Alu.add)
            v.tensor_tensor(out=s3[:], in0=s0[:], in1=s3[:], op=Alu.mult)
            v.tensor_tensor(out=s3[:], in0=s3[:], in1=posf[:], op=Alu.add)
            sc.activation(s3[:], s3[:], Act.Square)
            v.tensor_tensor(out=cl[:], in0=s3[:], in1=s2[:], op=Alu.mult)
            v.tensor_scalar(s3[:], posf[:], -0.5, 0.75, Alu.mult, Alu.add)
            v.tensor_tensor(out=cl[:], in0=cl[:], in1=s3[:], op=Alu.mult)
            v.tensor_scalar(cl[:], cl[:], 100.0, None, Alu.min)
            v.tensor_tensor(out=s4[:], in0=cl[:], in1=posf[:], op=Alu.mult)
            v.tensor_scalar(s5[:], s4[:], 1.0, 0.0, Alu.mult, Alu.add,
                            accum_out=possum_pp[:])
            v.tensor_tensor(out=nv[:], in0=cl[:], in1=s4[:], op=Alu.subtract)
            v.tensor_copy(nv16[:], nv[:])

            # ---------------- hard negative mining (baseline) ----------------
            v.tensor_reduce(out=maxv_pp[:], in_=nv[:], axis=Ax.X, op=Alu.max)
            mx_ps = pspool.tile([1, P], F32, name="mx_ps", tag="pss")
            pe.transpose(mx_ps[:], maxv_pp[:], identf[:])
            v.tensor_copy(mx_row[:], mx_ps[:])
            v.tensor_reduce(out=maxv1[:], in_=mx_row[:], axis=Ax.X, op=Alu.max)

            np_ps = pspool.tile([1, 1], F32, name="np_ps", tag="pss")
            nc.tensor.matmul(np_ps[:], ones_col[:], npp[:])
            v.tensor_copy(npos1[:], np_ps[:])
            v.tensor_scalar(k1[:], npos1[:], NEG_POS_RATIO, None, Alu.mult)
            v.tensor_scalar(k2[:], npos1[:], -1.0, float(A), Alu.mult, Alu.add)
            v.tensor_tensor(out=kk[:], in0=k1[:], in1=k2[:], op=Alu.min)

            pbcast(maxvb[:], maxv1[:])
            v.tensor_scalar(w1c[:], maxvb[:], 1.0 / NBIN, None, Alu.mult)

            for lev in range(NLEV):
                if lev == 0:
                    v.tensor_copy(wl[0][:], w1c[:])
                    v.tensor_scalar(thr[:], iota_f[:], wl[0][:], None, Alu.mult)
                else:
                    v.tensor_scalar(wl[lev][:], wl[lev - 1][:], 1.0 / NBIN, None,
                                    Alu.mult)
                    v.tensor_scalar(thr[:], iota_f[:], wl[lev][:], lo_b[lev - 1][:],
                                    Alu.mult, Alu.add)
                v.tensor_scalar(nthr[:], thr[:], -1.0, None, Alu.mult)
                for bn in range(NBIN):
                    sc.activation(sink16[:], nv16[:], Act.Sign,
                                  bias=nthr[:, bn:bn+1], accum_out=cge[:, bn:bn+1])
                cg_ps = pspool.tile([1, NBIN], F32, name="cg_ps", tag="pss")
                nc.tensor.matmul(cg_ps[:], ones_col[:], cge[:])
                v.tensor_copy(cget[:], cg_ps[:])
                # sign-sum -> count_gt: cnt = (acc + A)/2 (ties ~0, absorbed)
                v.tensor_scalar(cget[:], cget[:], 0.5, float(A) * 0.5, Alu.mult,
                                Alu.add)
                v.tensor_scalar(gek[:], cget[:], kk[:], None, Alu.is_ge)
                v.tensor_reduce(out=scnt[:], in_=gek[:], axis=Ax.X, op=Alu.add)
                v.tensor_scalar(lo_new[:], scnt[:], 1.0, wl[lev][0:1, :],
                                Alu.subtract, Alu.mult)
                v.tensor_scalar(tau[lev][:], scnt[:], wl[lev][0:1, :], None, Alu.mult)
                if lev > 0:
                    v.tensor_tensor(out=lo_new[:], in0=lo_new[:],
                                    in1=lo_b[lev - 1][0:1, :], op=Alu.add)
                    v.tensor_tensor(out=tau[lev][:], in0=tau[lev][:],
                                    in1=lo_b[lev - 1][0:1, :], op=Alu.add)
                pbcast(lo_b[lev][:], lo_new[:])

            pbcast(tau_b[:], tau[NLEV - 1][:])
            v.tensor_scalar(s4[:], nv[:], tau_b[:], 0.0, Alu.is_gt,
                            Alu.add, accum_out=cnt_pp[:])
            v.tensor_tensor(out=s5[:], in0=nv[:], in1=s4[:], op=Alu.mult)
            v.tensor_scalar(s5[:], s5[:], 1.0, 0.0, Alu.mult, Alu.add,
                            accum_out=sum_pp[:])

            # ---------------- gather scalars ----------------
            v.tensor_copy(stack[:, 0:1], npp[:])
            v.tensor_copy(stack[:, 1:2], locsum_pp[:])
            v.tensor_copy(stack[:, 2:3], possum_pp[:])
            v.tensor_copy(stack[:, 3:4], cnt_pp[:])
            st_ps = pspool.tile([1, 4], F32, name="st_ps", tag="pss")
            nc.tensor.matmul(st_ps[:], ones_col[:], stack[:])
            sm_ps = pspool.tile([1, 1], F32, name="sm_ps", tag="pss")
            nc.tensor.matmul(sm_ps[:], ones_col[:], sum_pp[:])

            v.tensor_copy(res_sb[:, 0:4], st_ps[:])
            v.tensor_copy(res_sb[:, 4:5], sm_ps[:])
            v.tensor_copy(res_sb[:, 5:6], tau[NLEV - 1][:])
            v.tensor_copy(res_sb[:, 6:7], maxv1[:])
            v.tensor_copy(res_sb[:, 7:8], kk[:])
            nc.sync.dma_start(res_d[b], res_sb[:])

    nc.compile()
    return nc


_NC_CACHE = None


def _get_nc():
    global _NC_CACHE
    if _NC_CACHE is None:
        _NC_CACHE = _build_nc()
    return _NC_CACHE


def _make_in_maps(inputs):
    bbox_pred = np.asarray(inputs["bbox_pred"])
    conf_pred = np.asarray(inputs["conf_pred"])
    anchors = np.asarray(inputs["anchors"])
    gt_boxes = np.asarray(inputs["gt_boxes"])
    anch_h = np.ascontiguousarray(anchors.reshape(P, COLS * 4), dtype=np.float32)
    in_maps = []
    for i in range(NCORE):
        bsl = slice(IMG * i, IMG * (i + 1))
        in_maps.append({
            "anch": anch_h,
            "bbox": np.ascontiguousarray(
                bbox_pred[bsl].reshape(IMG, P, COLS * 4), dtype=np.float32),
            "conf": np.ascontiguousarray(
                conf_pred[bsl].reshape(IMG, P, COLS), dtype=np.float32),
            "gtb": np.ascontiguousarray(
                gt_boxes[bsl].reshape(IMG, 1, G * 4), dtype=np.float32),
        })
    return in_maps


def kernel(bbox_pred, conf_pred, anchors, gt_boxes):
    nc = _get_nc()
    in_maps = _make_in_maps(dict(bbox_pred=bbox_pred, conf_pred=conf_pred,
                                 anchors=anchors, gt_boxes=gt_boxes))
    out = run_bass_kernel_spmd(nc, in_maps, core_ids=list(range(NCORE)))

    loc_total = np.float32(0.0)
    conf_total = np.float32(0.0)
    npos_total = np.float32(0.0)
    for i in range(NCORE):
        res = out.results[i]["res"]  # [IMG, 1, 8]
        for b in range(IMG):
            npos, locsum, possum, cnt_gt, sum_gt, tau_hi, maxv, kdev = \
                [np.float32(x) for x in res[b, 0, :8]]
            k = np.float32(min(NEG_POS_RATIO * npos, A - npos))
            wl_last = np.float32(maxv / NBIN ** NLEV)
            rem = max(np.float32(0.0), np.float32(k - cnt_gt))
            neg = np.float32(sum_gt + rem * (tau_hi - wl_last * np.float32(0.5)))
            loc_total = np.float32(loc_total + locsum)
            conf_total = np.float32(conf_total + possum + neg)
            npos_total = np.float32(npos_total + npos)
    num_pos = np.float32(max(1.0, npos_total))
    loc_loss = np.float32(loc_total / num_pos)
    conf_loss = np.float32(conf_total / num_pos)
    return (np.float32(loc_loss + conf_loss), conf_loss, loc_loss)


# revision 11
# speedup vs baseline: 1.5284x; 1.5284x over previous
"""Trainium2 Bass kernel v2 for nn_DetectionLoss — fp16 grid pipeline.

Data-parallel: 16 images over 8 cores (2 images/core). Per image, the
[A=65536, G=32] match grid is computed in fp16 (coords pre-scaled by 1/64 —
the loss is scale-invariant) in supertile-(g,u) layout so every DVE op is
packed-innermost and runs in 2x/4x mode. Division uses the Act engine's
table Reciprocal (~0.5% rel err — only feeds match *selection*, fine at the
2e-2 gate). Matched-gt coords go through fp16 PE transposes of the one-hot
grid + 16-wide block-diag matmuls. DIoU runs in fp16 on coord planes; focal
+ hard-negative mining keep the baseline f32/fp16 structure. Host combines
per-image scalars exactly like the reference."""
import sys

sys.path.insert(0, '/opt/trn_rl_repo')

import numpy as np
import concourse.bass as bass
import concourse.bacc as bacc
import concourse.mybir as mybir
from concourse.tile import TileContext
from concourse.bass_utils import run_bass_kernel_spmd
from concourse.masks import make_identity
from contextlib import ExitStack

Alu = mybir.AluOpType
Act = mybir.ActivationFunctionType
Ax = mybir.AxisListType
F32 = mybir.dt.float32
FP16 = mybir.dt.float16

P = 128
A = 65536
G = 32
IMG = 2
NCORE = 8
COLS = A // P       # 512
U = 32
W = G * U           # 1024 els per supertile block
NSUP = COLS // U    # 16
NQ = 4
QSUP = NSUP // NQ   # 4
QW = QSUP * W       # 4096
SC = 1.0 / 64.0
POS_THR = 1.0 / 3.0
NBIN = 16
NLEV = 2
NEG_POS_RATIO = 3.0


def _act_recip(nc, out, in_):
    """Raw Act-engine Reciprocal (table approx, ~0.5% rel err)."""
    sc = nc.scalar
    ins = [sc.lower_ap(in_)]
    for argv in (0.0, 1.0, 0.0):
        ins.append(mybir.ImmediateValue(dtype=mybir.dt.float32, value=argv))
    return sc.add_instruction(
        mybir.InstActivation(name=nc.get_next_instruction_name(),
                             func=Act.Reciprocal, ins=ins,
                             outs=[sc.lower_ap(out)]))


def _build_nc():
    nc = bacc.Bacc("TRN2", target_bir_lowering=False, debug=False)
    anch_d = nc.dram_tensor("anch", [P, COLS * 4], F32, kind="ExternalInput")
    bbox_d = nc.dram_tensor("bbox", [IMG, P, COLS * 4], F32, kind="ExternalInput")
    conf_d = nc.dram_tensor("conf", [IMG, P, COLS], F32, kind="ExternalInput")
    gt_d = nc.dram_tensor("gtb", [IMG, 1, G * 4], F32, kind="ExternalInput")
    res_d = nc.dram_tensor("res", [IMG, 1, 8], F32, kind="ExternalOutput")

    v = nc.vector
    sc = nc.scalar
    pe = nc.tensor

    with TileContext(nc) as tc, ExitStack() as ctx, \
            nc.allow_low_precision(reason="fp16 grid; host checks rel err"):
        pool = ctx.enter_context(tc.tile_pool(name="main", bufs=1))
        pspool = ctx.enter_context(tc.tile_pool(name="ps", bufs=1, space="PSUM"))

        def T(name, cols, parts=P, dt=F32):
            return pool.tile([parts, cols], dt, name=name)

        def T16(name, cols, parts=P):
            return pool.tile([parts, cols], FP16, name=name)

        # ---------------- per-core constants ----------------
        anch_sb = T("anch_sb", COLS * 4)
        nc.sync.dma_start(anch_sb[:, 0:COLS * 2], anch_d[:, 0:COLS * 2])
        nc.sync.dma_start(anch_sb[:, COLS * 2:], anch_d[:, COLS * 2:])
        anch3 = anch_sb[:].rearrange("p (n c) -> p n c", c=4)

        ax2h = T16("ax2h", COLS)
        ay2h = T16("ay2h", COLS)
        nax1h = T16("nax1h", COLS)
        nay1h = T16("nay1h", COLS)
        areaAh = T16("areaAh", COLS)
        f0 = T("f0", COLS)
        f1 = T("f1", COLS)
        sc.activation(ax2h[:], anch3[:, :, 2:3].squeeze(2), Act.Copy, scale=SC)
        sc.activation(ay2h[:], anch3[:, :, 3:4].squeeze(2), Act.Copy, scale=SC)
        sc.activation(nax1h[:], anch3[:, :, 0:1].squeeze(2), Act.Copy, scale=-SC)
        sc.activation(nay1h[:], anch3[:, :, 1:2].squeeze(2), Act.Copy, scale=-SC)
        v.tensor_tensor(out=f0[:], in0=anch3[:, :, 2:3].squeeze(2),
                        in1=anch3[:, :, 0:1].squeeze(2), op=Alu.subtract)
        v.tensor_tensor(out=f1[:], in0=anch3[:, :, 3:4].squeeze(2),
                        in1=anch3[:, :, 1:2].squeeze(2), op=Alu.subtract)
        v.tensor_tensor(out=f0[:], in0=f0[:], in1=f1[:], op=Alu.mult)
        sc.activation(areaAh[:], f0[:], Act.Copy, scale=SC * SC)

        ident = T16("ident", P)
        make_identity(nc, ident[:])
        identf = T("identf", P)
        make_identity(nc, identf[:])
        ones_col = T("ones_col", 1)
        ones_row = T("ones_row", P, parts=1)
        ones_row16 = T16("ones_row16", P, parts=1)
        v.memset(ones_col[:], 1.0)
        v.memset(ones_row[:], 1.0)
        v.memset(ones_row16[:], 1.0)
        iota_i = pool.tile([P, NBIN], mybir.dt.int32, name="iota_i")
        nc.gpsimd.iota(iota_i[:], pattern=[[1, NBIN]], base=0, channel_multiplier=0)
        iota_f = T("iota_f", NBIN)
        v.tensor_copy(iota_f[:], iota_i[:])

        def pbcast(dst, src_row):
            n = src_row.shape[-1]
            bc_ps = pspool.tile([P, G], F32, name="bc_ps", tag="pss")
            nc.tensor.matmul(bc_ps[:, 0:n], ones_row[:], src_row)
            v.tensor_copy(dst, bc_ps[:, 0:n])

        # ---------------- shared big tiles ----------------
        grid = T16("grid", NSUP * W)     # r values, (s, g, u) blocks
        tgrid = T16("tgrid", NSUP * W)   # one-hot, (s, u, g) blocks
        sA_l = [T16(f"sA{k}", QW) for k in range(2)]
        sB_l = [T16(f"sB{k}", QW) for k in range(2)]
        sCt_l = [T16("sCt0", QW)] * 2
        rm = T16("rm", COLS)
        colT = T16("colT", NSUP * G)
        cpa = T16("cpa", G)
        forced = T16("forced", COLS)
        pos16 = T16("pos16", COLS)
        posf = T("posf", COLS)
        cmb = T16("cmb", G)
        cmaxpl = T16("cmaxpl", W)

        gtrow_l = [T(f"gtrow{b}", G * 4, parts=1) for b in range(IMG)]
        gtall_l = [T(f"gtall{b}", G * 4) for b in range(IMG)]
        gsc = [T(f"gsc{c}", G) for c in range(4)]
        sGf = T("sGf", G)
        tG = T("tG", G)
        ngx1p = T16("ngx1p", W)
        ngy1p = T16("ngy1p", W)
        gx2p = T16("gx2p", W)
        gy2p = T16("gy2p", W)
        sGp = T16("sGp", W)
        gtmatf_l = [T(f"gtmatf{b}", 16) for b in range(IMG)]
        gtmath_l = [T16(f"gtmath{b}", 16) for b in range(IMG)]

        tsb = T16("tsb", W)
        matched = T16("matched", 4 * COLS)   # (s, u, c)
        mch = [T16(f"mch{c}", COLS) for c in range(4)]
        bxh = [T16(f"bxh{c}", COLS) for c in range(4)]
        areaPh = T16("areaPh", COLS)
        bbox_sb_l = [T(f"bbox_sb{b}", COLS * 4) for b in range(IMG)]
        conf_sb_l = [T(f"conf_sb{b}", COLS) for b in range(IMG)]

        d0 = T16("d0", COLS)
        d1 = T16("d1", COLS)
        d2 = T16("d2", COLS)
        d3 = T16("d3", COLS)
        d4 = T16("d4", COLS)

        s0 = T("s0", COLS)
        s1 = T("s1", COLS)
        s2 = T("s2", COLS)
        s3 = T("s3", COLS)
        s4 = T("s4", COLS)
        s5 = T("s5", COLS)
        cl = T("cl", COLS)
        nv = T("nv", COLS)
        nv16 = T16("nv16", COLS)
        sink16 = T16("sink16", COLS)

        npp = T("npp", 1)
        locsum_pp = T("locsum_pp", 1)
        possum_pp = T("possum_pp", 1)
        cnt_pp = T("cnt_pp", 1)
        sum_pp = T("sum_pp", 1)
        maxv_pp = T("maxv_pp", 1)
        maxvb = T("maxvb", 1)
        w1c = T("w1c", 1)
        tau_b = T("tau_b", 1)
        stack = T("stack", 4)
        thr = T("thr", NBIN)
        nthr = T("nthr", NBIN)
        cge = T("cge", NBIN)
        wl = [T(f"wl{l}", 1) for l in range(NLEV)]
        lo_b = [T(f"lo_b{l}", 1) for l in range(NLEV)]
        cget = T("cget", NBIN, parts=1)
        gek = T("gek", NBIN, parts=1)
        scnt = T("scnt", 1, parts=1)
        lo_new = T("lo_new", 1, parts=1)
        tau = [T(f"tau{l}", 1, parts=1) for l in range(NLEV)]
        maxv1 = T("maxv1", 1, parts=1)
        npos1 = T("npos1", 1, parts=1)
        k1 = T("k1", 1, parts=1)
        k2 = T("k2", 1, parts=1)
        kk = T("kk", 1, parts=1)
        mx_row = T("mx_row", P, parts=1)
        ctt = T16("ctt", P, parts=G)
        cmax_col = T16("cmax_col", 1, parts=G)
        cm_row = T16("cm_row", G, parts=1)
        res_sb = T("res_sb", 8, parts=1)

        # 4D view helpers
        def q4gu(t, q):  # (s, g, u) packed quarter
            return t[:, q*QW:(q+1)*QW].rearrange("p (s g u) -> p s g u", g=G, u=U)

        def aview(t, q):  # anchor [P, COLS] -> [p, s, g(b), u]
            return (t[:, q*QSUP*U:(q+1)*QSUP*U]
                    .rearrange("p (s u) -> p s u", u=U)
                    .unsqueeze(2).to_broadcast([P, QSUP, G, U]))

        def gview(t):     # gt plane [P, W] -> [p, s(b), g, u]
            return (t[:].rearrange("p (g u) -> p g u", u=U)
                    .unsqueeze(1).to_broadcast([P, QSUP, G, U]))

        for b in range(IMG):
            bbox_sb = bbox_sb_l[b]
            conf_sb = conf_sb_l[b]
            gtrow = gtrow_l[b]
            gtall = gtall_l[b]
            gtmatf = gtmatf_l[b]
            gtmath = gtmath_l[b]
            # ---------------- loads ----------------
            nc.sync.dma_start(bbox_sb[:, 0:COLS * 2], bbox_d[b][:, 0:COLS * 2])
            nc.sync.dma_start(bbox_sb[:, COLS * 2:], bbox_d[b][:, COLS * 2:])
            nc.sync.dma_start(conf_sb[:], conf_d[b])
            nc.scalar.dma_start(gtrow[:], gt_d[b])
            gt_ps = pspool.tile([P, G * 4], F32, name="gt_ps", tag="gtp")
            nc.tensor.matmul(gt_ps[:], ones_row[:], gtrow[:])
            v.tensor_copy(gtall[:], gt_ps[:])
            gt3 = gtall[:].rearrange("p (g c) -> p g c", c=4)
            gt2d = gt_d[b].rearrange("q (g c) -> (q g) c", c=4)
            v.memset(gtmatf[:], 0.0)
            for uu in range(4):
                nc.scalar.dma_start(gtmatf[uu*G:(uu+1)*G, uu*4:(uu+1)*4], gt2d)
            sc.activation(gtmath[:], gtmatf[:], Act.Copy, scale=SC)

            # ---------------- gt prep ----------------
            v.tensor_scalar(gsc[0][:], gt3[:, :, 0:1].squeeze(2), -SC, None, Alu.mult)
            v.tensor_scalar(gsc[1][:], gt3[:, :, 1:2].squeeze(2), -SC, None, Alu.mult)
            v.tensor_scalar(gsc[2][:], gt3[:, :, 2:3].squeeze(2), SC, None, Alu.mult)
            v.tensor_scalar(gsc[3][:], gt3[:, :, 3:4].squeeze(2), SC, None, Alu.mult)
            v.tensor_tensor(out=sGf[:], in0=gt3[:, :, 2:3].squeeze(2),
                            in1=gt3[:, :, 0:1].squeeze(2), op=Alu.subtract)
            v.tensor_tensor(out=tG[:], in0=gt3[:, :, 3:4].squeeze(2),
                            in1=gt3[:, :, 1:2].squeeze(2), op=Alu.subtract)
            v.tensor_tensor(out=sGf[:], in0=sGf[:], in1=tG[:], op=Alu.mult)
            v.tensor_scalar(sGf[:], sGf[:], SC * SC, None, Alu.mult)
            for pl, src in ((ngx1p, gsc[0]), (ngy1p, gsc[1]), (gx2p, gsc[2]),
                            (gy2p, gsc[3]), (sGp, sGf)):
                sc.activation(pl[:].rearrange("p (g u) -> p g u", u=U),
                              src[:].unsqueeze(2).to_broadcast([P, G, U]), Act.Copy)

            # ---------------- pass 1: grid ----------------
            for q in range(NQ):
                sA = sA_l[q % 2]
                sB = sB_l[q % 2]
                sCt = sCt_l[q % 2]
                gq = q4gu(grid, q)
                a4 = q4gu(sA, 0)
                b4 = q4gu(sB, 0)
                c4 = q4gu(sCt, 0)
                v.tensor_tensor(out=a4, in0=aview(nax1h, q), in1=gview(ngx1p), op=Alu.min)
                v.tensor_tensor(out=b4, in0=aview(ax2h, q), in1=gview(gx2p), op=Alu.min)
                v.tensor_tensor(out=a4, in0=b4, in1=a4, op=Alu.add)
                v.tensor_tensor(out=b4, in0=aview(nay1h, q), in1=gview(ngy1p), op=Alu.min)
                v.tensor_tensor(out=c4, in0=aview(ay2h, q), in1=gview(gy2p), op=Alu.min)
                v.tensor_tensor(out=b4, in0=c4, in1=b4, op=Alu.add)
                v.tensor_scalar(sA[:], sA[:], 0.0, None, Alu.max)
                v.tensor_scalar(sB[:], sB[:], 0.0, None, Alu.max)
                v.tensor_tensor(out=a4, in0=a4, in1=b4, op=Alu.mult)      # inter
                HW2 = QW // 2
                for hh in range(2):
                    s0q = q * QSUP + hh * (QSUP // 2)
                    hsl = slice(hh * HW2, (hh + 1) * HW2)
                    bh = (sB_l[q % 2][:, hsl]
                          .rearrange("p (s g u) -> p s g u", g=G, u=U))
                    ah = (sA_l[q % 2][:, hsl]
                          .rearrange("p (s g u) -> p s g u", g=G, u=U))
                    gh = (grid[:, q*QW + hh*HW2: q*QW + (hh+1)*HW2]
                          .rearrange("p (s g u) -> p s g u", g=G, u=U))
                    avh = (areaAh[:, s0q*U:(s0q + QSUP//2)*U]
                           .rearrange("p (s u) -> p s u", u=U)
                           .unsqueeze(2).to_broadcast([P, QSUP//2, G, U]))
                    gvh = (sGp[:].rearrange("p (g u) -> p g u", u=U)
                           .unsqueeze(1).to_broadcast([P, QSUP//2, G, U]))
                    v.tensor_tensor(out=bh, in0=avh, in1=gvh, op=Alu.add)
                    _act_recip(nc, sB_l[q % 2][:, hsl], sB_l[q % 2][:, hsl])
                    v.tensor_tensor(out=gh, in0=ah, in1=bh, op=Alu.mult)

                cur = gq
                width = G
                while width > 1:
                    h = width // 2
                    if h == 1:
                        dst = (rm[:, q*QSUP*U:(q+1)*QSUP*U]
                               .rearrange("p (s u) -> p s u", u=U).unsqueeze(2))
                    else:
                        dst = q4gu(sCt, 0)[:, :, 0:h, :]
                    v.tensor_tensor(out=dst, in0=cur[:, :, 0:h, :],
                                    in1=cur[:, :, h:width, :], op=Alu.max)
                    cur = dst
                    width = h
                curc = gq
                widc = U
                while widc > 1:
                    hc = widc // 2
                    if hc == 1:
                        dstc = (colT[:, q*QSUP*G:(q+1)*QSUP*G]
                                .rearrange("p (s g) -> p s g", g=G).unsqueeze(3))
                    else:
                        dstc = q4gu(sB, 0)[:, :, :, 0:hc]
                    v.tensor_tensor(out=dstc, in0=curc[:, :, :, 0:hc],
                                    in1=curc[:, :, :, hc:widc], op=Alu.max)
                    curc = dstc
                    widc = hc

            # ---------------- cmax finalize ----------------
            v.tensor_reduce(out=cpa[:], in_=colT[:].rearrange("p (s g) -> p g s", g=G),
                            axis=Ax.X, op=Alu.max)
            ct_ps = pspool.tile([G, P], FP16, name="ct_ps", tag="pst")
            pe.transpose(ct_ps[:], cpa[:], ident[:])
            v.tensor_copy(ctt[:], ct_ps[:])
            v.tensor_reduce(out=cmax_col[:], in_=ctt[:], axis=Ax.X, op=Alu.max)
            cm_ps = pspool.tile([1, G], FP16, name="cm_ps", tag="pst")
            pe.transpose(cm_ps[:], cmax_col[:], ident[:G, :G])
            v.tensor_copy(cm_row[:], cm_ps[:])
            bc_ps = pspool.tile([P, G], F32, name="bc_ps", tag="pss")
            nc.tensor.matmul(bc_ps[:], ones_row16[:], cm_row[:])
            v.tensor_copy(cmb[:], bc_ps[:])
            sc.activation(cmaxpl[:].rearrange("p (g u) -> p g u", u=U),
                          cmb[:].unsqueeze(2).to_broadcast([P, G, U]), Act.Copy)

            # ---------------- forced + pos ----------------
            for q in range(NQ):
                sA = sA_l[q % 2]
                gq = q4gu(grid, q)
                v.tensor_tensor(out=q4gu(sA, 0), in0=gq, in1=gview(cmaxpl),
                                op=Alu.is_equal)
                cur = q4gu(sA, 0)
                width = G
                while width > 1:
                    h = width // 2
                    if h == 1:
                        dst = (forced[:, q*QSUP*U:(q+1)*QSUP*U]
                               .rearrange("p (s u) -> p s u", u=U).unsqueeze(2))
                    else:
                        dst = cur[:, :, 0:h, :]
                    v.tensor_tensor(out=dst, in0=cur[:, :, 0:h, :],
                                    in1=cur[:, :, h:width, :], op=Alu.max)
                    cur = dst
                    width = h
            v.tensor_scalar(pos16[:], rm[:], POS_THR, None, Alu.is_gt)
            v.tensor_tensor(out=pos16[:], in0=pos16[:], in1=forced[:], op=Alu.max)
            v.tensor_scalar(sink16[:], pos16[:], 1.0, 0.0, Alu.mult, Alu.add,
                            accum_out=npp[:])
            v.tensor_copy(posf[:], pos16[:])

            # ---------------- ismax -> tgrid (s, u, g) ----------------
            for q in range(NQ):
                gin = grid[:, q*QW:(q+1)*QW].rearrange(
                    "p (s g u) -> p s u g", g=G, u=U)
                rmv = (rm[:, q*QSUP*U:(q+1)*QSUP*U]
                       .rearrange("p (s u) -> p s u", u=U)
                       .unsqueeze(3).to_broadcast([P, QSUP, U, G]))
                tout = tgrid[:, q*QW:(q+1)*QW].rearrange(
                    "p (s u g) -> p s u g", u=U, g=G)
                v.tensor_tensor(out=tout, in0=gin, in1=rmv, op=Alu.is_equal)

            # ---------------- matched coords (PE) ----------------
            for s in range(NSUP):
                mout = pspool.tile([P, P], F32, name=f"mo{s % 2}", tag=f"mo{s % 2}")
                for j in range(8):
                    tp = pspool.tile([P, P], FP16, name=f"tp{j % 2}", tag=f"tp{j % 2}")
                    pe.transpose(tp[:], tgrid[:, s*W + j*P: s*W + (j+1)*P], ident[:])
                    sc.copy(tsb[:, j*P:(j+1)*P], tp[:])
                    nc.tensor.matmul(mout[:, j*16:(j+1)*16], tsb[:, j*P:(j+1)*P],
                                     gtmath[:])
                sc.copy(matched[:, s*P:(s+1)*P], mout[:])

            m4 = matched[:].rearrange("p (n c) -> p n c", c=4)
            for c in range(4):
                sc.copy(mch[c][:], m4[:, :, c:c+1].squeeze(2))

            # ---------------- bbox planes + diou ----------------
            bb3 = bbox_sb[:].rearrange("p (n c) -> p n c", c=4)
            for c in range(4):
                sc.activation(bxh[c][:], bb3[:, :, c:c+1].squeeze(2), Act.Copy,
                              scale=SC)
            v.tensor_tensor(out=d0[:], in0=bxh[2][:], in1=bxh[0][:], op=Alu.subtract)
            v.tensor_tensor(out=d1[:], in0=bxh[3][:], in1=bxh[1][:], op=Alu.subtract)
            v.tensor_tensor(out=areaPh[:], in0=d0[:], in1=d1[:], op=Alu.mult)

            # inter
            v.tensor_tensor(out=d0[:], in0=bxh[0][:], in1=mch[0][:], op=Alu.max)
            v.tensor_tensor(out=d1[:], in0=bxh[2][:], in1=mch[2][:], op=Alu.min)
            v.tensor_tensor(out=d0[:], in0=d1[:], in1=d0[:], op=Alu.subtract)
            v.tensor_scalar(d0[:], d0[:], 0.0, None, Alu.max)
            v.tensor_tensor(out=d1[:], in0=bxh[1][:], in1=mch[1][:], op=Alu.max)
            v.tensor_tensor(out=d2[:], in0=bxh[3][:], in1=mch[3][:], op=Alu.min)
            v.tensor_tensor(out=d1[:], in0=d2[:], in1=d1[:], op=Alu.subtract)
            v.tensor_scalar(d1[:], d1[:], 0.0, None, Alu.max)
            v.tensor_tensor(out=d0[:], in0=d0[:], in1=d1[:], op=Alu.mult)  # inter
            # matched area
            v.tensor_tensor(out=d1[:], in0=mch[2][:], in1=mch[0][:], op=Alu.subtract)
            v.tensor_tensor(out=d2[:], in0=mch[3][:], in1=mch[1][:], op=Alu.subtract)
            v.tensor_tensor(out=d1[:], in0=d1[:], in1=d2[:], op=Alu.mult)
            # union, iou
            v.tensor_tensor(out=d1[:], in0=d1[:], in1=areaPh[:], op=Alu.add)
            v.tensor_tensor(out=d1[:], in0=d1[:], in1=d0[:], op=Alu.subtract)
            _act_recip(nc, d1[:], d1[:])
            v.tensor_tensor(out=d0[:], in0=d0[:], in1=d1[:], op=Alu.mult)  # iou
            # enclosing c2
            v.tensor_tensor(out=d1[:], in0=bxh[0][:], in1=mch[0][:], op=Alu.min)
            v.tensor_tensor(out=d2[:], in0=bxh[2][:], in1=mch[2][:], op=Alu.max)
            v.tensor_tensor(out=d1[:], in0=d2[:], in1=d1[:], op=Alu.subtract)
            v.tensor_tensor(out=d1[:], in0=d1[:], in1=d1[:], op=Alu.mult)
            v.tensor_tensor(out=d2[:], in0=bxh[1][:], in1=mch[1][:], op=Alu.min)
            v.tensor_tensor(out=d3[:], in0=bxh[3][:], in1=mch[3][:], op=Alu.max)
            v.tensor_tensor(out=d2[:], in0=d3[:], in1=d2[:], op=Alu.subtract)
            v.tensor_tensor(out=d2[:], in0=d2[:], in1=d2[:], op=Alu.mult)
            v.tensor_tensor(out=d1[:], in0=d1[:], in1=d2[:], op=Alu.add)   # c2
            _act_recip(nc, d1[:], d1[:])
            # center dist d2 (quarter-scaled: absorbed by c2 ratio using same /2)
            v.tensor_tensor(out=d2[:], in0=bxh[0][:], in1=bxh[2][:], op=Alu.add)
            v.tensor_tensor(out=d3[:], in0=mch[0][:], in1=mch[2][:], op=Alu.add)
            v.tensor_tensor(out=d2[:], in0=d2[:], in1=d3[:], op=Alu.subtract)
            v.tensor_tensor(out=d2[:], in0=d2[:], in1=d2[:], op=Alu.mult)
            v.tensor_tensor(out=d3[:], in0=bxh[1][:], in1=bxh[3][:], op=Alu.add)
            v.tensor_tensor(out=d4[:], in0=mch[1][:], in1=mch[3][:], op=Alu.add)
            v.tensor_tensor(out=d3[:], in0=d3[:], in1=d4[:], op=Alu.subtract)
            v.tensor_tensor(out=d3[:], in0=d3[:], in1=d3[:], op=Alu.mult)
            v.tensor_tensor(out=d2[:], in0=d2[:], in1=d3[:], op=Alu.add)   # 4*d2
            v.tensor_tensor(out=d2[:], in0=d2[:], in1=d1[:], op=Alu.mult)
            v.tensor_scalar(d2[:], d2[:], 0.25, None, Alu.mult)            # d2/c2
            v.tensor_scalar(d0[:], d0[:], -1.0, 1.0, Alu.mult, Alu.add)    # 1-iou
            v.tensor_tensor(out=d2[:], in0=d2[:], in1=d0[:], op=Alu.add)
            v.tensor_scalar(d2[:], d2[:], 100.0, None, Alu.min)
            v.tensor_tensor(out=d2[:], in0=d2[:], in1=pos16[:], op=Alu.mult)
            v.tensor_scalar(sink16[:], d2[:], 1.0, 0.0, Alu.mult, Alu.add,
                            accum_out=locsum_pp[:])

            # ---------------- focal conf loss (f32, baseline) ----------------
            sc.activation(s0[:], conf_sb[:], Act.Sigmoid)
            sc.activation(s1[:], conf_sb[:], Act.Exp)
            sc.activation(s1[:], s1[:], Act.Ln, bias=1.0)
            v.tensor_tensor(out=s2[:], in0=conf_sb[:], in1=posf[:], op=Alu.mult)
            v.tensor_tensor(out=s2[:], in0=s1[:], in1=s2[:], op=Alu.subtract)
            v.tensor_scalar(s3[:], posf[:], -2.0, 1.0, Alu.mult, Alu.add)
            v.tensor_tensor(out=s3[:], in0=s0[:], in1=s3[:], op=Alu.mult)
            v.tensor_tensor(out=s3[:], in0=s3[:], in1=posf[:], op=Alu.add)
            sc.activation(s3[:], s3[:], Act.Square)
            v.tensor_tensor(out=cl[:], in0=s3[:], in1=s2[:], op=Alu.mult)
            v.tensor_scalar(s3[:], posf[:], -0.5, 0.75, Alu.mult, Alu.add)
            v.tensor_tensor(out=cl[:], in0=cl[:], in1=s3[:], op=Alu.mult)
            v.tensor_scalar(cl[:], cl[:], 100.0, None, Alu.min)
            v.tensor_tensor(out=s4[:], in0=cl[:], in1=posf[:], op=Alu.mult)
            v.tensor_scalar(s5[:], s4[:], 1.0, 0.0, Alu.mult, Alu.add,
                            accum_out=possum_pp[:])
            v.tensor_tensor(out=nv[:], in0=cl[:], in1=s4[:], op=Alu.subtract)
            v.tensor_copy(nv16[:], nv[:])

            # ---------------- hard negative mining (baseline) ----------------
            v.tensor_reduce(out=maxv_pp[:], in_=nv[:], axis=Ax.X, op=Alu.max)
            mx_ps = pspool.tile([1, P], F32, name="mx_ps", tag="pss")
            pe.transpose(mx_ps[:], maxv_pp[:], identf[:])
            v.tensor_copy(mx_row[:], mx_ps[:])
            v.tensor_reduce(out=maxv1[:], in_=mx_row[:], axis=Ax.X, op=Alu.max)

            np_ps = pspool.tile([1, 1], F32, name="np_ps", tag="pss")
            nc.tensor.matmul(np_ps[:], ones_col[:], npp[:])
            v.tensor_copy(npos1[:], np_ps[:])
            v.tensor_scalar(k1[:], npos1[:], NEG_POS_RATIO, None, Alu.mult)
            v.tensor_scalar(k2[:], npos1[:], -1.0, float(A), Alu.mult, Alu.add)
            v.tensor_tensor(out=kk[:], in0=k1[:], in1=k2[:], op=Alu.min)

            pbcast(maxvb[:], maxv1[:])
            v.tensor_scalar(w1c[:], maxvb[:], 1.0 / NBIN, None, Alu.mult)

            for lev in range(NLEV):
                if lev == 0:
                    v.tensor_copy(wl[0][:], w1c[:])
                    v.tensor_scalar(thr[:], iota_f[:], wl[0][:], None, Alu.mult)
                else:
                    v.tensor_scalar(wl[lev][:], wl[lev - 1][:], 1.0 / NBIN, None,
                                    Alu.mult)
                    v.tensor_scalar(thr[:], iota_f[:], wl[lev][:], lo_b[lev - 1][:],
                                    Alu.mult, Alu.add)
                v.tensor_scalar(nthr[:], thr[:], -1.0, None, Alu.mult)
                for bn in range(NBIN):
                    sc.activation(sink16[:], nv16[:], Act.Sign,
                                  bias=nthr[:, bn:bn+1], accum_out=cge[:, bn:bn+1])
                cg_ps = pspool.tile([1, NBIN], F32, name="cg_ps", tag="pss")
                nc.tensor.matmul(cg_ps[:], ones_col[:], cge[:])
                v.tensor_copy(cget[:], cg_ps[:])
                # sign-sum -> count_gt: cnt = (acc + A)/2 (ties ~0, absorbed)
                v.tensor_scalar(cget[:], cget[:], 0.5, float(A) * 0.5, Alu.mult,
                                Alu.add)
                v.tensor_scalar(gek[:], cget[:], kk[:], None, Alu.is_ge)
                v.tensor_reduce(out=scnt[:], in_=gek[:], axis=Ax.X, op=Alu.add)
                v.tensor_scalar(lo_new[:], scnt[:], 1.0, wl[lev][0:1, :],
                                Alu.subtract, Alu.mult)
                v.tensor_scalar(tau[lev][:], scnt[:], wl[lev][0:1, :], None, Alu.mult)
                if lev > 0:
                    v.tensor_tensor(out=lo_new[:], in0=lo_new[:],
                                    in1=lo_b[lev - 1][0:1, :], op=Alu.add)
                    v.tensor_tensor(out=tau[lev][:], in0=tau[lev][:],
                                    in1=lo_b[lev - 1][0:1, :], op=Alu.add)
                pbcast(lo_b[lev][:], lo_new[:])

            pbcast(tau_b[:], tau[NLEV - 1][:])
            v.tensor_scalar(s4[:], nv[:], tau_b[:], 0.0, Alu.is_gt,
                            Alu.add, accum_out=cnt_pp[:])
            v.tensor_tensor(out=s5[:], in0=nv[:], in1=s4[:], op=Alu.mult)
            v.tensor_scalar(s5[:], s5[:], 1.0, 0.0, Alu.mult, Alu.add,
                            accum_out=sum_pp[:])

            # ---------------- gather scalars ----------------
            v.tensor_copy(stack[:, 0:1], npp[:])
            v.tensor_copy(stack[:, 1:2], locsum_pp[:])
            v.tensor_copy(stack[:, 2:3], possum_pp[:])
            v.tensor_copy(stack[:, 3:4], cnt_pp[:])
            st_ps = pspool.tile([1, 4], F32, name="st_ps", tag="pss")
            nc.tensor.matmul(st_ps[:], ones_col[:], stack[:])
            sm_ps = pspool.tile([1, 1], F32, name="sm_ps", tag="pss")
            nc.tensor.matmul(sm_ps[:], ones_col[:], sum_pp[:])

            v.tensor_copy(res_sb[:, 0:4], st_ps[:])
            v.tensor_copy(res_sb[:, 4:5], sm_ps[:])
            v.tensor_copy(res_sb[:, 5:6], tau[NLEV - 1][:])
            v.tensor_copy(res_sb[:, 6:7], maxv1[:])
            v.tensor_copy(res_sb[:, 7:8], kk[:])
            nc.sync.dma_start(res_d[b], res_sb[:])

    nc.compile()
    return nc


_NC_CACHE = None


def _get_nc():
    global _NC_CACHE
    if _NC_CACHE is None:
        _NC_CACHE = _build_nc()
    return _NC_CACHE


def _make_in_maps(inputs):
    bbox_pred = np.asarray(inputs["bbox_pred"])
    conf_pred = np.asarray(inputs["conf_pred"])
    anchors = np.asarray(inputs["anchors"])
    gt_boxes = np.asarray(inputs["gt_boxes"])
    anch_h = np.ascontiguousarray(anchors.reshape(P, COLS * 4), dtype=np.float32)
    in_maps = []
    for i in range(NCORE):
        bsl = slice(IMG * i, IMG * (i + 1))
        in_maps.append({
            "anch": anch_h,
            "bbox": np.ascontiguousarray(
                bbox_pred[bsl].reshape(IMG, P, COLS * 4), dtype=np.float32),
            "conf": np.ascontiguousarray(
                conf_pred[bsl].reshape(IMG, P, COLS), dtype=np.float32),
            "gtb": np.ascontiguousarray(
                gt_boxes[bsl].reshape(IMG, 1, G * 4), dtype=np.float32),
        })
    return in_maps


def kernel(bbox_pred, conf_pred, anchors, gt_boxes):
    nc = _get_nc()
    in_maps = _make_in_maps(dict(bbox_pred=bbox_pred, conf_pred=conf_pred,
                                 anchors=anchors, gt_boxes=gt_boxes))
    out = run_bass_kernel_spmd(nc, in_maps, core_ids=list(range(NCORE)))

    loc_total = np.float32(0.0)
    conf_total = np.float32(0.0)
    npos_total = np.float32(0.0)
    for i in range(NCORE):
        res = out.results[i]["res"]  # [IMG, 1, 8]
        for b in range(IMG):
            npos, locsum, possum, cnt_gt, sum_gt, tau_hi, maxv, kdev = \
                [np.float32(x) for x in res[b, 0, :8]]
            k = np.float32(min(NEG_POS_RATIO * npos, A - npos))
            wl_last = np.float32(maxv / NBIN ** NLEV)
            rem = max(np.float32(0.0), np.float32(k - cnt_gt))
            neg = np.float32(sum_gt + rem * (tau_hi - wl_last * np.float32(0.5)))
            loc_total = np.float32(loc_total + locsum)
            conf_total = np.float32(conf_total + possum + neg)
            npos_total = np.float32(npos_total + npos)
    num_pos = np.float32(max(1.0, npos_total))
    loc_loss = np.float32(loc_total / num_pos)
    conf_loss = np.float32(conf_total / num_pos)
    return (np.float32(loc_loss + conf_loss), conf_loss, loc_loss)


# revision 12
# speedup vs baseline: 1.5368x; 1.0054x over previous
"""Trainium2 Bass kernel v2 for nn_DetectionLoss — fp16 grid pipeline.

Data-parallel: 16 images over 8 cores (2 images/core). Per image, the
[A=65536, G=32] match grid is computed in fp16 (coords pre-scaled by 1/64 —
the loss is scale-invariant) in supertile-(g,u) layout so every DVE op is
packed-innermost and runs in 2x/4x mode. Division uses the Act engine's
table Reciprocal (~0.5% rel err — only feeds match *selection*, fine at the
2e-2 gate). Matched-gt coords go through fp16 PE transposes of the one-hot
grid + 16-wide block-diag matmuls. DIoU runs in fp16 on coord planes; focal
+ hard-negative mining keep the baseline f32/fp16 structure. Host combines
per-image scalars exactly like the reference."""
import sys

sys.path.insert(0, '/opt/trn_rl_repo')

import numpy as np
import concourse.bass as bass
import concourse.bacc as bacc
import concourse.mybir as mybir
from concourse.tile import TileContext
from concourse.bass_utils import run_bass_kernel_spmd
from concourse.masks import make_identity
from contextlib import ExitStack

Alu = mybir.AluOpType
Act = mybir.ActivationFunctionType
Ax = mybir.AxisListType
F32 = mybir.dt.float32
FP16 = mybir.dt.float16

P = 128
A = 65536
G = 32
IMG = 2
NCORE = 8
COLS = A // P       # 512
U = 32
W = G * U           # 1024 els per supertile block
NSUP = COLS // U    # 16
NQ = 4
QSUP = NSUP // NQ   # 4
QW = QSUP * W       # 4096
SC = 1.0 / 64.0
POS_THR = 1.0 / 3.0
NBIN = 16
NLEV = 2
NEG_POS_RATIO = 3.0


def _act_recip(nc, out, in_):
    """Raw Act-engine Reciprocal (table approx, ~0.5% rel err)."""
    sc = nc.scalar
    ins = [sc.lower_ap(in_)]
    for argv in (0.0, 1.0, 0.0):
        ins.append(mybir.ImmediateValue(dtype=mybir.dt.float32, value=argv))
    return sc.add_instruction(
        mybir.InstActivation(name=nc.get_next_instruction_name(),
                             func=Act.Reciprocal, ins=ins,
                             outs=[sc.lower_ap(out)]))


def _build_nc():
    nc = bacc.Bacc("TRN2", target_bir_lowering=False, debug=False)
    anch_d = nc.dram_tensor("anch", [P, COLS * 4], F32, kind="ExternalInput")
    bbox_d = nc.dram_tensor("bbox", [IMG, P, COLS * 4], F32, kind="ExternalInput")
    conf_d = nc.dram_tensor("conf", [IMG, P, COLS], F32, kind="ExternalInput")
    gt_d = nc.dram_tensor("gtb", [IMG, 1, G * 4], F32, kind="ExternalInput")
    res_d = nc.dram_tensor("res", [IMG, 1, 8], F32, kind="ExternalOutput")

    v = nc.vector
    sc = nc.scalar
    pe = nc.tensor

    with TileContext(nc) as tc, ExitStack() as ctx, \
            nc.allow_low_precision(reason="fp16 grid; host checks rel err"):
        pool = ctx.enter_context(tc.tile_pool(name="main", bufs=1))
        pspool = ctx.enter_context(tc.tile_pool(name="ps", bufs=1, space="PSUM"))

        def T(name, cols, parts=P, dt=F32):
            return pool.tile([parts, cols], dt, name=name)

        def T16(name, cols, parts=P):
            return pool.tile([parts, cols], FP16, name=name)

        # ---------------- per-core constants ----------------
        anch_sb = T("anch_sb", COLS * 4)
        nc.sync.dma_start(anch_sb[:, 0:COLS * 2], anch_d[:, 0:COLS * 2])
        nc.sync.dma_start(anch_sb[:, COLS * 2:], anch_d[:, COLS * 2:])
        anch3 = anch_sb[:].rearrange("p (n c) -> p n c", c=4)

        ax2h = T16("ax2h", COLS)
        ay2h = T16("ay2h", COLS)
        nax1h = T16("nax1h", COLS)
        nay1h = T16("nay1h", COLS)
        areaAh = T16("areaAh", COLS)
        f0 = T("f0", COLS)
        f1 = T("f1", COLS)
        sc.activation(ax2h[:], anch3[:, :, 2:3].squeeze(2), Act.Copy, scale=SC)
        sc.activation(ay2h[:], anch3[:, :, 3:4].squeeze(2), Act.Copy, scale=SC)
        sc.activation(nax1h[:], anch3[:, :, 0:1].squeeze(2), Act.Copy, scale=-SC)
        sc.activation(nay1h[:], anch3[:, :, 1:2].squeeze(2), Act.Copy, scale=-SC)
        v.tensor_tensor(out=f0[:], in0=anch3[:, :, 2:3].squeeze(2),
                        in1=anch3[:, :, 0:1].squeeze(2), op=Alu.subtract)
        v.tensor_tensor(out=f1[:], in0=anch3[:, :, 3:4].squeeze(2),
                        in1=anch3[:, :, 1:2].squeeze(2), op=Alu.subtract)
        v.tensor_tensor(out=f0[:], in0=f0[:], in1=f1[:], op=Alu.mult)
        sc.activation(areaAh[:], f0[:], Act.Copy, scale=SC * SC)

        ident = T16("ident", P)
        make_identity(nc, ident[:])
        identf = T("identf", P)
        make_identity(nc, identf[:])
        ones_col = T("ones_col", 1)
        ones_row = T("ones_row", P, parts=1)
        ones_row16 = T16("ones_row16", P, parts=1)
        v.memset(ones_col[:], 1.0)
        v.memset(ones_row[:], 1.0)
        v.memset(ones_row16[:], 1.0)
        iota_i = pool.tile([P, NBIN], mybir.dt.int32, name="iota_i")
        nc.gpsimd.iota(iota_i[:], pattern=[[1, NBIN]], base=0, channel_multiplier=0)
        iota_f = T("iota_f", NBIN)
        v.tensor_copy(iota_f[:], iota_i[:])

        def pbcast(dst, src_row):
            n = src_row.shape[-1]
            bc_ps = pspool.tile([P, G], F32, name="bc_ps", tag="pss")
            nc.tensor.matmul(bc_ps[:, 0:n], ones_row[:], src_row)
            v.tensor_copy(dst, bc_ps[:, 0:n])

        # ---------------- shared big tiles ----------------
        grid = T16("grid", NSUP * W)     # r values, (s, g, u) blocks
        tgrid = T16("tgrid", NSUP * W)   # one-hot, (s, u, g) blocks
        sA_l = [T16(f"sA{k}", QW) for k in range(2)]
        sB_l = [T16(f"sB{k}", QW) for k in range(2)]
        sCt_l = [T16("sCt0", QW)] * 2
        rm = T16("rm", COLS)
        colT = T16("colT", NSUP * G)
        cpa = T16("cpa", G)
        forced = T16("forced", COLS)
        pos16 = T16("pos16", COLS)
        posf = T("posf", COLS)
        cmb = T16("cmb", G)
        cmaxpl = T16("cmaxpl", W)

        gtrow_l = [T(f"gtrow{b}", G * 4, parts=1) for b in range(IMG)]
        gtall_l = [T(f"gtall{b}", G * 4) for b in range(IMG)]
        gsc = [T(f"gsc{c}", G) for c in range(4)]
        sGf = T("sGf", G)
        tG = T("tG", G)
        ngx1p = T16("ngx1p", W)
        ngy1p = T16("ngy1p", W)
        gx2p = T16("gx2p", W)
        gy2p = T16("gy2p", W)
        sGp = T16("sGp", W)
        gtmatf_l = [T(f"gtmatf{b}", 16) for b in range(IMG)]
        gtmath_l = [T16(f"gtmath{b}", 16) for b in range(IMG)]

        tsb = T16("tsb", W)
        matched = T16("matched", 4 * COLS)   # (s, u, c)
        mch = [T16(f"mch{c}", COLS) for c in range(4)]
        bxh = [T16(f"bxh{c}", COLS) for c in range(4)]
        areaPh = T16("areaPh", COLS)
        bbox_sb_l = [T(f"bbox_sb{b}", COLS * 4) for b in range(IMG)]
        conf_sb_l = [T(f"conf_sb{b}", COLS) for b in range(IMG)]

        d0 = T16("d0", COLS)
        d1 = T16("d1", COLS)
        d2 = T16("d2", COLS)
        d3 = T16("d3", COLS)
        d4 = T16("d4", COLS)

        s0 = T("s0", COLS)
        s1 = T("s1", COLS)
        s2 = T("s2", COLS)
        s3 = T("s3", COLS)
        s4 = T("s4", COLS)
        s5 = T("s5", COLS)
        cl = T("cl", COLS)
        nv = T("nv", COLS)
        nv16 = T16("nv16", COLS)
        sink16 = T16("sink16", COLS)

        npp = T("npp", 1)
        locsum_pp = T("locsum_pp", 1)
        possum_pp = T("possum_pp", 1)
        cnt_pp = T("cnt_pp", 1)
        sum_pp = T("sum_pp", 1)
        maxv_pp = T("maxv_pp", 1)
        maxvb = T("maxvb", 1)
        w1c = T("w1c", 1)
        tau_b = T("tau_b", 1)
        stack = T("stack", 4)
        thr = T("thr", NBIN)
        nthr = T("nthr", NBIN)
        cge = T("cge", NBIN)
        wl = [T(f"wl{l}", 1) for l in range(NLEV)]
        lo_b = [T(f"lo_b{l}", 1) for l in range(NLEV)]
        cget = T("cget", NBIN, parts=1)
        gek = T("gek", NBIN, parts=1)
        scnt = T("scnt", 1, parts=1)
        lo_new = T("lo_new", 1, parts=1)
        tau = [T(f"tau{l}", 1, parts=1) for l in range(NLEV)]
        maxv1 = T("maxv1", 1, parts=1)
        npos1 = T("npos1", 1, parts=1)
        k1 = T("k1", 1, parts=1)
        k2 = T("k2", 1, parts=1)
        kk = T("kk", 1, parts=1)
        mx_row = T("mx_row", P, parts=1)
        ctt = T16("ctt", P, parts=G)
        cmax_col = T16("cmax_col", 1, parts=G)
        cm_row = T16("cm_row", G, parts=1)
        res_sb = T("res_sb", 8, parts=1)

        # 4D view helpers
        def q4gu(t, q):  # (s, g, u) packed quarter
            return t[:, q*QW:(q+1)*QW].rearrange("p (s g u) -> p s g u", g=G, u=U)

        def aview(t, q):  # anchor [P, COLS] -> [p, s, g(b), u]
            return (t[:, q*QSUP*U:(q+1)*QSUP*U]
                    .rearrange("p (s u) -> p s u", u=U)
                    .unsqueeze(2).to_broadcast([P, QSUP, G, U]))

        def gview(t):     # gt plane [P, W] -> [p, s(b), g, u]
            return (t[:].rearrange("p (g u) -> p g u", u=U)
                    .unsqueeze(1).to_broadcast([P, QSUP, G, U]))

        for b in range(IMG):
            bbox_sb = bbox_sb_l[b]
            conf_sb = conf_sb_l[b]
            gtrow = gtrow_l[b]
            gtall = gtall_l[b]
            gtmatf = gtmatf_l[b]
            gtmath = gtmath_l[b]
            # ---------------- loads ----------------
            nc.sync.dma_start(bbox_sb[:, 0:COLS * 2], bbox_d[b][:, 0:COLS * 2])
            nc.sync.dma_start(bbox_sb[:, COLS * 2:], bbox_d[b][:, COLS * 2:])
            nc.sync.dma_start(conf_sb[:], conf_d[b])
            nc.scalar.dma_start(gtrow[:], gt_d[b])
            gt_ps = pspool.tile([P, G * 4], F32, name="gt_ps", tag="gtp")
            nc.tensor.matmul(gt_ps[:], ones_row[:], gtrow[:])
            v.tensor_copy(gtall[:], gt_ps[:])
            gt3 = gtall[:].rearrange("p (g c) -> p g c", c=4)
            gt2d = gt_d[b].rearrange("q (g c) -> (q g) c", c=4)
            v.memset(gtmatf[:], 0.0)
            for uu in range(4):
                nc.scalar.dma_start(gtmatf[uu*G:(uu+1)*G, uu*4:(uu+1)*4], gt2d)
            sc.activation(gtmath[:], gtmatf[:], Act.Copy, scale=SC)

            # ---------------- gt prep ----------------
            v.tensor_scalar(gsc[0][:], gt3[:, :, 0:1].squeeze(2), -SC, None, Alu.mult)
            v.tensor_scalar(gsc[1][:], gt3[:, :, 1:2].squeeze(2), -SC, None, Alu.mult)
            v.tensor_scalar(gsc[2][:], gt3[:, :, 2:3].squeeze(2), SC, None, Alu.mult)
            v.tensor_scalar(gsc[3][:], gt3[:, :, 3:4].squeeze(2), SC, None, Alu.mult)
            v.tensor_tensor(out=sGf[:], in0=gt3[:, :, 2:3].squeeze(2),
                            in1=gt3[:, :, 0:1].squeeze(2), op=Alu.subtract)
            v.tensor_tensor(out=tG[:], in0=gt3[:, :, 3:4].squeeze(2),
                            in1=gt3[:, :, 1:2].squeeze(2), op=Alu.subtract)
            v.tensor_tensor(out=sGf[:], in0=sGf[:], in1=tG[:], op=Alu.mult)
            v.tensor_scalar(sGf[:], sGf[:], SC * SC, None, Alu.mult)
            for pl, src in ((ngx1p, gsc[0]), (ngy1p, gsc[1]), (gx2p, gsc[2]),
                            (gy2p, gsc[3]), (sGp, sGf)):
                sc.activation(pl[:].rearrange("p (g u) -> p g u", u=U),
                              src[:].unsqueeze(2).to_broadcast([P, G, U]), Act.Copy)

            # ---------------- pass 1: grid ----------------
            for q in range(NQ):
                sA = sA_l[q % 2]
                sB = sB_l[q % 2]
                sCt = sCt_l[q % 2]
                gq = q4gu(grid, q)
                a4 = q4gu(sA, 0)
                b4 = q4gu(sB, 0)
                c4 = q4gu(sCt, 0)
                v.tensor_tensor(out=a4, in0=aview(nax1h, q), in1=gview(ngx1p), op=Alu.min)
                v.tensor_tensor(out=b4, in0=aview(ax2h, q), in1=gview(gx2p), op=Alu.min)
                v.tensor_tensor(out=a4, in0=b4, in1=a4, op=Alu.add)
                v.tensor_tensor(out=b4, in0=aview(nay1h, q), in1=gview(ngy1p), op=Alu.min)
                v.tensor_tensor(out=c4, in0=aview(ay2h, q), in1=gview(gy2p), op=Alu.min)
                v.tensor_tensor(out=b4, in0=c4, in1=b4, op=Alu.add)
                v.tensor_scalar(sA[:], sA[:], 0.0, None, Alu.max)
                v.tensor_scalar(sB[:], sB[:], 0.0, None, Alu.max)
                v.tensor_tensor(out=a4, in0=a4, in1=b4, op=Alu.mult)      # inter
                HW2 = QW // 2
                for hh in range(2):
                    s0q = q * QSUP + hh * (QSUP // 2)
                    hsl = slice(hh * HW2, (hh + 1) * HW2)
                    bh = (sB_l[q % 2][:, hsl]
                          .rearrange("p (s g u) -> p s g u", g=G, u=U))
                    ah = (sA_l[q % 2][:, hsl]
                          .rearrange("p (s g u) -> p s g u", g=G, u=U))
                    gh = (grid[:, q*QW + hh*HW2: q*QW + (hh+1)*HW2]
                          .rearrange("p (s g u) -> p s g u", g=G, u=U))
                    avh = (areaAh[:, s0q*U:(s0q + QSUP//2)*U]
                           .rearrange("p (s u) -> p s u", u=U)
                           .unsqueeze(2).to_broadcast([P, QSUP//2, G, U]))
                    gvh = (sGp[:].rearrange("p (g u) -> p g u", u=U)
                           .unsqueeze(1).to_broadcast([P, QSUP//2, G, U]))
                    v.tensor_tensor(out=bh, in0=avh, in1=gvh, op=Alu.add)
                    _act_recip(nc, sB_l[q % 2][:, hsl], sB_l[q % 2][:, hsl])
                    v.tensor_tensor(out=gh, in0=ah, in1=bh, op=Alu.mult)

                cur = gq
                width = G
                while width > 1:
                    h = width // 2
                    if h == 1:
                        dst = (rm[:, q*QSUP*U:(q+1)*QSUP*U]
                               .rearrange("p (s u) -> p s u", u=U).unsqueeze(2))
                    else:
                        dst = q4gu(sCt, 0)[:, :, 0:h, :]
                    v.tensor_tensor(out=dst, in0=cur[:, :, 0:h, :],
                                    in1=cur[:, :, h:width, :], op=Alu.max)
                    cur = dst
                    width = h
                curc = gq
                widc = U
                while widc > 1:
                    hc = widc // 2
                    if hc == 1:
                        dstc = (colT[:, q*QSUP*G:(q+1)*QSUP*G]
                                .rearrange("p (s g) -> p s g", g=G).unsqueeze(3))
                    else:
                        dstc = q4gu(sB, 0)[:, :, :, 0:hc]
                    v.tensor_tensor(out=dstc, in0=curc[:, :, :, 0:hc],
                                    in1=curc[:, :, :, hc:widc], op=Alu.max)
                    curc = dstc
                    widc = hc

            # ---------------- cmax finalize ----------------
            v.tensor_reduce(out=cpa[:], in_=colT[:].rearrange("p (s g) -> p g s", g=G),
                            axis=Ax.X, op=Alu.max)
            ct_ps = pspool.tile([G, P], FP16, name="ct_ps", tag="pst")
            pe.transpose(ct_ps[:], cpa[:], ident[:])
            v.tensor_copy(ctt[:], ct_ps[:])
            v.tensor_reduce(out=cmax_col[:], in_=ctt[:], axis=Ax.X, op=Alu.max)
            cm_ps = pspool.tile([1, G], FP16, name="cm_ps", tag="pst")
            pe.transpose(cm_ps[:], cmax_col[:], ident[:G, :G])
            v.tensor_copy(cm_row[:], cm_ps[:])
            bc_ps = pspool.tile([P, G], F32, name="bc_ps", tag="pss")
            nc.tensor.matmul(bc_ps[:], ones_row16[:], cm_row[:])
            v.tensor_copy(cmb[:], bc_ps[:])
            sc.activation(cmaxpl[:].rearrange("p (g u) -> p g u", u=U),
                          cmb[:].unsqueeze(2).to_broadcast([P, G, U]), Act.Copy)

            # ---------------- forced + pos ----------------
            for q in range(NQ):
                sA = sA_l[q % 2]
                gq = q4gu(grid, q)
                v.tensor_tensor(out=q4gu(sA, 0), in0=gq, in1=gview(cmaxpl),
                                op=Alu.is_equal)
                cur = q4gu(sA, 0)
                width = G
                while width > 1:
                    h = width // 2
                    if h == 1:
                        dst = (forced[:, q*QSUP*U:(q+1)*QSUP*U]
                               .rearrange("p (s u) -> p s u", u=U).unsqueeze(2))
                    else:
                        dst = cur[:, :, 0:h, :]
                    v.tensor_tensor(out=dst, in0=cur[:, :, 0:h, :],
                                    in1=cur[:, :, h:width, :], op=Alu.max)
                    cur = dst
                    width = h
            v.tensor_scalar(pos16[:], rm[:], POS_THR, None, Alu.is_gt)
            v.tensor_tensor(out=pos16[:], in0=pos16[:], in1=forced[:], op=Alu.max)
            v.tensor_scalar(sink16[:], pos16[:], 1.0, 0.0, Alu.mult, Alu.add,
                            accum_out=npp[:])
            v.tensor_copy(posf[:], pos16[:])

            # ---------------- ismax -> tgrid (s, u, g) ----------------
            for q in range(NQ):
                gin = grid[:, q*QW:(q+1)*QW].rearrange(
                    "p (s g u) -> p s u g", g=G, u=U)
                rmv = (rm[:, q*QSUP*U:(q+1)*QSUP*U]
                       .rearrange("p (s u) -> p s u", u=U)
                       .unsqueeze(3).to_broadcast([P, QSUP, U, G]))
                tout = tgrid[:, q*QW:(q+1)*QW].rearrange(
                    "p (s u g) -> p s u g", u=U, g=G)
                v.tensor_tensor(out=tout, in0=gin, in1=rmv, op=Alu.is_equal)

            # ---------------- matched coords (PE) ----------------
            for s in range(NSUP):
                mout = pspool.tile([P, P], F32, name=f"mo{s % 2}", tag=f"mo{s % 2}")
                for j in range(8):
                    tp = pspool.tile([P, P], FP16, name=f"tp{j % 2}", tag=f"tp{j % 2}")
                    pe.transpose(tp[:], tgrid[:, s*W + j*P: s*W + (j+1)*P], ident[:])
                    sc.copy(tsb[:, j*P:(j+1)*P], tp[:])
                    nc.tensor.matmul(mout[:, j*16:(j+1)*16], tsb[:, j*P:(j+1)*P],
                                     gtmath[:])
                sc.copy(matched[:, s*P:(s+1)*P], mout[:])

            m4 = matched[:].rearrange("p (n c) -> p n c", c=4)
            for c in range(4):
                sc.copy(mch[c][:], m4[:, :, c:c+1].squeeze(2))

            # ---------------- bbox planes + diou ----------------
            bb3 = bbox_sb[:].rearrange("p (n c) -> p n c", c=4)
            for c in range(4):
                sc.activation(bxh[c][:], bb3[:, :, c:c+1].squeeze(2), Act.Copy,
                              scale=SC)
            v.tensor_tensor(out=d0[:], in0=bxh[2][:], in1=bxh[0][:], op=Alu.subtract)
            v.tensor_tensor(out=d1[:], in0=bxh[3][:], in1=bxh[1][:], op=Alu.subtract)
            v.tensor_tensor(out=areaPh[:], in0=d0[:], in1=d1[:], op=Alu.mult)

            # inter
            v.tensor_tensor(out=d0[:], in0=bxh[0][:], in1=mch[0][:], op=Alu.max)
            v.tensor_tensor(out=d1[:], in0=bxh[2][:], in1=mch[2][:], op=Alu.min)
            v.tensor_tensor(out=d0[:], in0=d1[:], in1=d0[:], op=Alu.subtract)
            v.tensor_scalar(d0[:], d0[:], 0.0, None, Alu.max)
            v.tensor_tensor(out=d1[:], in0=bxh[1][:], in1=mch[1][:], op=Alu.max)
            v.tensor_tensor(out=d2[:], in0=bxh[3][:], in1=mch[3][:], op=Alu.min)
            v.tensor_tensor(out=d1[:], in0=d2[:], in1=d1[:], op=Alu.subtract)
            v.tensor_scalar(d1[:], d1[:], 0.0, None, Alu.max)
            v.tensor_tensor(out=d0[:], in0=d0[:], in1=d1[:], op=Alu.mult)  # inter
            # matched area
            v.tensor_tensor(out=d1[:], in0=mch[2][:], in1=mch[0][:], op=Alu.subtract)
            v.tensor_tensor(out=d2[:], in0=mch[3][:], in1=mch[1][:], op=Alu.subtract)
            v.tensor_tensor(out=d1[:], in0=d1[:], in1=d2[:], op=Alu.mult)
            # union, iou
            v.tensor_tensor(out=d1[:], in0=d1[:], in1=areaPh[:], op=Alu.add)
            v.tensor_tensor(out=d1[:], in0=d1[:], in1=d0[:], op=Alu.subtract)
            _act_recip(nc, d1[:], d1[:])
            v.tensor_tensor(out=d0[:], in0=d0[:], in1=d1[:], op=Alu.mult)  # iou
            # enclosing c2
            v.tensor_tensor(out=d1[:], in0=bxh[0][:], in1=mch[0][:], op=Alu.min)
            v.tensor_tensor(out=d2[:], in0=bxh[2][:], in1=mch[2][:], op=Alu.max)
            v.tensor_tensor(out=d1[:], in0=d2[:], in1=d1[:], op=Alu.subtract)
            v.tensor_tensor(out=d1[:], in0=d1[:], in1=d1[:], op=Alu.mult)
            v.tensor_tensor(out=d2[:], in0=bxh[1][:], in1=mch[1][:], op=Alu.min)
            v.tensor_tensor(out=d3[:], in0=bxh[3][:], in1=mch[3][:], op=Alu.max)
            v.tensor_tensor(out=d2[:], in0=d3[:], in1=d2[:], op=Alu.subtract)
            v.tensor_tensor(out=d2[:], in0=d2[:], in1=d2[:], op=Alu.mult)
            v.tensor_tensor(out=d1[:], in0=d1[:], in1=d2[:], op=Alu.add)   # c2
            _act_recip(nc, d1[:], d1[:])
            # center dist d2 (quarter-scaled: absorbed by c2 ratio using same /2)
            v.tensor_tensor(out=d2[:], in0=bxh[0][:], in1=bxh[2][:], op=Alu.add)
            v.tensor_tensor(out=d3[:], in0=mch[0][:], in1=mch[2][:], op=Alu.add)
            v.tensor_tensor(out=d2[:], in0=d2[:], in1=d3[:], op=Alu.subtract)
            v.tensor_tensor(out=d2[:], in0=d2[:], in1=d2[:], op=Alu.mult)
            v.tensor_tensor(out=d3[:], in0=bxh[1][:], in1=bxh[3][:], op=Alu.add)
            v.tensor_tensor(out=d4[:], in0=mch[1][:], in1=mch[3][:], op=Alu.add)
            v.tensor_tensor(out=d3[:], in0=d3[:], in1=d4[:], op=Alu.subtract)
            v.tensor_tensor(out=d3[:], in0=d3[:], in1=d3[:], op=Alu.mult)
            v.tensor_tensor(out=d2[:], in0=d2[:], in1=d3[:], op=Alu.add)   # 4*d2
            v.tensor_tensor(out=d2[:], in0=d2[:], in1=d1[:], op=Alu.mult)
            v.tensor_scalar(d2[:], d2[:], 0.25, None, Alu.mult)            # d2/c2
            v.tensor_scalar(d0[:], d0[:], -1.0, 1.0, Alu.mult, Alu.add)    # 1-iou
            v.tensor_tensor(out=d2[:], in0=d2[:], in1=d0[:], op=Alu.add)
            v.tensor_scalar(d2[:], d2[:], 100.0, None, Alu.min)
            v.tensor_tensor(out=d2[:], in0=d2[:], in1=pos16[:], op=Alu.mult)
            v.tensor_scalar(sink16[:], d2[:], 1.0, 0.0, Alu.mult, Alu.add,
                            accum_out=locsum_pp[:])

            # ---------------- focal conf loss (f32, baseline) ----------------
            sc.activation(s0[:], conf_sb[:], Act.Sigmoid)
            sc.activation(s1[:], conf_sb[:], Act.Exp)
            sc.activation(s1[:], s1[:], Act.Ln, bias=1.0)
            v.tensor_tensor(out=s2[:], in0=conf_sb[:], in1=posf[:], op=Alu.mult)
            v.tensor_tensor(out=s2[:], in0=s1[:], in1=s2[:], op=Alu.subtract)
            v.tensor_scalar(s3[:], posf[:], -2.0, 1.0, Alu.mult, Alu.add)
            v.tensor_tensor(out=s3[:], in0=s0[:], in1=s3[:], op=Alu.mult)
            v.tensor_tensor(out=s3[:], in0=s3[:], in1=posf[:], op=Alu.add)
            sc.activation(s3[:], s3[:], Act.Square)
            v.tensor_tensor(out=cl[:], in0=s3[:], in1=s2[:], op=Alu.mult)
            v.tensor_scalar(s3[:], posf[:], -0.5, 0.75, Alu.mult, Alu.add)
            v.tensor_tensor(out=cl[:], in0=cl[:], in1=s3[:], op=Alu.mult)
            v.tensor_scalar(cl[:], cl[:], 100.0, None, Alu.min)
            v.tensor_tensor(out=s4[:], in0=cl[:], in1=posf[:], op=Alu.mult)
            v.tensor_scalar(s5[:], s4[:], 1.0, 0.0, Alu.mult, Alu.add,
                            accum_out=possum_pp[:])
            v.tensor_tensor(out=nv[:], in0=cl[:], in1=s4[:], op=Alu.subtract)
            v.tensor_copy(nv16[:], nv[:])

            # ---------------- hard negative mining (baseline) ----------------
            v.tensor_reduce(out=maxv_pp[:], in_=nv[:], axis=Ax.X, op=Alu.max)
            mx_ps = pspool.tile([1, P], F32, name="mx_ps", tag="pss")
            pe.transpose(mx_ps[:], maxv_pp[:], identf[:])
            v.tensor_copy(mx_row[:], mx_ps[:])
            v.tensor_reduce(out=maxv1[:], in_=mx_row[:], axis=Ax.X, op=Alu.max)

            np_ps = pspool.tile([1, 1], F32, name="np_ps", tag="pss")
            nc.tensor.matmul(np_ps[:], ones_col[:], npp[:])
            v.tensor_copy(npos1[:], np_ps[:])
            v.tensor_scalar(k1[:], npos1[:], NEG_POS_RATIO, None, Alu.mult)
            v.tensor_scalar(k2[:], npos1[:], -1.0, float(A), Alu.mult, Alu.add)
            v.tensor_tensor(out=kk[:], in0=k1[:], in1=k2[:], op=Alu.min)

            pbcast(maxvb[:], maxv1[:])
            v.tensor_scalar(w1c[:], maxvb[:], 1.0 / NBIN, None, Alu.mult)

            for lev in range(NLEV):
                if lev == 0:
                    v.tensor_copy(wl[0][:], w1c[:])
                    v.tensor_scalar(thr[:], iota_f[:], wl[0][:], None, Alu.mult)
                else:
                    v.tensor_scalar(wl[lev][:], wl[lev - 1][:], 1.0 / NBIN, None,
                                    Alu.mult)
                    v.tensor_scalar(thr[:], iota_f[:], wl[lev][:], lo_b[lev - 1][:],
                                    Alu.mult, Alu.add)
                v.tensor_scalar(nthr[:], thr[:], -1.0, None, Alu.mult)
                # split bins: even half on Act (sign-sum), odd half on DVE (is_gt)
                for bn in range(NBIN // 2):
                    sc.activation(sink16[:], nv16[:], Act.Sign,
                                  bias=nthr[:, bn:bn+1], accum_out=cge[:, bn:bn+1])
                for bn in range(NBIN // 2, NBIN):
                    v.tensor_scalar(d4[:], nv16[:], thr[:, bn:bn+1], 0.0,
                                    Alu.is_gt, Alu.add, accum_out=cge[:, bn:bn+1])
                cg_ps = pspool.tile([1, NBIN], F32, name="cg_ps", tag="pss")
                nc.tensor.matmul(cg_ps[:], ones_col[:], cge[:])
                v.tensor_copy(cget[:], cg_ps[:])
                # Act half holds sign-sums: cnt = (acc + A)/2; DVE half is exact
                v.tensor_scalar(cget[:, 0:NBIN // 2], cget[:, 0:NBIN // 2], 0.5,
                                float(A) * 0.5, Alu.mult, Alu.add)
                v.tensor_scalar(gek[:], cget[:], kk[:], None, Alu.is_ge)
                v.tensor_reduce(out=scnt[:], in_=gek[:], axis=Ax.X, op=Alu.add)
                v.tensor_scalar(lo_new[:], scnt[:], 1.0, wl[lev][0:1, :],
                                Alu.subtract, Alu.mult)
                v.tensor_scalar(tau[lev][:], scnt[:], wl[lev][0:1, :], None, Alu.mult)
                if lev > 0:
                    v.tensor_tensor(out=lo_new[:], in0=lo_new[:],
                                    in1=lo_b[lev - 1][0:1, :], op=Alu.add)
                    v.tensor_tensor(out=tau[lev][:], in0=tau[lev][:],
                                    in1=lo_b[lev - 1][0:1, :], op=Alu.add)
                pbcast(lo_b[lev][:], lo_new[:])

            pbcast(tau_b[:], tau[NLEV - 1][:])
            v.tensor_scalar(s4[:], nv[:], tau_b[:], 0.0, Alu.is_gt,
                            Alu.add, accum_out=cnt_pp[:])
            v.tensor_tensor(out=s5[:], in0=nv[:], in1=s4[:], op=Alu.mult)
            v.tensor_scalar(s5[:], s5[:], 1.0, 0.0, Alu.mult, Alu.add,
                            accum_out=sum_pp[:])

            # ---------------- gather scalars ----------------
            v.tensor_copy(stack[:, 0:1], npp[:])
            v.tensor_copy(stack[:, 1:2], locsum_pp[:])
            v.tensor_copy(stack[:, 2:3], possum_pp[:])
            v.tensor_copy(stack[:, 3:4], cnt_pp[:])
            st_ps = pspool.tile([1, 4], F32, name="st_ps", tag="pss")
            nc.tensor.matmul(st_ps[:], ones_col[:], stack[:])
            sm_ps = pspool.tile([1, 1], F32, name="sm_ps", tag="pss")
            nc.tensor.matmul(sm_ps[:], ones_col[:], sum_pp[:])

            v.tensor_copy(res_sb[:, 0:4], st_ps[:])
            v.tensor_copy(res_sb[:, 4:5], sm_ps[:])
            v.tensor_copy(res_sb[:, 5:6], tau[NLEV - 1][:])
            v.tensor_copy(res_sb[:, 6:7], maxv1[:])
            v.tensor_copy(res_sb[:, 7:8], kk[:])
            nc.sync.dma_start(res_d[b], res_sb[:])

    nc.compile()
    return nc


_NC_CACHE = None


def _get_nc():
    global _NC_CACHE
    if _NC_CACHE is None:
        _NC_CACHE = _build_nc()
    return _NC_CACHE


def _make_in_maps(inputs):
    bbox_pred = np.asarray(inputs["bbox_pred"])
    conf_pred = np.asarray(inputs["conf_pred"])
    anchors = np.asarray(inputs["anchors"])
    gt_boxes = np.asarray(inputs["gt_boxes"])
    anch_h = np.ascontiguousarray(anchors.reshape(P, COLS * 4), dtype=np.float32)
    in_maps = []
    for i in range(NCORE):
        bsl = slice(IMG * i, IMG * (i + 1))
        in_maps.append({
            "anch": anch_h,
            "bbox": np.ascontiguousarray(
                bbox_pred[bsl].reshape(IMG, P, COLS * 4), dtype=np.float32),
            "conf": np.ascontiguousarray(
                conf_pred[bsl].reshape(IMG, P, COLS), dtype=np.float32),
            "gtb": np.ascontiguousarray(
                gt_boxes[bsl].reshape(IMG, 1, G * 4), dtype=np.float32),
        })
    return in_maps


def kernel(bbox_pred, conf_pred, anchors, gt_boxes):
    nc = _get_nc()
    in_maps = _make_in_maps(dict(bbox_pred=bbox_pred, conf_pred=conf_pred,
                                 anchors=anchors, gt_boxes=gt_boxes))
    out = run_bass_kernel_spmd(nc, in_maps, core_ids=list(range(NCORE)))

    loc_total = np.float32(0.0)
    conf_total = np.float32(0.0)
    npos_total = np.float32(0.0)
    for i in range(NCORE):
        res = out.results[i]["res"]  # [IMG, 1, 8]
        for b in range(IMG):
            npos, locsum, possum, cnt_gt, sum_gt, tau_hi, maxv, kdev = \
                [np.float32(x) for x in res[b, 0, :8]]
            k = np.float32(min(NEG_POS_RATIO * npos, A - npos))
            wl_last = np.float32(maxv / NBIN ** NLEV)
            rem = max(np.float32(0.0), np.float32(k - cnt_gt))
            neg = np.float32(sum_gt + rem * (tau_hi - wl_last * np.float32(0.5)))
            loc_total = np.float32(loc_total + locsum)
            conf_total = np.float32(conf_total + possum + neg)
            npos_total = np.float32(npos_total + npos)
    num_pos = np.float32(max(1.0, npos_total))
    loc_loss = np.float32(loc_total / num_pos)
    conf_loss = np.float32(conf_total / num_pos)
    return (np.float32(loc_loss + conf_loss), conf_loss, loc_loss)


# revision 13
# speedup vs baseline: 1.5412x; 1.0029x over previous
"""Trainium2 Bass kernel v2 for nn_DetectionLoss — fp16 grid pipeline.

Data-parallel: 16 images over 8 cores (2 images/core). Per image, the
[A=65536, G=32] match grid is computed in fp16 (coords pre-scaled by 1/64 —
the loss is scale-invariant) in supertile-(g,u) layout so every DVE op is
packed-innermost and runs in 2x/4x mode. Division uses the Act engine's
table Reciprocal (~0.5% rel err — only feeds match *selection*, fine at the
2e-2 gate). Matched-gt coords go through fp16 PE transposes of the one-hot
grid + 16-wide block-diag matmuls. DIoU runs in fp16 on coord planes; focal
+ hard-negative mining keep the baseline f32/fp16 structure. Host combines
per-image scalars exactly like the reference."""
import sys

sys.path.insert(0, '/opt/trn_rl_repo')

import numpy as np
import concourse.bass as bass
import concourse.bacc as bacc
import concourse.mybir as mybir
from concourse.tile import TileContext
from concourse.bass_utils import run_bass_kernel_spmd
from concourse.masks import make_identity
from contextlib import ExitStack

Alu = mybir.AluOpType
Act = mybir.ActivationFunctionType
Ax = mybir.AxisListType
F32 = mybir.dt.float32
FP16 = mybir.dt.float16

P = 128
A = 65536
G = 32
IMG = 2
NCORE = 8
COLS = A // P       # 512
U = 32
W = G * U           # 1024 els per supertile block
NSUP = COLS // U    # 16
NQ = 4
QSUP = NSUP // NQ   # 4
QW = QSUP * W       # 4096
SC = 1.0 / 64.0
POS_THR = 1.0 / 3.0
NBIN = 16
NLEV = 2
NEG_POS_RATIO = 3.0


def _act_recip(nc, out, in_):
    """Raw Act-engine Reciprocal (table approx, ~0.5% rel err)."""
    sc = nc.scalar
    ins = [sc.lower_ap(in_)]
    for argv in (0.0, 1.0, 0.0):
        ins.append(mybir.ImmediateValue(dtype=mybir.dt.float32, value=argv))
    return sc.add_instruction(
        mybir.InstActivation(name=nc.get_next_instruction_name(),
                             func=Act.Reciprocal, ins=ins,
                             outs=[sc.lower_ap(out)]))


def _build_nc():
    nc = bacc.Bacc("TRN2", target_bir_lowering=False, debug=False)
    anch_d = nc.dram_tensor("anch", [P, COLS * 4], F32, kind="ExternalInput")
    bbox_d = nc.dram_tensor("bbox", [IMG, P, COLS * 4], F32, kind="ExternalInput")
    conf_d = nc.dram_tensor("conf", [IMG, P, COLS], F32, kind="ExternalInput")
    gt_d = nc.dram_tensor("gtb", [IMG, 1, G * 4], F32, kind="ExternalInput")
    res_d = nc.dram_tensor("res", [IMG, 1, 8], F32, kind="ExternalOutput")

    v = nc.vector
    sc = nc.scalar
    pe = nc.tensor

    with TileContext(nc) as tc, ExitStack() as ctx, \
            nc.allow_low_precision(reason="fp16 grid; host checks rel err"):
        pool = ctx.enter_context(tc.tile_pool(name="main", bufs=1))
        pspool = ctx.enter_context(tc.tile_pool(name="ps", bufs=1, space="PSUM"))

        def T(name, cols, parts=P, dt=F32):
            return pool.tile([parts, cols], dt, name=name)

        def T16(name, cols, parts=P):
            return pool.tile([parts, cols], FP16, name=name)

        # ---------------- per-core constants ----------------
        anch_sb = T("anch_sb", COLS * 4)
        nc.sync.dma_start(anch_sb[:, 0:COLS * 2], anch_d[:, 0:COLS * 2])
        nc.sync.dma_start(anch_sb[:, COLS * 2:], anch_d[:, COLS * 2:])
        anch3 = anch_sb[:].rearrange("p (n c) -> p n c", c=4)

        ax2h = T16("ax2h", COLS)
        ay2h = T16("ay2h", COLS)
        nax1h = T16("nax1h", COLS)
        nay1h = T16("nay1h", COLS)
        areaAh = T16("areaAh", COLS)
        f0 = T("f0", COLS)
        f1 = T("f1", COLS)
        sc.activation(ax2h[:], anch3[:, :, 2:3].squeeze(2), Act.Copy, scale=SC)
        sc.activation(ay2h[:], anch3[:, :, 3:4].squeeze(2), Act.Copy, scale=SC)
        sc.activation(nax1h[:], anch3[:, :, 0:1].squeeze(2), Act.Copy, scale=-SC)
        sc.activation(nay1h[:], anch3[:, :, 1:2].squeeze(2), Act.Copy, scale=-SC)
        v.tensor_tensor(out=f0[:], in0=anch3[:, :, 2:3].squeeze(2),
                        in1=anch3[:, :, 0:1].squeeze(2), op=Alu.subtract)
        v.tensor_tensor(out=f1[:], in0=anch3[:, :, 3:4].squeeze(2),
                        in1=anch3[:, :, 1:2].squeeze(2), op=Alu.subtract)
        v.tensor_tensor(out=f0[:], in0=f0[:], in1=f1[:], op=Alu.mult)
        sc.activation(areaAh[:], f0[:], Act.Copy, scale=SC * SC)

        ident = T16("ident", P)
        make_identity(nc, ident[:])
        identf = T("identf", P)
        make_identity(nc, identf[:])
        ones_col = T("ones_col", 1)
        ones_row = T("ones_row", P, parts=1)
        ones_row16 = T16("ones_row16", P, parts=1)
        v.memset(ones_col[:], 1.0)
        v.memset(ones_row[:], 1.0)
        v.memset(ones_row16[:], 1.0)
        iota_i = pool.tile([P, NBIN], mybir.dt.int32, name="iota_i")
        nc.gpsimd.iota(iota_i[:], pattern=[[1, NBIN]], base=0, channel_multiplier=0)
        iota_f = T("iota_f", NBIN)
        v.tensor_copy(iota_f[:], iota_i[:])

        def pbcast(dst, src_row):
            n = src_row.shape[-1]
            bc_ps = pspool.tile([P, G], F32, name="bc_ps", tag="pss")
            nc.tensor.matmul(bc_ps[:, 0:n], ones_row[:], src_row)
            v.tensor_copy(dst, bc_ps[:, 0:n])

        # ---------------- shared big tiles ----------------
        grid = T16("grid", NSUP * W)     # r values, (s, g, u) blocks
        tgrid = T16("tgrid", NSUP * W)   # one-hot, (s, u, g) blocks
        sA_l = [T16(f"sA{k}", QW) for k in range(2)]
        sB_l = [T16(f"sB{k}", QW) for k in range(2)]
        sCt_l = [T16("sCt0", QW)] * 2
        rm = T16("rm", COLS)
        colT = T16("colT", NSUP * G)
        cpa = T16("cpa", G)
        forced = T16("forced", COLS)
        pos16 = T16("pos16", COLS)
        posf = T("posf", COLS)
        cmb = T16("cmb", G)
        cmaxpl = T16("cmaxpl", W)

        gtrow_l = [T(f"gtrow{b}", G * 4, parts=1) for b in range(IMG)]
        gtall_l = [T(f"gtall{b}", G * 4) for b in range(IMG)]
        gsc = [T(f"gsc{c}", G) for c in range(4)]
        sGf = T("sGf", G)
        tG = T("tG", G)
        ngx1p = T16("ngx1p", W)
        ngy1p = T16("ngy1p", W)
        gx2p = T16("gx2p", W)
        gy2p = T16("gy2p", W)
        sGp = T16("sGp", W)
        gtmatf_l = [T(f"gtmatf{b}", 16) for b in range(IMG)]
        gtmath_l = [T16(f"gtmath{b}", 16) for b in range(IMG)]

        tsb_l = [T16(f"tsb{k}", W) for k in range(2)]
        matched = T16("matched", 4 * COLS)   # (s, u, c)
        mch = [T16(f"mch{c}", COLS) for c in range(4)]
        bxh = [T16(f"bxh{c}", COLS) for c in range(4)]
        areaPh = T16("areaPh", COLS)
        bbox_sb_l = [T(f"bbox_sb{b}", COLS * 4) for b in range(IMG)]
        conf_sb_l = [T(f"conf_sb{b}", COLS) for b in range(IMG)]

        d0 = T16("d0", COLS)
        d1 = T16("d1", COLS)
        d2 = T16("d2", COLS)
        d3 = T16("d3", COLS)
        d4 = T16("d4", COLS)

        s0 = T("s0", COLS)
        s1 = T("s1", COLS)
        s2 = T("s2", COLS)
        s3 = T("s3", COLS)
        s4 = T("s4", COLS)
        s5 = T("s5", COLS)
        cl = T("cl", COLS)
        nv = T("nv", COLS)
        nv16 = T16("nv16", COLS)
        sink16 = T16("sink16", COLS)

        npp = T("npp", 1)
        locsum_pp = T("locsum_pp", 1)
        possum_pp = T("possum_pp", 1)
        cnt_pp = T("cnt_pp", 1)
        sum_pp = T("sum_pp", 1)
        maxv_pp = T("maxv_pp", 1)
        maxvb = T("maxvb", 1)
        w1c = T("w1c", 1)
        tau_b = T("tau_b", 1)
        stack = T("stack", 4)
        thr = T("thr", NBIN)
        nthr = T("nthr", NBIN)
        cge = T("cge", NBIN)
        wl = [T(f"wl{l}", 1) for l in range(NLEV)]
        lo_b = [T(f"lo_b{l}", 1) for l in range(NLEV)]
        cget = T("cget", NBIN, parts=1)
        gek = T("gek", NBIN, parts=1)
        scnt = T("scnt", 1, parts=1)
        lo_new = T("lo_new", 1, parts=1)
        tau = [T(f"tau{l}", 1, parts=1) for l in range(NLEV)]
        maxv1 = T("maxv1", 1, parts=1)
        npos1 = T("npos1", 1, parts=1)
        k1 = T("k1", 1, parts=1)
        k2 = T("k2", 1, parts=1)
        kk = T("kk", 1, parts=1)
        mx_row = T("mx_row", P, parts=1)
        ctt = T16("ctt", P, parts=G)
        cmax_col = T16("cmax_col", 1, parts=G)
        cm_row = T16("cm_row", G, parts=1)
        res_sb = T("res_sb", 8, parts=1)

        # 4D view helpers
        def q4gu(t, q):  # (s, g, u) packed quarter
            return t[:, q*QW:(q+1)*QW].rearrange("p (s g u) -> p s g u", g=G, u=U)

        def aview(t, q):  # anchor [P, COLS] -> [p, s, g(b), u]
            return (t[:, q*QSUP*U:(q+1)*QSUP*U]
                    .rearrange("p (s u) -> p s u", u=U)
                    .unsqueeze(2).to_broadcast([P, QSUP, G, U]))

        def gview(t):     # gt plane [P, W] -> [p, s(b), g, u]
            return (t[:].rearrange("p (g u) -> p g u", u=U)
                    .unsqueeze(1).to_broadcast([P, QSUP, G, U]))

        for b in range(IMG):
            bbox_sb = bbox_sb_l[b]
            conf_sb = conf_sb_l[b]
            gtrow = gtrow_l[b]
            gtall = gtall_l[b]
            gtmatf = gtmatf_l[b]
            gtmath = gtmath_l[b]
            # ---------------- loads ----------------
            nc.sync.dma_start(bbox_sb[:, 0:COLS * 2], bbox_d[b][:, 0:COLS * 2])
            nc.sync.dma_start(bbox_sb[:, COLS * 2:], bbox_d[b][:, COLS * 2:])
            nc.sync.dma_start(conf_sb[:], conf_d[b])
            nc.scalar.dma_start(gtrow[:], gt_d[b])
            gt_ps = pspool.tile([P, G * 4], F32, name="gt_ps", tag="gtp")
            nc.tensor.matmul(gt_ps[:], ones_row[:], gtrow[:])
            v.tensor_copy(gtall[:], gt_ps[:])
            gt3 = gtall[:].rearrange("p (g c) -> p g c", c=4)
            gt2d = gt_d[b].rearrange("q (g c) -> (q g) c", c=4)
            v.memset(gtmatf[:], 0.0)
            for uu in range(4):
                nc.scalar.dma_start(gtmatf[uu*G:(uu+1)*G, uu*4:(uu+1)*4], gt2d)
            sc.activation(gtmath[:], gtmatf[:], Act.Copy, scale=SC)

            # ---------------- gt prep ----------------
            v.tensor_scalar(gsc[0][:], gt3[:, :, 0:1].squeeze(2), -SC, None, Alu.mult)
            v.tensor_scalar(gsc[1][:], gt3[:, :, 1:2].squeeze(2), -SC, None, Alu.mult)
            v.tensor_scalar(gsc[2][:], gt3[:, :, 2:3].squeeze(2), SC, None, Alu.mult)
            v.tensor_scalar(gsc[3][:], gt3[:, :, 3:4].squeeze(2), SC, None, Alu.mult)
            v.tensor_tensor(out=sGf[:], in0=gt3[:, :, 2:3].squeeze(2),
                            in1=gt3[:, :, 0:1].squeeze(2), op=Alu.subtract)
            v.tensor_tensor(out=tG[:], in0=gt3[:, :, 3:4].squeeze(2),
                            in1=gt3[:, :, 1:2].squeeze(2), op=Alu.subtract)
            v.tensor_tensor(out=sGf[:], in0=sGf[:], in1=tG[:], op=Alu.mult)
            v.tensor_scalar(sGf[:], sGf[:], SC * SC, None, Alu.mult)
            for pl, src in ((ngx1p, gsc[0]), (ngy1p, gsc[1]), (gx2p, gsc[2]),
                            (gy2p, gsc[3]), (sGp, sGf)):
                sc.activation(pl[:].rearrange("p (g u) -> p g u", u=U),
                              src[:].unsqueeze(2).to_broadcast([P, G, U]), Act.Copy)

            # ---------------- pass 1: grid ----------------
            for q in range(NQ):
                sA = sA_l[q % 2]
                sB = sB_l[q % 2]
                sCt = sCt_l[q % 2]
                gq = q4gu(grid, q)
                a4 = q4gu(sA, 0)
                b4 = q4gu(sB, 0)
                c4 = q4gu(sCt, 0)
                v.tensor_tensor(out=a4, in0=aview(nax1h, q), in1=gview(ngx1p), op=Alu.min)
                v.tensor_tensor(out=b4, in0=aview(ax2h, q), in1=gview(gx2p), op=Alu.min)
                v.tensor_tensor(out=a4, in0=b4, in1=a4, op=Alu.add)
                v.tensor_tensor(out=b4, in0=aview(nay1h, q), in1=gview(ngy1p), op=Alu.min)
                v.tensor_tensor(out=c4, in0=aview(ay2h, q), in1=gview(gy2p), op=Alu.min)
                v.tensor_tensor(out=b4, in0=c4, in1=b4, op=Alu.add)
                v.tensor_scalar(sA[:], sA[:], 0.0, None, Alu.max)
                v.tensor_scalar(sB[:], sB[:], 0.0, None, Alu.max)
                v.tensor_tensor(out=a4, in0=a4, in1=b4, op=Alu.mult)      # inter
                HW2 = QW // 2
                for hh in range(2):
                    s0q = q * QSUP + hh * (QSUP // 2)
                    hsl = slice(hh * HW2, (hh + 1) * HW2)
                    bh = (sB_l[q % 2][:, hsl]
                          .rearrange("p (s g u) -> p s g u", g=G, u=U))
                    ah = (sA_l[q % 2][:, hsl]
                          .rearrange("p (s g u) -> p s g u", g=G, u=U))
                    gh = (grid[:, q*QW + hh*HW2: q*QW + (hh+1)*HW2]
                          .rearrange("p (s g u) -> p s g u", g=G, u=U))
                    avh = (areaAh[:, s0q*U:(s0q + QSUP//2)*U]
                           .rearrange("p (s u) -> p s u", u=U)
                           .unsqueeze(2).to_broadcast([P, QSUP//2, G, U]))
                    gvh = (sGp[:].rearrange("p (g u) -> p g u", u=U)
                           .unsqueeze(1).to_broadcast([P, QSUP//2, G, U]))
                    v.tensor_tensor(out=bh, in0=avh, in1=gvh, op=Alu.add)
                    _act_recip(nc, sB_l[q % 2][:, hsl], sB_l[q % 2][:, hsl])
                    v.tensor_tensor(out=gh, in0=ah, in1=bh, op=Alu.mult)

                cur = gq
                width = G
                while width > 1:
                    h = width // 2
                    if h == 1:
                        dst = (rm[:, q*QSUP*U:(q+1)*QSUP*U]
                               .rearrange("p (s u) -> p s u", u=U).unsqueeze(2))
                    else:
                        dst = q4gu(sCt, 0)[:, :, 0:h, :]
                    v.tensor_tensor(out=dst, in0=cur[:, :, 0:h, :],
                                    in1=cur[:, :, h:width, :], op=Alu.max)
                    cur = dst
                    width = h
                curc = gq
                widc = U
                while widc > 1:
                    hc = widc // 2
                    if hc == 1:
                        dstc = (colT[:, q*QSUP*G:(q+1)*QSUP*G]
                                .rearrange("p (s g) -> p s g", g=G).unsqueeze(3))
                    else:
                        dstc = q4gu(sB, 0)[:, :, :, 0:hc]
                    v.tensor_tensor(out=dstc, in0=curc[:, :, :, 0:hc],
                                    in1=curc[:, :, :, hc:widc], op=Alu.max)
                    curc = dstc
                    widc = hc

            # ---------------- cmax finalize ----------------
            v.tensor_reduce(out=cpa[:], in_=colT[:].rearrange("p (s g) -> p g s", g=G),
                            axis=Ax.X, op=Alu.max)
            ct_ps = pspool.tile([G, P], FP16, name="ct_ps", tag="pst")
            pe.transpose(ct_ps[:], cpa[:], ident[:])
            v.tensor_copy(ctt[:], ct_ps[:])
            v.tensor_reduce(out=cmax_col[:], in_=ctt[:], axis=Ax.X, op=Alu.max)
            cm_ps = pspool.tile([1, G], FP16, name="cm_ps", tag="pst")
            pe.transpose(cm_ps[:], cmax_col[:], ident[:G, :G])
            v.tensor_copy(cm_row[:], cm_ps[:])
            bc_ps = pspool.tile([P, G], F32, name="bc_ps", tag="pss")
            nc.tensor.matmul(bc_ps[:], ones_row16[:], cm_row[:])
            v.tensor_copy(cmb[:], bc_ps[:])
            sc.activation(cmaxpl[:].rearrange("p (g u) -> p g u", u=U),
                          cmb[:].unsqueeze(2).to_broadcast([P, G, U]), Act.Copy)

            # ---------------- forced + pos ----------------
            for q in range(NQ):
                sA = sA_l[q % 2]
                gq = q4gu(grid, q)
                v.tensor_tensor(out=q4gu(sA, 0), in0=gq, in1=gview(cmaxpl),
                                op=Alu.is_equal)
                cur = q4gu(sA, 0)
                width = G
                while width > 1:
                    h = width // 2
                    if h == 1:
                        dst = (forced[:, q*QSUP*U:(q+1)*QSUP*U]
                               .rearrange("p (s u) -> p s u", u=U).unsqueeze(2))
                    else:
                        dst = cur[:, :, 0:h, :]
                    v.tensor_tensor(out=dst, in0=cur[:, :, 0:h, :],
                                    in1=cur[:, :, h:width, :], op=Alu.max)
                    cur = dst
                    width = h
            v.tensor_scalar(pos16[:], rm[:], POS_THR, None, Alu.is_gt)
            v.tensor_tensor(out=pos16[:], in0=pos16[:], in1=forced[:], op=Alu.max)
            v.tensor_scalar(sink16[:], pos16[:], 1.0, 0.0, Alu.mult, Alu.add,
                            accum_out=npp[:])
            v.tensor_copy(posf[:], pos16[:])

            # ---------------- ismax -> tgrid (s, u, g) ----------------
            for q in range(NQ):
                gin = grid[:, q*QW:(q+1)*QW].rearrange(
                    "p (s g u) -> p s u g", g=G, u=U)
                rmv = (rm[:, q*QSUP*U:(q+1)*QSUP*U]
                       .rearrange("p (s u) -> p s u", u=U)
                       .unsqueeze(3).to_broadcast([P, QSUP, U, G]))
                tout = tgrid[:, q*QW:(q+1)*QW].rearrange(
                    "p (s u g) -> p s u g", u=U, g=G)
                v.tensor_tensor(out=tout, in0=gin, in1=rmv, op=Alu.is_equal)

            # ---------------- matched coords (PE) ----------------
            for s in range(NSUP):
                tsb = tsb_l[s % 2]
                mout = pspool.tile([P, P], F32, name=f"mo{s % 2}", tag=f"mo{s % 2}")
                for j in range(8):
                    tp = pspool.tile([P, P], FP16, name=f"tp{j % 2}", tag=f"tp{j % 2}")
                    pe.transpose(tp[:], tgrid[:, s*W + j*P: s*W + (j+1)*P], ident[:])
                    sc.copy(tsb[:, j*P:(j+1)*P], tp[:])
                    nc.tensor.matmul(mout[:, j*16:(j+1)*16], tsb[:, j*P:(j+1)*P],
                                     gtmath[:])
                sc.copy(matched[:, s*P:(s+1)*P], mout[:])

            m4 = matched[:].rearrange("p (n c) -> p n c", c=4)
            for c in range(4):
                sc.copy(mch[c][:], m4[:, :, c:c+1].squeeze(2))

            # ---------------- bbox planes + diou ----------------
            bb3 = bbox_sb[:].rearrange("p (n c) -> p n c", c=4)
            for c in range(4):
                sc.activation(bxh[c][:], bb3[:, :, c:c+1].squeeze(2), Act.Copy,
                              scale=SC)
            v.tensor_tensor(out=d0[:], in0=bxh[2][:], in1=bxh[0][:], op=Alu.subtract)
            v.tensor_tensor(out=d1[:], in0=bxh[3][:], in1=bxh[1][:], op=Alu.subtract)
            v.tensor_tensor(out=areaPh[:], in0=d0[:], in1=d1[:], op=Alu.mult)

            # inter
            v.tensor_tensor(out=d0[:], in0=bxh[0][:], in1=mch[0][:], op=Alu.max)
            v.tensor_tensor(out=d1[:], in0=bxh[2][:], in1=mch[2][:], op=Alu.min)
            v.tensor_tensor(out=d0[:], in0=d1[:], in1=d0[:], op=Alu.subtract)
            v.tensor_scalar(d0[:], d0[:], 0.0, None, Alu.max)
            v.tensor_tensor(out=d1[:], in0=bxh[1][:], in1=mch[1][:], op=Alu.max)
            v.tensor_tensor(out=d2[:], in0=bxh[3][:], in1=mch[3][:], op=Alu.min)
            v.tensor_tensor(out=d1[:], in0=d2[:], in1=d1[:], op=Alu.subtract)
            v.tensor_scalar(d1[:], d1[:], 0.0, None, Alu.max)
            v.tensor_tensor(out=d0[:], in0=d0[:], in1=d1[:], op=Alu.mult)  # inter
            # matched area
            v.tensor_tensor(out=d1[:], in0=mch[2][:], in1=mch[0][:], op=Alu.subtract)
            v.tensor_tensor(out=d2[:], in0=mch[3][:], in1=mch[1][:], op=Alu.subtract)
            v.tensor_tensor(out=d1[:], in0=d1[:], in1=d2[:], op=Alu.mult)
            # union, iou
            v.tensor_tensor(out=d1[:], in0=d1[:], in1=areaPh[:], op=Alu.add)
            v.tensor_tensor(out=d1[:], in0=d1[:], in1=d0[:], op=Alu.subtract)
            _act_recip(nc, d1[:], d1[:])
            v.tensor_tensor(out=d0[:], in0=d0[:], in1=d1[:], op=Alu.mult)  # iou
            # enclosing c2
            v.tensor_tensor(out=d1[:], in0=bxh[0][:], in1=mch[0][:], op=Alu.min)
            v.tensor_tensor(out=d2[:], in0=bxh[2][:], in1=mch[2][:], op=Alu.max)
            v.tensor_tensor(out=d1[:], in0=d2[:], in1=d1[:], op=Alu.subtract)
            v.tensor_tensor(out=d1[:], in0=d1[:], in1=d1[:], op=Alu.mult)
            v.tensor_tensor(out=d2[:], in0=bxh[1][:], in1=mch[1][:], op=Alu.min)
            v.tensor_tensor(out=d3[:], in0=bxh[3][:], in1=mch[3][:], op=Alu.max)
            v.tensor_tensor(out=d2[:], in0=d3[:], in1=d2[:], op=Alu.subtract)
            v.tensor_tensor(out=d2[:], in0=d2[:], in1=d2[:], op=Alu.mult)
            v.tensor_tensor(out=d1[:], in0=d1[:], in1=d2[:], op=Alu.add)   # c2
            _act_recip(nc, d1[:], d1[:])
            # center dist d2 (quarter-scaled: absorbed by c2 ratio using same /2)
            v.tensor_tensor(out=d2[:], in0=bxh[0][:], in1=bxh[2][:], op=Alu.add)
            v.tensor_tensor(out=d3[:], in0=mch[0][:], in1=mch[2][:], op=Alu.add)
            v.tensor_tensor(out=d2[:], in0=d2[:], in1=d3[:], op=Alu.subtract)
            v.tensor_tensor(out=d2[:], in0=d2[:], in1=d2[:], op=Alu.mult)
            v.tensor_tensor(out=d3[:], in0=bxh[1][:], in1=bxh[3][:], op=Alu.add)
            v.tensor_tensor(out=d4[:], in0=mch[1][:], in1=mch[3][:], op=Alu.add)
            v.tensor_tensor(out=d3[:], in0=d3[:], in1=d4[:], op=Alu.subtract)
            v.tensor_tensor(out=d3[:], in0=d3[:], in1=d3[:], op=Alu.mult)
            v.tensor_tensor(out=d2[:], in0=d2[:], in1=d3[:], op=Alu.add)   # 4*d2
            v.tensor_tensor(out=d2[:], in0=d2[:], in1=d1[:], op=Alu.mult)
            v.tensor_scalar(d2[:], d2[:], 0.25, None, Alu.mult)            # d2/c2
            v.tensor_scalar(d0[:], d0[:], -1.0, 1.0, Alu.mult, Alu.add)    # 1-iou
            v.tensor_tensor(out=d2[:], in0=d2[:], in1=d0[:], op=Alu.add)
            v.tensor_scalar(d2[:], d2[:], 100.0, None, Alu.min)
            v.tensor_tensor(out=d2[:], in0=d2[:], in1=pos16[:], op=Alu.mult)
            v.tensor_scalar(sink16[:], d2[:], 1.0, 0.0, Alu.mult, Alu.add,
                            accum_out=locsum_pp[:])

            # ---------------- focal conf loss (f32, baseline) ----------------
            sc.activation(s0[:], conf_sb[:], Act.Sigmoid)
            sc.activation(s1[:], conf_sb[:], Act.Exp)
            sc.activation(s1[:], s1[:], Act.Ln, bias=1.0)
            v.tensor_tensor(out=s2[:], in0=conf_sb[:], in1=posf[:], op=Alu.mult)
            v.tensor_tensor(out=s2[:], in0=s1[:], in1=s2[:], op=Alu.subtract)
            v.tensor_scalar(s3[:], posf[:], -2.0, 1.0, Alu.mult, Alu.add)
            v.tensor_tensor(out=s3[:], in0=s0[:], in1=s3[:], op=Alu.mult)
            v.tensor_tensor(out=s3[:], in0=s3[:], in1=posf[:], op=Alu.add)
            sc.activation(s3[:], s3[:], Act.Square)
            v.tensor_tensor(out=cl[:], in0=s3[:], in1=s2[:], op=Alu.mult)
            v.tensor_scalar(s3[:], posf[:], -0.5, 0.75, Alu.mult, Alu.add)
            v.tensor_tensor(out=cl[:], in0=cl[:], in1=s3[:], op=Alu.mult)
            v.tensor_scalar(cl[:], cl[:], 100.0, None, Alu.min)
            v.tensor_tensor(out=s4[:], in0=cl[:], in1=posf[:], op=Alu.mult)
            v.tensor_scalar(s5[:], s4[:], 1.0, 0.0, Alu.mult, Alu.add,
                            accum_out=possum_pp[:])
            v.tensor_tensor(out=nv[:], in0=cl[:], in1=s4[:], op=Alu.subtract)
            v.tensor_copy(nv16[:], nv[:])

            # ---------------- hard negative mining (baseline) ----------------
            v.tensor_reduce(out=maxv_pp[:], in_=nv[:], axis=Ax.X, op=Alu.max)
            mx_ps = pspool.tile([1, P], F32, name="mx_ps", tag="pss")
            pe.transpose(mx_ps[:], maxv_pp[:], identf[:])
            v.tensor_copy(mx_row[:], mx_ps[:])
            v.tensor_reduce(out=maxv1[:], in_=mx_row[:], axis=Ax.X, op=Alu.max)

            np_ps = pspool.tile([1, 1], F32, name="np_ps", tag="pss")
            nc.tensor.matmul(np_ps[:], ones_col[:], npp[:])
            v.tensor_copy(npos1[:], np_ps[:])
            v.tensor_scalar(k1[:], npos1[:], NEG_POS_RATIO, None, Alu.mult)
            v.tensor_scalar(k2[:], npos1[:], -1.0, float(A), Alu.mult, Alu.add)
            v.tensor_tensor(out=kk[:], in0=k1[:], in1=k2[:], op=Alu.min)

            pbcast(maxvb[:], maxv1[:])
            v.tensor_scalar(w1c[:], maxvb[:], 1.0 / NBIN, None, Alu.mult)

            for lev in range(NLEV):
                if lev == 0:
                    v.tensor_copy(wl[0][:], w1c[:])
                    v.tensor_scalar(thr[:], iota_f[:], wl[0][:], None, Alu.mult)
                else:
                    v.tensor_scalar(wl[lev][:], wl[lev - 1][:], 1.0 / NBIN, None,
                                    Alu.mult)
                    v.tensor_scalar(thr[:], iota_f[:], wl[lev][:], lo_b[lev - 1][:],
                                    Alu.mult, Alu.add)
                v.tensor_scalar(nthr[:], thr[:], -1.0, None, Alu.mult)
                # split bins: even half on Act (sign-sum), odd half on DVE (is_gt)
                for bn in range(NBIN // 2):
                    sc.activation(sink16[:], nv16[:], Act.Sign,
                                  bias=nthr[:, bn:bn+1], accum_out=cge[:, bn:bn+1])
                for bn in range(NBIN // 2, NBIN):
                    v.tensor_scalar(d4[:], nv16[:], thr[:, bn:bn+1], 0.0,
                                    Alu.is_gt, Alu.add, accum_out=cge[:, bn:bn+1])
                cg_ps = pspool.tile([1, NBIN], F32, name="cg_ps", tag="pss")
                nc.tensor.matmul(cg_ps[:], ones_col[:], cge[:])
                v.tensor_copy(cget[:], cg_ps[:])
                # Act half holds sign-sums: cnt = (acc + A)/2; DVE half is exact
                v.tensor_scalar(cget[:, 0:NBIN // 2], cget[:, 0:NBIN // 2], 0.5,
                                float(A) * 0.5, Alu.mult, Alu.add)
                v.tensor_scalar(gek[:], cget[:], kk[:], None, Alu.is_ge)
                v.tensor_reduce(out=scnt[:], in_=gek[:], axis=Ax.X, op=Alu.add)
                v.tensor_scalar(lo_new[:], scnt[:], 1.0, wl[lev][0:1, :],
                                Alu.subtract, Alu.mult)
                v.tensor_scalar(tau[lev][:], scnt[:], wl[lev][0:1, :], None, Alu.mult)
                if lev > 0:
                    v.tensor_tensor(out=lo_new[:], in0=lo_new[:],
                                    in1=lo_b[lev - 1][0:1, :], op=Alu.add)
                    v.tensor_tensor(out=tau[lev][:], in0=tau[lev][:],
                                    in1=lo_b[lev - 1][0:1, :], op=Alu.add)
                pbcast(lo_b[lev][:], lo_new[:])

            pbcast(tau_b[:], tau[NLEV - 1][:])
            v.tensor_scalar(s4[:], nv[:], tau_b[:], 0.0, Alu.is_gt,
                            Alu.add, accum_out=cnt_pp[:])
            v.tensor_tensor(out=s5[:], in0=nv[:], in1=s4[:], op=Alu.mult)
            v.tensor_scalar(s5[:], s5[:], 1.0, 0.0, Alu.mult, Alu.add,
                            accum_out=sum_pp[:])

            # ---------------- gather scalars ----------------
            v.tensor_copy(stack[:, 0:1], npp[:])
            v.tensor_copy(stack[:, 1:2], locsum_pp[:])
            v.tensor_copy(stack[:, 2:3], possum_pp[:])
            v.tensor_copy(stack[:, 3:4], cnt_pp[:])
            st_ps = pspool.tile([1, 4], F32, name="st_ps", tag="pss")
            nc.tensor.matmul(st_ps[:], ones_col[:], stack[:])
            sm_ps = pspool.tile([1, 1], F32, name="sm_ps", tag="pss")
            nc.tensor.matmul(sm_ps[:], ones_col[:], sum_pp[:])

            v.tensor_copy(res_sb[:, 0:4], st_ps[:])
            v.tensor_copy(res_sb[:, 4:5], sm_ps[:])
            v.tensor_copy(res_sb[:, 5:6], tau[NLEV - 1][:])
            v.tensor_copy(res_sb[:, 6:7], maxv1[:])
            v.tensor_copy(res_sb[:, 7:8], kk[:])
            nc.sync.dma_start(res_d[b], res_sb[:])

    nc.compile()
    return nc


_NC_CACHE = None


def _get_nc():
    global _NC_CACHE
    if _NC_CACHE is None:
        _NC_CACHE = _build_nc()
    return _NC_CACHE


def _make_in_maps(inputs):
    bbox_pred = np.asarray(inputs["bbox_pred"])
    conf_pred = np.asarray(inputs["conf_pred"])
    anchors = np.asarray(inputs["anchors"])
    gt_boxes = np.asarray(inputs["gt_boxes"])
    anch_h = np.ascontiguousarray(anchors.reshape(P, COLS * 4), dtype=np.float32)
    in_maps = []
    for i in range(NCORE):
        bsl = slice(IMG * i, IMG * (i + 1))
        in_maps.append({
            "anch": anch_h,
            "bbox": np.ascontiguousarray(
                bbox_pred[bsl].reshape(IMG, P, COLS * 4), dtype=np.float32),
            "conf": np.ascontiguousarray(
                conf_pred[bsl].reshape(IMG, P, COLS), dtype=np.float32),
            "gtb": np.ascontiguousarray(
                gt_boxes[bsl].reshape(IMG, 1, G * 4), dtype=np.float32),
        })
    return in_maps


def kernel(bbox_pred, conf_pred, anchors, gt_boxes):
    nc = _get_nc()
    in_maps = _make_in_maps(dict(bbox_pred=bbox_pred, conf_pred=conf_pred,
                                 anchors=anchors, gt_boxes=gt_boxes))
    out = run_bass_kernel_spmd(nc, in_maps, core_ids=list(range(NCORE)))

    loc_total = np.float32(0.0)
    conf_total = np.float32(0.0)
    npos_total = np.float32(0.0)
    for i in range(NCORE):
        res = out.results[i]["res"]  # [IMG, 1, 8]
        for b in range(IMG):
            npos, locsum, possum, cnt_gt, sum_gt, tau_hi, maxv, kdev = \
                [np.float32(x) for x in res[b, 0, :8]]
            k = np.float32(min(NEG_POS_RATIO * npos, A - npos))
            wl_last = np.float32(maxv / NBIN ** NLEV)
            rem = max(np.float32(0.0), np.float32(k - cnt_gt))
            neg = np.float32(sum_gt + rem * (tau_hi - wl_last * np.float32(0.5)))
            loc_total = np.float32(loc_total + locsum)
            conf_total = np.float32(conf_total + possum + neg)
            npos_total = np.float32(npos_total + npos)
    num_pos = np.float32(max(1.0, npos_total))
    loc_loss = np.float32(loc_total / num_pos)
    conf_loss = np.float32(conf_total / num_pos)
    return (np.float32(loc_loss + conf_loss), conf_loss, loc_loss)


# revision 14
# speedup vs baseline: 1.5815x; 1.0262x over previous
"""Trainium2 Bass kernel v2 for nn_DetectionLoss — fp16 grid pipeline.

Data-parallel: 16 images over 8 cores (2 images/core). Per image, the
[A=65536, G=32] match grid is computed in fp16 (coords pre-scaled by 1/64 —
the loss is scale-invariant) in supertile-(g,u) layout so every DVE op is
packed-innermost and runs in 2x/4x mode. Division uses the Act engine's
table Reciprocal (~0.5% rel err — only feeds match *selection*, fine at the
2e-2 gate). Matched-gt coords go through fp16 PE transposes of the one-hot
grid + 16-wide block-diag matmuls. DIoU runs in fp16 on coord planes; focal
+ hard-negative mining keep the baseline f32/fp16 structure. Host combines
per-image scalars exactly like the reference."""
import sys

sys.path.insert(0, '/opt/trn_rl_repo')

import numpy as np
import concourse.bass as bass
import concourse.bacc as bacc
import concourse.mybir as mybir
from concourse.tile import TileContext
from concourse.bass_utils import run_bass_kernel_spmd
from concourse.masks import make_identity
from contextlib import ExitStack

Alu = mybir.AluOpType
Act = mybir.ActivationFunctionType
Ax = mybir.AxisListType
F32 = mybir.dt.float32
FP16 = mybir.dt.float16

P = 128
A = 65536
G = 32
IMG = 2
NCORE = 8
COLS = A // P       # 512
U = 32
W = G * U           # 1024 els per supertile block
NSUP = COLS // U    # 16
NQ = 4
QSUP = NSUP // NQ   # 4
QW = QSUP * W       # 4096
SC = 1.0 / 64.0
POS_THR = 1.0 / 3.0
NBIN = 16
NLEV = 2
NEG_POS_RATIO = 3.0


def _act_recip(nc, out, in_):
    """Raw Act-engine Reciprocal (table approx, ~0.5% rel err)."""
    sc = nc.scalar
    ins = [sc.lower_ap(in_)]
    for argv in (0.0, 1.0, 0.0):
        ins.append(mybir.ImmediateValue(dtype=mybir.dt.float32, value=argv))
    return sc.add_instruction(
        mybir.InstActivation(name=nc.get_next_instruction_name(),
                             func=Act.Reciprocal, ins=ins,
                             outs=[sc.lower_ap(out)]))


def _build_nc():
    nc = bacc.Bacc("TRN2", target_bir_lowering=False, debug=False)
    anch_d = nc.dram_tensor("anch", [P, COLS * 4], F32, kind="ExternalInput")
    bbox_d = nc.dram_tensor("bbox", [IMG, P, COLS * 4], F32, kind="ExternalInput")
    conf_d = nc.dram_tensor("conf", [IMG, P, COLS], F32, kind="ExternalInput")
    gt_d = nc.dram_tensor("gtb", [IMG, 1, G * 4], F32, kind="ExternalInput")
    res_d = nc.dram_tensor("res", [IMG, 1, 8], F32, kind="ExternalOutput")

    v = nc.vector
    sc = nc.scalar
    pe = nc.tensor

    with TileContext(nc) as tc, ExitStack() as ctx, \
            nc.allow_low_precision(reason="fp16 grid; host checks rel err"):
        pool = ctx.enter_context(tc.tile_pool(name="main", bufs=1))
        pspool = ctx.enter_context(tc.tile_pool(name="ps", bufs=1, space="PSUM"))

        def T(name, cols, parts=P, dt=F32):
            return pool.tile([parts, cols], dt, name=name)

        def T16(name, cols, parts=P):
            return pool.tile([parts, cols], FP16, name=name)

        # ---------------- per-core constants ----------------
        anch_sb = T("anch_sb", COLS * 4)
        nc.sync.dma_start(anch_sb[:, 0:COLS * 2], anch_d[:, 0:COLS * 2])
        nc.sync.dma_start(anch_sb[:, COLS * 2:], anch_d[:, COLS * 2:])
        anch3 = anch_sb[:].rearrange("p (n c) -> p n c", c=4)

        ax2h = T16("ax2h", COLS)
        ay2h = T16("ay2h", COLS)
        nax1h = T16("nax1h", COLS)
        nay1h = T16("nay1h", COLS)
        areaAh = T16("areaAh", COLS)
        f0 = T("f0", COLS)
        f1 = T("f1", COLS)
        sc.activation(ax2h[:], anch3[:, :, 2:3].squeeze(2), Act.Copy, scale=SC)
        sc.activation(ay2h[:], anch3[:, :, 3:4].squeeze(2), Act.Copy, scale=SC)
        sc.activation(nax1h[:], anch3[:, :, 0:1].squeeze(2), Act.Copy, scale=-SC)
        sc.activation(nay1h[:], anch3[:, :, 1:2].squeeze(2), Act.Copy, scale=-SC)
        v.tensor_tensor(out=f0[:], in0=anch3[:, :, 2:3].squeeze(2),
                        in1=anch3[:, :, 0:1].squeeze(2), op=Alu.subtract)
        v.tensor_tensor(out=f1[:], in0=anch3[:, :, 3:4].squeeze(2),
                        in1=anch3[:, :, 1:2].squeeze(2), op=Alu.subtract)
        v.tensor_tensor(out=f0[:], in0=f0[:], in1=f1[:], op=Alu.mult)
        sc.activation(areaAh[:], f0[:], Act.Copy, scale=SC * SC)

        ident = T16("ident", P)
        make_identity(nc, ident[:])
        identf = T("identf", P)
        make_identity(nc, identf[:])
        ones_col = T("ones_col", 1)
        ones_row = T("ones_row", P, parts=1)
        ones_row16 = T16("ones_row16", P, parts=1)
        v.memset(ones_col[:], 1.0)
        v.memset(ones_row[:], 1.0)
        v.memset(ones_row16[:], 1.0)
        iota_i = pool.tile([P, NBIN], mybir.dt.int32, name="iota_i")
        nc.gpsimd.iota(iota_i[:], pattern=[[1, NBIN]], base=0, channel_multiplier=0)
        iota_f = T("iota_f", NBIN)
        v.tensor_copy(iota_f[:], iota_i[:])

        def pbcast(dst, src_row):
            n = src_row.shape[-1]
            bc_ps = pspool.tile([P, G], F32, name="bc_ps", tag="pss")
            nc.tensor.matmul(bc_ps[:, 0:n], ones_row[:], src_row)
            v.tensor_copy(dst, bc_ps[:, 0:n])

        # ---------------- shared big tiles ----------------
        grid = T16("grid", NSUP * W)     # r values, (s, g, u) blocks
        tgrid = T16("tgrid", NSUP * W)   # one-hot, (s, u, g) blocks
        sA_l = [T16(f"sA{k}", QW) for k in range(2)]
        sB_l = [T16(f"sB{k}", QW) for k in range(2)]
        sCt_l = [T16("sCt0", QW)] * 2
        rm = T16("rm", COLS)
        colT = T16("colT", NSUP * G)
        cpa = T16("cpa", G)
        forced = T16("forced", COLS)
        pos16 = T16("pos16", COLS)
        posf = T("posf", COLS)
        cmb = T16("cmb", G)
        cmaxpl = T16("cmaxpl", W)

        gtrow_l = [T(f"gtrow{b}", G * 4, parts=1) for b in range(IMG)]
        gtall_l = [T(f"gtall{b}", G * 4) for b in range(IMG)]
        gsc = [T(f"gsc{c}", G) for c in range(4)]
        sGf = T("sGf", G)
        tG = T("tG", G)
        ngx1p = T16("ngx1p", W)
        ngy1p = T16("ngy1p", W)
        gx2p = T16("gx2p", W)
        gy2p = T16("gy2p", W)
        sGp = T16("sGp", W)
        gtmatf_l = [T(f"gtmatf{b}", 16) for b in range(IMG)]
        gtmath_l = [T16(f"gtmath{b}", 16) for b in range(IMG)]

        tsb_l = [T16(f"tsb{k}", W) for k in range(2)]
        matched = T16("matched", 4 * COLS)   # (s, u, c)
        mch = [T16(f"mch{c}", COLS) for c in range(4)]
        bxh = [T16(f"bxh{c}", COLS) for c in range(4)]
        areaPh = T16("areaPh", COLS)
        bbox_sb_l = [T(f"bbox_sb{b}", COLS * 4) for b in range(IMG)]
        conf_sb_l = [T(f"conf_sb{b}", COLS) for b in range(IMG)]

        d0 = T16("d0", COLS)
        d1 = T16("d1", COLS)
        d2 = T16("d2", COLS)
        d3 = T16("d3", COLS)
        d4 = T16("d4", COLS)

        s0 = T("s0", COLS)
        s1 = T("s1", COLS)
        s2 = T("s2", COLS)
        s3 = T("s3", COLS)
        s4 = T("s4", COLS)
        s5 = T("s5", COLS)
        cl = T("cl", COLS)
        nv = T("nv", COLS)
        nv16 = T16("nv16", COLS)
        sink16 = T16("sink16", COLS)

        npp = T("npp", 1)
        locsum_pp = T("locsum_pp", 1)
        possum_pp = T("possum_pp", 1)
        cnt_pp = T("cnt_pp", 1)
        sum_pp = T("sum_pp", 1)
        maxv_pp = T("maxv_pp", 1)
        maxvb = T("maxvb", 1)
        w1c = T("w1c", 1)
        tau_b = T("tau_b", 1)
        stack = T("stack", 4)
        thr = T("thr", NBIN)
        nthr = T("nthr", NBIN)
        cge = T("cge", NBIN)
        wl = [T(f"wl{l}", 1) for l in range(NLEV)]
        lo_b = [T(f"lo_b{l}", 1) for l in range(NLEV)]
        cget = T("cget", NBIN, parts=1)
        gek = T("gek", NBIN, parts=1)
        scnt = T("scnt", 1, parts=1)
        lo_new = T("lo_new", 1, parts=1)
        tau = [T(f"tau{l}", 1, parts=1) for l in range(NLEV)]
        maxv1 = T("maxv1", 1, parts=1)
        npos1 = T("npos1", 1, parts=1)
        k1 = T("k1", 1, parts=1)
        k2 = T("k2", 1, parts=1)
        kk = T("kk", 1, parts=1)
        mx_row = T("mx_row", P, parts=1)
        ctt = T16("ctt", P, parts=G)
        cmax_col = T16("cmax_col", 1, parts=G)
        cm_row = T16("cm_row", G, parts=1)
        res_sb = T("res_sb", 8, parts=1)

        # 4D view helpers
        def q4gu(t, q):  # (s, g, u) packed quarter
            return t[:, q*QW:(q+1)*QW].rearrange("p (s g u) -> p s g u", g=G, u=U)

        def aview(t, q):  # anchor [P, COLS] -> [p, s, g(b), u]
            return (t[:, q*QSUP*U:(q+1)*QSUP*U]
                    .rearrange("p (s u) -> p s u", u=U)
                    .unsqueeze(2).to_broadcast([P, QSUP, G, U]))

        def gview(t):     # gt plane [P, W] -> [p, s(b), g, u]
            return (t[:].rearrange("p (g u) -> p g u", u=U)
                    .unsqueeze(1).to_broadcast([P, QSUP, G, U]))

        for b in range(IMG):
            bbox_sb = bbox_sb_l[b]
            conf_sb = conf_sb_l[b]
            gtrow = gtrow_l[b]
            gtall = gtall_l[b]
            gtmatf = gtmatf_l[b]
            gtmath = gtmath_l[b]
            # ---------------- loads ----------------
            nc.sync.dma_start(bbox_sb[:, 0:COLS * 2], bbox_d[b][:, 0:COLS * 2])
            nc.sync.dma_start(bbox_sb[:, COLS * 2:], bbox_d[b][:, COLS * 2:])
            nc.sync.dma_start(conf_sb[:], conf_d[b])
            nc.scalar.dma_start(gtrow[:], gt_d[b])
            gt_ps = pspool.tile([P, G * 4], F32, name="gt_ps", tag="gtp")
            nc.tensor.matmul(gt_ps[:], ones_row[:], gtrow[:])
            v.tensor_copy(gtall[:], gt_ps[:])
            gt3 = gtall[:].rearrange("p (g c) -> p g c", c=4)
            gt2d = gt_d[b].rearrange("q (g c) -> (q g) c", c=4)
            v.memset(gtmatf[:], 0.0)
            for uu in range(4):
                nc.scalar.dma_start(gtmatf[uu*G:(uu+1)*G, uu*4:(uu+1)*4], gt2d)
            sc.activation(gtmath[:], gtmatf[:], Act.Copy, scale=SC)

            # ---------------- gt prep ----------------
            v.tensor_scalar(gsc[0][:], gt3[:, :, 0:1].squeeze(2), -SC, None, Alu.mult)
            v.tensor_scalar(gsc[1][:], gt3[:, :, 1:2].squeeze(2), -SC, None, Alu.mult)
            v.tensor_scalar(gsc[2][:], gt3[:, :, 2:3].squeeze(2), SC, None, Alu.mult)
            v.tensor_scalar(gsc[3][:], gt3[:, :, 3:4].squeeze(2), SC, None, Alu.mult)
            v.tensor_tensor(out=sGf[:], in0=gt3[:, :, 2:3].squeeze(2),
                            in1=gt3[:, :, 0:1].squeeze(2), op=Alu.subtract)
            v.tensor_tensor(out=tG[:], in0=gt3[:, :, 3:4].squeeze(2),
                            in1=gt3[:, :, 1:2].squeeze(2), op=Alu.subtract)
            v.tensor_tensor(out=sGf[:], in0=sGf[:], in1=tG[:], op=Alu.mult)
            v.tensor_scalar(sGf[:], sGf[:], SC * SC, None, Alu.mult)
            for pl, src in ((ngx1p, gsc[0]), (ngy1p, gsc[1]), (gx2p, gsc[2]),
                            (gy2p, gsc[3]), (sGp, sGf)):
                sc.activation(pl[:].rearrange("p (g u) -> p g u", u=U),
                              src[:].unsqueeze(2).to_broadcast([P, G, U]), Act.Copy)

            # ---------------- pass 1: grid ----------------
            for q in range(NQ):
                sA = sA_l[q % 2]
                sB = sB_l[q % 2]
                sCt = sCt_l[q % 2]
                gq = q4gu(grid, q)
                a4 = q4gu(sA, 0)
                b4 = q4gu(sB, 0)
                c4 = q4gu(sCt, 0)
                v.tensor_tensor(out=a4, in0=aview(nax1h, q), in1=gview(ngx1p), op=Alu.min)
                v.tensor_tensor(out=b4, in0=aview(ax2h, q), in1=gview(gx2p), op=Alu.min)
                v.tensor_tensor(out=a4, in0=b4, in1=a4, op=Alu.add)
                v.tensor_tensor(out=b4, in0=aview(nay1h, q), in1=gview(ngy1p), op=Alu.min)
                v.tensor_tensor(out=c4, in0=aview(ay2h, q), in1=gview(gy2p), op=Alu.min)
                v.tensor_tensor(out=b4, in0=c4, in1=b4, op=Alu.add)
                v.tensor_scalar(sA[:], sA[:], 0.0, None, Alu.max)
                v.tensor_scalar(sB[:], sB[:], 0.0, None, Alu.max)
                v.tensor_tensor(out=a4, in0=a4, in1=b4, op=Alu.mult)      # inter
                HW2 = QW // 2
                for hh in range(2):
                    s0q = q * QSUP + hh * (QSUP // 2)
                    hsl = slice(hh * HW2, (hh + 1) * HW2)
                    bh = (sB_l[q % 2][:, hsl]
                          .rearrange("p (s g u) -> p s g u", g=G, u=U))
                    ah = (sA_l[q % 2][:, hsl]
                          .rearrange("p (s g u) -> p s g u", g=G, u=U))
                    gh = (grid[:, q*QW + hh*HW2: q*QW + (hh+1)*HW2]
                          .rearrange("p (s g u) -> p s g u", g=G, u=U))
                    avh = (areaAh[:, s0q*U:(s0q + QSUP//2)*U]
                           .rearrange("p (s u) -> p s u", u=U)
                           .unsqueeze(2).to_broadcast([P, QSUP//2, G, U]))
                    gvh = (sGp[:].rearrange("p (g u) -> p g u", u=U)
                           .unsqueeze(1).to_broadcast([P, QSUP//2, G, U]))
                    v.tensor_tensor(out=bh, in0=avh, in1=gvh, op=Alu.add)
                    _act_recip(nc, sB_l[q % 2][:, hsl], sB_l[q % 2][:, hsl])
                    v.tensor_tensor(out=gh, in0=ah, in1=bh, op=Alu.mult)

                cur = gq
                width = G
                while width > 1:
                    h = width // 2
                    if h == 1:
                        dst = (rm[:, q*QSUP*U:(q+1)*QSUP*U]
                               .rearrange("p (s u) -> p s u", u=U).unsqueeze(2))
                    else:
                        dst = q4gu(sCt, 0)[:, :, 0:h, :]
                    v.tensor_tensor(out=dst, in0=cur[:, :, 0:h, :],
                                    in1=cur[:, :, h:width, :], op=Alu.max)
                    cur = dst
                    width = h
                curc = gq
                widc = U
                while widc > 1:
                    hc = widc // 2
                    if hc == 1:
                        dstc = (colT[:, q*QSUP*G:(q+1)*QSUP*G]
                                .rearrange("p (s g) -> p s g", g=G).unsqueeze(3))
                    else:
                        dstc = q4gu(sB, 0)[:, :, :, 0:hc]
                    v.tensor_tensor(out=dstc, in0=curc[:, :, :, 0:hc],
                                    in1=curc[:, :, :, hc:widc], op=Alu.max)
                    curc = dstc
                    widc = hc

            # ---------------- cmax finalize ----------------
            v.tensor_reduce(out=cpa[:], in_=colT[:].rearrange("p (s g) -> p g s", g=G),
                            axis=Ax.X, op=Alu.max)
            ct_ps = pspool.tile([G, P], FP16, name="ct_ps", tag="pst")
            pe.transpose(ct_ps[:], cpa[:], ident[:])
            v.tensor_copy(ctt[:], ct_ps[:])
            v.tensor_reduce(out=cmax_col[:], in_=ctt[:], axis=Ax.X, op=Alu.max)
            cm_ps = pspool.tile([1, G], FP16, name="cm_ps", tag="pst")
            pe.transpose(cm_ps[:], cmax_col[:], ident[:G, :G])
            v.tensor_copy(cm_row[:], cm_ps[:])
            bc_ps = pspool.tile([P, G], F32, name="bc_ps", tag="pss")
            nc.tensor.matmul(bc_ps[:], ones_row16[:], cm_row[:])
            v.tensor_copy(cmb[:], bc_ps[:])
            sc.activation(cmaxpl[:].rearrange("p (g u) -> p g u", u=U),
                          cmb[:].unsqueeze(2).to_broadcast([P, G, U]), Act.Copy)

            # ---------------- forced + pos ----------------
            for q in range(NQ):
                sA = sA_l[q % 2]
                gq = q4gu(grid, q)
                v.tensor_tensor(out=q4gu(sA, 0), in0=gq, in1=gview(cmaxpl),
                                op=Alu.is_equal)
                cur = q4gu(sA, 0)
                width = G
                while width > 1:
                    h = width // 2
                    if h == 1:
                        dst = (forced[:, q*QSUP*U:(q+1)*QSUP*U]
                               .rearrange("p (s u) -> p s u", u=U).unsqueeze(2))
                    else:
                        dst = cur[:, :, 0:h, :]
                    v.tensor_tensor(out=dst, in0=cur[:, :, 0:h, :],
                                    in1=cur[:, :, h:width, :], op=Alu.max)
                    cur = dst
                    width = h
            v.tensor_scalar(pos16[:], rm[:], POS_THR, None, Alu.is_gt)
            v.tensor_tensor(out=pos16[:], in0=pos16[:], in1=forced[:], op=Alu.max)
            v.tensor_scalar(sink16[:], pos16[:], 1.0, 0.0, Alu.mult, Alu.add,
                            accum_out=npp[:])
            v.tensor_copy(posf[:], pos16[:])

            # ---------------- ismax -> tgrid (s, u, g) ----------------
            for q in range(NQ):
                gin = grid[:, q*QW:(q+1)*QW].rearrange(
                    "p (s g u) -> p s u g", g=G, u=U)
                rmv = (rm[:, q*QSUP*U:(q+1)*QSUP*U]
                       .rearrange("p (s u) -> p s u", u=U)
                       .unsqueeze(3).to_broadcast([P, QSUP, U, G]))
                tout = tgrid[:, q*QW:(q+1)*QW].rearrange(
                    "p (s u g) -> p s u g", u=U, g=G)
                v.tensor_tensor(out=tout, in0=gin, in1=rmv, op=Alu.is_equal)

            # ---------------- matched coords (PE) ----------------
            for s in range(NSUP):
                tsb = tsb_l[s % 2]
                mout = pspool.tile([P, P], F32, name=f"mo{s % 2}", tag=f"mo{s % 2}")
                for j in range(8):
                    tp = pspool.tile([P, P], FP16, name=f"tp{j % 2}", tag=f"tp{j % 2}")
                    pe.transpose(tp[:], tgrid[:, s*W + j*P: s*W + (j+1)*P], ident[:])
                    sc.copy(tsb[:, j*P:(j+1)*P], tp[:])
                    nc.tensor.matmul(mout[:, j*16:(j+1)*16], tsb[:, j*P:(j+1)*P],
                                     gtmath[:])
                sc.copy(matched[:, s*P:(s+1)*P], mout[:])

            m4 = matched[:].rearrange("p (n c) -> p n c", c=4)
            for c in range(4):
                sc.copy(mch[c][:], m4[:, :, c:c+1].squeeze(2))

            # ---------------- bbox planes + diou ----------------
            bb3 = bbox_sb[:].rearrange("p (n c) -> p n c", c=4)
            for c in range(4):
                sc.activation(bxh[c][:], bb3[:, :, c:c+1].squeeze(2), Act.Copy,
                              scale=SC)
            v.tensor_tensor(out=d0[:], in0=bxh[2][:], in1=bxh[0][:], op=Alu.subtract)
            v.tensor_tensor(out=d1[:], in0=bxh[3][:], in1=bxh[1][:], op=Alu.subtract)
            v.tensor_tensor(out=areaPh[:], in0=d0[:], in1=d1[:], op=Alu.mult)

            # inter
            v.tensor_tensor(out=d0[:], in0=bxh[0][:], in1=mch[0][:], op=Alu.max)
            v.tensor_tensor(out=d1[:], in0=bxh[2][:], in1=mch[2][:], op=Alu.min)
            v.tensor_tensor(out=d0[:], in0=d1[:], in1=d0[:], op=Alu.subtract)
            v.tensor_scalar(d0[:], d0[:], 0.0, None, Alu.max)
            v.tensor_tensor(out=d1[:], in0=bxh[1][:], in1=mch[1][:], op=Alu.max)
            v.tensor_tensor(out=d2[:], in0=bxh[3][:], in1=mch[3][:], op=Alu.min)
            v.tensor_tensor(out=d1[:], in0=d2[:], in1=d1[:], op=Alu.subtract)
            v.tensor_scalar(d1[:], d1[:], 0.0, None, Alu.max)
            v.tensor_tensor(out=d0[:], in0=d0[:], in1=d1[:], op=Alu.mult)  # inter
            # matched area
            v.tensor_tensor(out=d1[:], in0=mch[2][:], in1=mch[0][:], op=Alu.subtract)
            v.tensor_tensor(out=d2[:], in0=mch[3][:], in1=mch[1][:], op=Alu.subtract)
            v.tensor_tensor(out=d1[:], in0=d1[:], in1=d2[:], op=Alu.mult)
            # union, iou
            v.tensor_tensor(out=d1[:], in0=d1[:], in1=areaPh[:], op=Alu.add)
            v.tensor_tensor(out=d1[:], in0=d1[:], in1=d0[:], op=Alu.subtract)
            _act_recip(nc, d1[:], d1[:])
            v.tensor_tensor(out=d0[:], in0=d0[:], in1=d1[:], op=Alu.mult)  # iou
            # enclosing c2
            v.tensor_tensor(out=d1[:], in0=bxh[0][:], in1=mch[0][:], op=Alu.min)
            v.tensor_tensor(out=d2[:], in0=bxh[2][:], in1=mch[2][:], op=Alu.max)
            v.tensor_tensor(out=d1[:], in0=d2[:], in1=d1[:], op=Alu.subtract)
            v.tensor_tensor(out=d1[:], in0=d1[:], in1=d1[:], op=Alu.mult)
            v.tensor_tensor(out=d2[:], in0=bxh[1][:], in1=mch[1][:], op=Alu.min)
            v.tensor_tensor(out=d3[:], in0=bxh[3][:], in1=mch[3][:], op=Alu.max)
            v.tensor_tensor(out=d2[:], in0=d3[:], in1=d2[:], op=Alu.subtract)
            v.tensor_tensor(out=d2[:], in0=d2[:], in1=d2[:], op=Alu.mult)
            v.tensor_tensor(out=d1[:], in0=d1[:], in1=d2[:], op=Alu.add)   # c2
            _act_recip(nc, d1[:], d1[:])
            # center dist d2 (quarter-scaled: absorbed by c2 ratio using same /2)
            v.tensor_tensor(out=d2[:], in0=bxh[0][:], in1=bxh[2][:], op=Alu.add)
            v.tensor_tensor(out=d3[:], in0=mch[0][:], in1=mch[2][:], op=Alu.add)
            v.tensor_tensor(out=d2[:], in0=d2[:], in1=d3[:], op=Alu.subtract)
            v.tensor_tensor(out=d2[:], in0=d2[:], in1=d2[:], op=Alu.mult)
            v.tensor_tensor(out=d3[:], in0=bxh[1][:], in1=bxh[3][:], op=Alu.add)
            v.tensor_tensor(out=d4[:], in0=mch[1][:], in1=mch[3][:], op=Alu.add)
            v.tensor_tensor(out=d3[:], in0=d3[:], in1=d4[:], op=Alu.subtract)
            v.tensor_tensor(out=d3[:], in0=d3[:], in1=d3[:], op=Alu.mult)
            v.tensor_tensor(out=d2[:], in0=d2[:], in1=d3[:], op=Alu.add)   # 4*d2
            v.tensor_tensor(out=d2[:], in0=d2[:], in1=d1[:], op=Alu.mult)
            v.tensor_scalar(d2[:], d2[:], 0.25, None, Alu.mult)            # d2/c2
            v.tensor_scalar(d0[:], d0[:], -1.0, 1.0, Alu.mult, Alu.add)    # 1-iou
            v.tensor_tensor(out=d2[:], in0=d2[:], in1=d0[:], op=Alu.add)
            v.tensor_scalar(d2[:], d2[:], 100.0, None, Alu.min)
            v.tensor_tensor(out=d2[:], in0=d2[:], in1=pos16[:], op=Alu.mult)
            v.tensor_scalar(sink16[:], d2[:], 1.0, 0.0, Alu.mult, Alu.add,
                            accum_out=locsum_pp[:])

            # ---------------- focal conf loss (f32, baseline) ----------------
            sc.activation(s0[:], conf_sb[:], Act.Sigmoid)
            sc.activation(s1[:], conf_sb[:], Act.Exp)
            sc.activation(s1[:], s1[:], Act.Ln, bias=1.0)
            v.tensor_tensor(out=s2[:], in0=conf_sb[:], in1=posf[:], op=Alu.mult)
            v.tensor_tensor(out=s2[:], in0=s1[:], in1=s2[:], op=Alu.subtract)
            v.tensor_scalar(s3[:], posf[:], -2.0, 1.0, Alu.mult, Alu.add)
            v.tensor_tensor(out=s3[:], in0=s0[:], in1=s3[:], op=Alu.mult)
            v.tensor_tensor(out=s3[:], in0=s3[:], in1=posf[:], op=Alu.add)
            sc.activation(s3[:], s3[:], Act.Square)
            v.tensor_tensor(out=cl[:], in0=s3[:], in1=s2[:], op=Alu.mult)
            v.tensor_scalar(s3[:], posf[:], -0.5, 0.75, Alu.mult, Alu.add)
            v.tensor_tensor(out=cl[:], in0=cl[:], in1=s3[:], op=Alu.mult)
            v.tensor_scalar(cl[:], cl[:], 100.0, None, Alu.min)
            v.tensor_tensor(out=s4[:], in0=cl[:], in1=posf[:], op=Alu.mult)
            v.tensor_scalar(s5[:], s4[:], 1.0, 0.0, Alu.mult, Alu.add,
                            accum_out=possum_pp[:])
            v.tensor_tensor(out=nv[:], in0=cl[:], in1=s4[:], op=Alu.subtract)
            v.tensor_copy(nv16[:], nv[:])

            # ---------------- hard negative mining (baseline) ----------------
            v.tensor_reduce(out=maxv_pp[:], in_=nv[:], axis=Ax.X, op=Alu.max)
            mx_ps = pspool.tile([1, P], F32, name="mx_ps", tag="pss")
            pe.transpose(mx_ps[:], maxv_pp[:], identf[:])
            v.tensor_copy(mx_row[:], mx_ps[:])
            v.tensor_reduce(out=maxv1[:], in_=mx_row[:], axis=Ax.X, op=Alu.max)

            np_ps = pspool.tile([1, 1], F32, name="np_ps", tag="pss")
            nc.tensor.matmul(np_ps[:], ones_col[:], npp[:])
            v.tensor_copy(npos1[:], np_ps[:])
            v.tensor_scalar(k1[:], npos1[:], NEG_POS_RATIO, None, Alu.mult)
            v.tensor_scalar(k2[:], npos1[:], -1.0, float(A), Alu.mult, Alu.add)
            v.tensor_tensor(out=kk[:], in0=k1[:], in1=k2[:], op=Alu.min)

            pbcast(maxvb[:], maxv1[:])
            v.tensor_scalar(w1c[:], maxvb[:], 1.0 / NBIN, None, Alu.mult)

            for lev in range(NLEV):
                if lev == 0:
                    v.tensor_copy(wl[0][:], w1c[:])
                    v.tensor_scalar(thr[:], iota_f[:], wl[0][:], None, Alu.mult)
                else:
                    v.tensor_scalar(wl[lev][:], wl[lev - 1][:], 1.0 / NBIN, None,
                                    Alu.mult)
                    v.tensor_scalar(thr[:], iota_f[:], wl[lev][:], lo_b[lev - 1][:],
                                    Alu.mult, Alu.add)
                v.tensor_scalar(nthr[:], thr[:], -1.0, None, Alu.mult)
                # img0 mining overlaps img1 grid (DVE-bound): all bins on Act.
                # img1 mining is the tail: split bins across Act and DVE.
                nact = NBIN if b == 0 else NBIN // 2
                for bn in range(nact):
                    sc.activation(sink16[:], nv16[:], Act.Sign,
                                  bias=nthr[:, bn:bn+1], accum_out=cge[:, bn:bn+1])
                for bn in range(nact, NBIN):
                    v.tensor_scalar(d4[:], nv16[:], thr[:, bn:bn+1], 0.0,
                                    Alu.is_gt, Alu.add, accum_out=cge[:, bn:bn+1])
                cg_ps = pspool.tile([1, NBIN], F32, name="cg_ps", tag="pss")
                nc.tensor.matmul(cg_ps[:], ones_col[:], cge[:])
                v.tensor_copy(cget[:], cg_ps[:])
                # Act bins hold sign-sums: cnt = (acc + A)/2; DVE bins exact
                v.tensor_scalar(cget[:, 0:nact], cget[:, 0:nact], 0.5,
                                float(A) * 0.5, Alu.mult, Alu.add)
                v.tensor_scalar(gek[:], cget[:], kk[:], None, Alu.is_ge)
                v.tensor_reduce(out=scnt[:], in_=gek[:], axis=Ax.X, op=Alu.add)
                v.tensor_scalar(lo_new[:], scnt[:], 1.0, wl[lev][0:1, :],
                                Alu.subtract, Alu.mult)
                v.tensor_scalar(tau[lev][:], scnt[:], wl[lev][0:1, :], None, Alu.mult)
                if lev > 0:
                    v.tensor_tensor(out=lo_new[:], in0=lo_new[:],
                                    in1=lo_b[lev - 1][0:1, :], op=Alu.add)
                    v.tensor_tensor(out=tau[lev][:], in0=tau[lev][:],
                                    in1=lo_b[lev - 1][0:1, :], op=Alu.add)
                pbcast(lo_b[lev][:], lo_new[:])

            pbcast(tau_b[:], tau[NLEV - 1][:])
            v.tensor_scalar(s4[:], nv[:], tau_b[:], 0.0, Alu.is_gt,
                            Alu.add, accum_out=cnt_pp[:])
            v.tensor_tensor(out=s5[:], in0=nv[:], in1=s4[:], op=Alu.mult)
            v.tensor_scalar(s5[:], s5[:], 1.0, 0.0, Alu.mult, Alu.add,
                            accum_out=sum_pp[:])

            # ---------------- gather scalars ----------------
            v.tensor_copy(stack[:, 0:1], npp[:])
            v.tensor_copy(stack[:, 1:2], locsum_pp[:])
            v.tensor_copy(stack[:, 2:3], possum_pp[:])
            v.tensor_copy(stack[:, 3:4], cnt_pp[:])
            st_ps = pspool.tile([1, 4], F32, name="st_ps", tag="pss")
            nc.tensor.matmul(st_ps[:], ones_col[:], stack[:])
            sm_ps = pspool.tile([1, 1], F32, name="sm_ps", tag="pss")
            nc.tensor.matmul(sm_ps[:], ones_col[:], sum_pp[:])

            v.tensor_copy(res_sb[:, 0:4], st_ps[:])
            v.tensor_copy(res_sb[:, 4:5], sm_ps[:])
            v.tensor_copy(res_sb[:, 5:6], tau[NLEV - 1][:])
            v.tensor_copy(res_sb[:, 6:7], maxv1[:])
            v.tensor_copy(res_sb[:, 7:8], kk[:])
            nc.sync.dma_start(res_d[b], res_sb[:])

    nc.compile()
    return nc


_NC_CACHE = None


def _get_nc():
    global _NC_CACHE
    if _NC_CACHE is None:
        _NC_CACHE = _build_nc()
    return _NC_CACHE


def _make_in_maps(inputs):
    bbox_pred = np.asarray(inputs["bbox_pred"])
    conf_pred = np.asarray(inputs["conf_pred"])
    anchors = np.asarray(inputs["anchors"])
    gt_boxes = np.asarray(inputs["gt_boxes"])
    anch_h = np.ascontiguousarray(anchors.reshape(P, COLS * 4), dtype=np.float32)
    in_maps = []
    for i in range(NCORE):
        bsl = slice(IMG * i, IMG * (i + 1))
        in_maps.append({
            "anch": anch_h,
            "bbox": np.ascontiguousarray(
                bbox_pred[bsl].reshape(IMG, P, COLS * 4), dtype=np.float32),
            "conf": np.ascontiguousarray(
                conf_pred[bsl].reshape(IMG, P, COLS), dtype=np.float32),
            "gtb": np.ascontiguousarray(
                gt_boxes[bsl].reshape(IMG, 1, G * 4), dtype=np.float32),
        })
    return in_maps


def kernel(bbox_pred, conf_pred, anchors, gt_boxes):
    nc = _get_nc()
    in_maps = _make_in_maps(dict(bbox_pred=bbox_pred, conf_pred=conf_pred,
                                 anchors=anchors, gt_boxes=gt_boxes))
    out = run_bass_kernel_spmd(nc, in_maps, core_ids=list(range(NCORE)))

    loc_total = np.float32(0.0)
    conf_total = np.float32(0.0)
    npos_total = np.float32(0.0)
    for i in range(NCORE):
        res = out.results[i]["res"]  # [IMG, 1, 8]
        for b in range(IMG):
            npos, locsum, possum, cnt_gt, sum_gt, tau_hi, maxv, kdev = \
                [np.float32(x) for x in res[b, 0, :8]]
            k = np.float32(min(NEG_POS_RATIO * npos, A - npos))
            wl_last = np.float32(maxv / NBIN ** NLEV)
            rem = max(np.float32(0.0), np.float32(k - cnt_gt))
            neg = np.float32(sum_gt + rem * (tau_hi - wl_last * np.float32(0.5)))
            loc_total = np.float32(loc_total + locsum)
            conf_total = np.float32(conf_total + possum + neg)
            npos_total = np.float32(npos_total + npos)
    num_pos = np.float32(max(1.0, npos_total))
    loc_loss = np.float32(loc_total / num_pos)
    conf_loss = np.float32(conf_total / num_pos)
    return (np.float32(loc_loss + conf_loss), conf_loss, loc_loss)


# revision 15
# speedup vs baseline: 1.6110x; 1.0187x over previous
"""Trainium2 Bass kernel v2 for nn_DetectionLoss — fp16 grid pipeline.

Data-parallel: 16 images over 8 cores (2 images/core). Per image, the
[A=65536, G=32] match grid is computed in fp16 (coords pre-scaled by 1/64 —
the loss is scale-invariant) in supertile-(g,u) layout so every DVE op is
packed-innermost and runs in 2x/4x mode. Division uses the Act engine's
table Reciprocal (~0.5% rel err — only feeds match *selection*, fine at the
2e-2 gate). Matched-gt coords go through fp16 PE transposes of the one-hot
grid + 16-wide block-diag matmuls. DIoU runs in fp16 on coord planes; focal
+ hard-negative mining keep the baseline f32/fp16 structure. Host combines
per-image scalars exactly like the reference."""
import sys

sys.path.insert(0, '/opt/trn_rl_repo')

import numpy as np
import concourse.bass as bass
import concourse.bacc as bacc
import concourse.mybir as mybir
from concourse.tile import TileContext
from concourse.bass_utils import run_bass_kernel_spmd
from concourse.masks import make_identity
from contextlib import ExitStack

Alu = mybir.AluOpType
Act = mybir.ActivationFunctionType
Ax = mybir.AxisListType
F32 = mybir.dt.float32
FP16 = mybir.dt.float16

P = 128
A = 65536
G = 32
IMG = 2
NCORE = 8
COLS = A // P       # 512
U = 32
W = G * U           # 1024 els per supertile block
NSUP = COLS // U    # 16
NQ = 4
QSUP = NSUP // NQ   # 4
QW = QSUP * W       # 4096
SC = 1.0 / 64.0
POS_THR = 1.0 / 3.0
NBIN = 16
NLEV = 2
NEG_POS_RATIO = 3.0


def _act_recip(nc, out, in_):
    """Raw Act-engine Reciprocal (table approx, ~0.5% rel err)."""
    sc = nc.scalar
    ins = [sc.lower_ap(in_)]
    for argv in (0.0, 1.0, 0.0):
        ins.append(mybir.ImmediateValue(dtype=mybir.dt.float32, value=argv))
    return sc.add_instruction(
        mybir.InstActivation(name=nc.get_next_instruction_name(),
                             func=Act.Reciprocal, ins=ins,
                             outs=[sc.lower_ap(out)]))


def _build_nc():
    nc = bacc.Bacc("TRN2", target_bir_lowering=False, debug=False)
    anch_d = nc.dram_tensor("anch", [P, COLS * 4], F32, kind="ExternalInput")
    bbox_d = nc.dram_tensor("bbox", [IMG, P, COLS * 4], F32, kind="ExternalInput")
    conf_d = nc.dram_tensor("conf", [IMG, P, COLS], F32, kind="ExternalInput")
    gt_d = nc.dram_tensor("gtb", [IMG, 1, G * 4], F32, kind="ExternalInput")
    res_d = nc.dram_tensor("res", [IMG, 1, 8], F32, kind="ExternalOutput")

    v = nc.vector
    sc = nc.scalar
    pe = nc.tensor

    with TileContext(nc) as tc, ExitStack() as ctx, \
            nc.allow_low_precision(reason="fp16 grid; host checks rel err"):
        pool = ctx.enter_context(tc.tile_pool(name="main", bufs=1))
        pspool = ctx.enter_context(tc.tile_pool(name="ps", bufs=1, space="PSUM"))

        def T(name, cols, parts=P, dt=F32):
            return pool.tile([parts, cols], dt, name=name)

        def T16(name, cols, parts=P):
            return pool.tile([parts, cols], FP16, name=name)

        # ---------------- per-core constants ----------------
        anch_sb = T("anch_sb", COLS * 4)
        nc.sync.dma_start(anch_sb[:, 0:COLS * 2], anch_d[:, 0:COLS * 2])
        nc.sync.dma_start(anch_sb[:, COLS * 2:], anch_d[:, COLS * 2:])
        anch3 = anch_sb[:].rearrange("p (n c) -> p n c", c=4)

        ax2h = T16("ax2h", COLS)
        ay2h = T16("ay2h", COLS)
        nax1h = T16("nax1h", COLS)
        nay1h = T16("nay1h", COLS)
        areaAh = T16("areaAh", COLS)
        f0 = T("f0", COLS)
        f1 = T("f1", COLS)
        sc.activation(ax2h[:], anch3[:, :, 2:3].squeeze(2), Act.Copy, scale=SC)
        sc.activation(ay2h[:], anch3[:, :, 3:4].squeeze(2), Act.Copy, scale=SC)
        sc.activation(nax1h[:], anch3[:, :, 0:1].squeeze(2), Act.Copy, scale=-SC)
        sc.activation(nay1h[:], anch3[:, :, 1:2].squeeze(2), Act.Copy, scale=-SC)
        v.tensor_tensor(out=f0[:], in0=anch3[:, :, 2:3].squeeze(2),
                        in1=anch3[:, :, 0:1].squeeze(2), op=Alu.subtract)
        v.tensor_tensor(out=f1[:], in0=anch3[:, :, 3:4].squeeze(2),
                        in1=anch3[:, :, 1:2].squeeze(2), op=Alu.subtract)
        v.tensor_tensor(out=f0[:], in0=f0[:], in1=f1[:], op=Alu.mult)
        sc.activation(areaAh[:], f0[:], Act.Copy, scale=SC * SC)

        ident = T16("ident", P)
        make_identity(nc, ident[:])
        identf = T("identf", P)
        make_identity(nc, identf[:])
        ones_col = T("ones_col", 1)
        ones_row = T("ones_row", P, parts=1)
        ones_row16 = T16("ones_row16", P, parts=1)
        v.memset(ones_col[:], 1.0)
        v.memset(ones_row[:], 1.0)
        v.memset(ones_row16[:], 1.0)
        iota_i = pool.tile([P, NBIN], mybir.dt.int32, name="iota_i")
        nc.gpsimd.iota(iota_i[:], pattern=[[1, NBIN]], base=0, channel_multiplier=0)
        iota_f = T("iota_f", NBIN)
        v.tensor_copy(iota_f[:], iota_i[:])

        def pbcast(dst, src_row):
            n = src_row.shape[-1]
            bc_ps = pspool.tile([P, G], F32, name="bc_ps", tag="pss")
            nc.tensor.matmul(bc_ps[:, 0:n], ones_row[:], src_row)
            v.tensor_copy(dst, bc_ps[:, 0:n])

        # ---------------- shared big tiles ----------------
        grid = T16("grid", NSUP * W)     # r values, (s, g, u) blocks
        tgrid = T16("tgrid", NSUP * W)   # one-hot, (s, u, g) blocks
        sA_l = [T16(f"sA{k}", QW) for k in range(2)]
        sB_l = [T16(f"sB{k}", QW) for k in range(2)]
        sCt_l = [T16("sCt0", QW)] * 2
        rm = T16("rm", COLS)
        colT = T16("colT", NSUP * G)
        cpa = T16("cpa", G)
        forced = T16("forced", COLS)
        pos16 = T16("pos16", COLS)
        posf = T("posf", COLS)
        cmb = T16("cmb", G)
        cmaxpl = T16("cmaxpl", W)

        gtrow_l = [T(f"gtrow{b}", G * 4, parts=1) for b in range(IMG)]
        gtall_l = [T(f"gtall{b}", G * 4) for b in range(IMG)]
        gsc = [T(f"gsc{c}", G) for c in range(4)]
        sGf = T("sGf", G)
        tG = T("tG", G)
        ngx1p = T16("ngx1p", W)
        ngy1p = T16("ngy1p", W)
        gx2p = T16("gx2p", W)
        gy2p = T16("gy2p", W)
        sGp = T16("sGp", W)
        gtmatf_l = [T(f"gtmatf{b}", 16) for b in range(IMG)]
        gtmath_l = [T16(f"gtmath{b}", 16) for b in range(IMG)]

        tsb_l = [T16(f"tsb{k}", W) for k in range(2)]
        matched = T16("matched", 4 * COLS)   # (s, u, c)
        mch = [T16(f"mch{c}", COLS) for c in range(4)]
        bxh = [T16(f"bxh{c}", COLS) for c in range(4)]
        areaPh = T16("areaPh", COLS)
        bbox_sb_l = [T(f"bbox_sb{b}", COLS * 4) for b in range(IMG)]
        conf_sb_l = [T(f"conf_sb{b}", COLS) for b in range(IMG)]

        d0 = T16("d0", COLS)
        d1 = T16("d1", COLS)
        d2 = T16("d2", COLS)
        d3 = T16("d3", COLS)
        d4 = T16("d4", COLS)

        s0 = T("s0", COLS)
        s1 = T("s1", COLS)
        s2 = T("s2", COLS)
        s3 = T("s3", COLS)
        s4 = T("s4", COLS)
        s5 = T("s5", COLS)
        cl = T("cl", COLS)
        nv = T("nv", COLS)
        nv16 = T16("nv16", COLS)
        sink16 = T16("sink16", COLS)

        npp = T("npp", 1)
        locsum_pp = T("locsum_pp", 1)
        possum_pp = T("possum_pp", 1)
        cnt_pp = T("cnt_pp", 1)
        sum_pp = T("sum_pp", 1)
        maxv_pp = T("maxv_pp", 1)
        maxvb = T("maxvb", 1)
        w1c = T("w1c", 1)
        tau_b = T("tau_b", 1)
        stack = T("stack", 4)
        thr = T("thr", NBIN)
        nthr = T("nthr", NBIN)
        cge = T("cge", NBIN)
        wl = [T(f"wl{l}", 1) for l in range(NLEV)]
        lo_b = [T(f"lo_b{l}", 1) for l in range(NLEV)]
        cget = T("cget", NBIN, parts=1)
        gek = T("gek", NBIN, parts=1)
        scnt = T("scnt", 1, parts=1)
        lo_new = T("lo_new", 1, parts=1)
        tau = [T(f"tau{l}", 1, parts=1) for l in range(NLEV)]
        maxv1 = T("maxv1", 1, parts=1)
        npos1 = T("npos1", 1, parts=1)
        k1 = T("k1", 1, parts=1)
        k2 = T("k2", 1, parts=1)
        kk = T("kk", 1, parts=1)
        mx_row = T("mx_row", P, parts=1)
        ctt = T16("ctt", P, parts=G)
        cmax_col = T16("cmax_col", 1, parts=G)
        cm_row = T16("cm_row", G, parts=1)
        res_sb = T("res_sb", 8, parts=1)

        # 4D view helpers
        def q4gu(t, q):  # (s, g, u) packed quarter
            return t[:, q*QW:(q+1)*QW].rearrange("p (s g u) -> p s g u", g=G, u=U)

        def aview(t, q):  # anchor [P, COLS] -> [p, s, g(b), u]
            return (t[:, q*QSUP*U:(q+1)*QSUP*U]
                    .rearrange("p (s u) -> p s u", u=U)
                    .unsqueeze(2).to_broadcast([P, QSUP, G, U]))

        def gview(t):     # gt plane [P, W] -> [p, s(b), g, u]
            return (t[:].rearrange("p (g u) -> p g u", u=U)
                    .unsqueeze(1).to_broadcast([P, QSUP, G, U]))

        for b in range(IMG):
            bbox_sb = bbox_sb_l[b]
            conf_sb = conf_sb_l[b]
            gtrow = gtrow_l[b]
            gtall = gtall_l[b]
            gtmatf = gtmatf_l[b]
            gtmath = gtmath_l[b]
            # ---------------- loads ----------------
            nc.sync.dma_start(bbox_sb[:, 0:COLS * 2], bbox_d[b][:, 0:COLS * 2])
            nc.sync.dma_start(bbox_sb[:, COLS * 2:], bbox_d[b][:, COLS * 2:])
            nc.sync.dma_start(conf_sb[:], conf_d[b])
            nc.scalar.dma_start(gtrow[:], gt_d[b])
            gt_ps = pspool.tile([P, G * 4], F32, name="gt_ps", tag="gtp")
            nc.tensor.matmul(gt_ps[:], ones_row[:], gtrow[:])
            v.tensor_copy(gtall[:], gt_ps[:])
            gt3 = gtall[:].rearrange("p (g c) -> p g c", c=4)
            gt2d = gt_d[b].rearrange("q (g c) -> (q g) c", c=4)
            v.memset(gtmatf[:], 0.0)
            for uu in range(4):
                nc.scalar.dma_start(gtmatf[uu*G:(uu+1)*G, uu*4:(uu+1)*4], gt2d)
            sc.activation(gtmath[:], gtmatf[:], Act.Copy, scale=SC)

            # ---------------- gt prep ----------------
            v.tensor_scalar(gsc[0][:], gt3[:, :, 0:1].squeeze(2), -SC, None, Alu.mult)
            v.tensor_scalar(gsc[1][:], gt3[:, :, 1:2].squeeze(2), -SC, None, Alu.mult)
            v.tensor_scalar(gsc[2][:], gt3[:, :, 2:3].squeeze(2), SC, None, Alu.mult)
            v.tensor_scalar(gsc[3][:], gt3[:, :, 3:4].squeeze(2), SC, None, Alu.mult)
            v.tensor_tensor(out=sGf[:], in0=gt3[:, :, 2:3].squeeze(2),
                            in1=gt3[:, :, 0:1].squeeze(2), op=Alu.subtract)
            v.tensor_tensor(out=tG[:], in0=gt3[:, :, 3:4].squeeze(2),
                            in1=gt3[:, :, 1:2].squeeze(2), op=Alu.subtract)
            v.tensor_tensor(out=sGf[:], in0=sGf[:], in1=tG[:], op=Alu.mult)
            v.tensor_scalar(sGf[:], sGf[:], SC * SC, None, Alu.mult)
            for pl, src in ((ngx1p, gsc[0]), (ngy1p, gsc[1]), (gx2p, gsc[2]),
                            (gy2p, gsc[3]), (sGp, sGf)):
                sc.activation(pl[:].rearrange("p (g u) -> p g u", u=U),
                              src[:].unsqueeze(2).to_broadcast([P, G, U]), Act.Copy)

            # ---------------- pass 1: grid ----------------
            for q in range(NQ):
                sA = sA_l[q % 2]
                sB = sB_l[q % 2]
                sCt = sCt_l[q % 2]
                gq = q4gu(grid, q)
                a4 = q4gu(sA, 0)
                b4 = q4gu(sB, 0)
                c4 = q4gu(sCt, 0)
                v.tensor_tensor(out=a4, in0=aview(nax1h, q), in1=gview(ngx1p), op=Alu.min)
                v.tensor_tensor(out=b4, in0=aview(ax2h, q), in1=gview(gx2p), op=Alu.min)
                v.tensor_tensor(out=a4, in0=b4, in1=a4, op=Alu.add)
                sc.activation(sA[:], sA[:], Act.Relu)
                v.tensor_tensor(out=b4, in0=aview(nay1h, q), in1=gview(ngy1p), op=Alu.min)
                v.tensor_tensor(out=c4, in0=aview(ay2h, q), in1=gview(gy2p), op=Alu.min)
                v.tensor_tensor(out=b4, in0=c4, in1=b4, op=Alu.add)
                v.tensor_scalar(sB[:], sB[:], 0.0, None, Alu.max)
                v.tensor_tensor(out=a4, in0=a4, in1=b4, op=Alu.mult)      # inter
                HW2 = QW // 2
                for hh in range(2):
                    s0q = q * QSUP + hh * (QSUP // 2)
                    hsl = slice(hh * HW2, (hh + 1) * HW2)
                    bh = (sB_l[q % 2][:, hsl]
                          .rearrange("p (s g u) -> p s g u", g=G, u=U))
                    ah = (sA_l[q % 2][:, hsl]
                          .rearrange("p (s g u) -> p s g u", g=G, u=U))
                    gh = (grid[:, q*QW + hh*HW2: q*QW + (hh+1)*HW2]
                          .rearrange("p (s g u) -> p s g u", g=G, u=U))
                    avh = (areaAh[:, s0q*U:(s0q + QSUP//2)*U]
                           .rearrange("p (s u) -> p s u", u=U)
                           .unsqueeze(2).to_broadcast([P, QSUP//2, G, U]))
                    gvh = (sGp[:].rearrange("p (g u) -> p g u", u=U)
                           .unsqueeze(1).to_broadcast([P, QSUP//2, G, U]))
                    v.tensor_tensor(out=bh, in0=avh, in1=gvh, op=Alu.add)
                    _act_recip(nc, sB_l[q % 2][:, hsl], sB_l[q % 2][:, hsl])
                    v.tensor_tensor(out=gh, in0=ah, in1=bh, op=Alu.mult)

                cur = gq
                width = G
                while width > 1:
                    h = width // 2
                    if h == 1:
                        dst = (rm[:, q*QSUP*U:(q+1)*QSUP*U]
                               .rearrange("p (s u) -> p s u", u=U).unsqueeze(2))
                    else:
                        dst = q4gu(sCt, 0)[:, :, 0:h, :]
                    v.tensor_tensor(out=dst, in0=cur[:, :, 0:h, :],
                                    in1=cur[:, :, h:width, :], op=Alu.max)
                    cur = dst
                    width = h
                curc = gq
                widc = U
                while widc > 1:
                    hc = widc // 2
                    if hc == 1:
                        dstc = (colT[:, q*QSUP*G:(q+1)*QSUP*G]
                                .rearrange("p (s g) -> p s g", g=G).unsqueeze(3))
                    else:
                        dstc = q4gu(sB, 0)[:, :, :, 0:hc]
                    v.tensor_tensor(out=dstc, in0=curc[:, :, :, 0:hc],
                                    in1=curc[:, :, :, hc:widc], op=Alu.max)
                    curc = dstc
                    widc = hc

            # ---------------- cmax finalize ----------------
            v.tensor_reduce(out=cpa[:], in_=colT[:].rearrange("p (s g) -> p g s", g=G),
                            axis=Ax.X, op=Alu.max)
            ct_ps = pspool.tile([G, P], FP16, name="ct_ps", tag="pst")
            pe.transpose(ct_ps[:], cpa[:], ident[:])
            v.tensor_copy(ctt[:], ct_ps[:])
            v.tensor_reduce(out=cmax_col[:], in_=ctt[:], axis=Ax.X, op=Alu.max)
            cm_ps = pspool.tile([1, G], FP16, name="cm_ps", tag="pst")
            pe.transpose(cm_ps[:], cmax_col[:], ident[:G, :G])
            v.tensor_copy(cm_row[:], cm_ps[:])
            bc_ps = pspool.tile([P, G], F32, name="bc_ps", tag="pss")
            nc.tensor.matmul(bc_ps[:], ones_row16[:], cm_row[:])
            v.tensor_copy(cmb[:], bc_ps[:])
            sc.activation(cmaxpl[:].rearrange("p (g u) -> p g u", u=U),
                          cmb[:].unsqueeze(2).to_broadcast([P, G, U]), Act.Copy)

            # ---------------- forced + pos ----------------
            for q in range(NQ):
                sA = sA_l[q % 2]
                gq = q4gu(grid, q)
                v.tensor_tensor(out=q4gu(sA, 0), in0=gq, in1=gview(cmaxpl),
                                op=Alu.is_equal)
                cur = q4gu(sA, 0)
                width = G
                while width > 1:
                    h = width // 2
                    if h == 1:
                        dst = (forced[:, q*QSUP*U:(q+1)*QSUP*U]
                               .rearrange("p (s u) -> p s u", u=U).unsqueeze(2))
                    else:
                        dst = cur[:, :, 0:h, :]
                    v.tensor_tensor(out=dst, in0=cur[:, :, 0:h, :],
                                    in1=cur[:, :, h:width, :], op=Alu.max)
                    cur = dst
                    width = h
            v.tensor_scalar(pos16[:], rm[:], POS_THR, None, Alu.is_gt)
            v.tensor_tensor(out=pos16[:], in0=pos16[:], in1=forced[:], op=Alu.max)
            v.tensor_scalar(sink16[:], pos16[:], 1.0, 0.0, Alu.mult, Alu.add,
                            accum_out=npp[:])
            v.tensor_copy(posf[:], pos16[:])

            # ---------------- ismax -> tgrid (s, u, g) ----------------
            for q in range(NQ):
                gin = grid[:, q*QW:(q+1)*QW].rearrange(
                    "p (s g u) -> p s u g", g=G, u=U)
                rmv = (rm[:, q*QSUP*U:(q+1)*QSUP*U]
                       .rearrange("p (s u) -> p s u", u=U)
                       .unsqueeze(3).to_broadcast([P, QSUP, U, G]))
                tout = tgrid[:, q*QW:(q+1)*QW].rearrange(
                    "p (s u g) -> p s u g", u=U, g=G)
                v.tensor_tensor(out=tout, in0=gin, in1=rmv, op=Alu.is_equal)

            # ---------------- matched coords (PE) ----------------
            for s in range(NSUP):
                tsb = tsb_l[s % 2]
                mout = pspool.tile([P, P], F32, name=f"mo{s % 2}", tag=f"mo{s % 2}")
                for j in range(8):
                    tp = pspool.tile([P, P], FP16, name=f"tp{j % 2}", tag=f"tp{j % 2}")
                    pe.transpose(tp[:], tgrid[:, s*W + j*P: s*W + (j+1)*P], ident[:])
                    sc.copy(tsb[:, j*P:(j+1)*P], tp[:])
                    nc.tensor.matmul(mout[:, j*16:(j+1)*16], tsb[:, j*P:(j+1)*P],
                                     gtmath[:])
                sc.copy(matched[:, s*P:(s+1)*P], mout[:])

            m4 = matched[:].rearrange("p (n c) -> p n c", c=4)
            for c in range(4):
                sc.copy(mch[c][:], m4[:, :, c:c+1].squeeze(2))

            # ---------------- bbox planes + diou ----------------
            bb3 = bbox_sb[:].rearrange("p (n c) -> p n c", c=4)
            for c in range(4):
                sc.activation(bxh[c][:], bb3[:, :, c:c+1].squeeze(2), Act.Copy,
                              scale=SC)
            v.tensor_tensor(out=d0[:], in0=bxh[2][:], in1=bxh[0][:], op=Alu.subtract)
            v.tensor_tensor(out=d1[:], in0=bxh[3][:], in1=bxh[1][:], op=Alu.subtract)
            v.tensor_tensor(out=areaPh[:], in0=d0[:], in1=d1[:], op=Alu.mult)

            # inter
            v.tensor_tensor(out=d0[:], in0=bxh[0][:], in1=mch[0][:], op=Alu.max)
            v.tensor_tensor(out=d1[:], in0=bxh[2][:], in1=mch[2][:], op=Alu.min)
            v.tensor_tensor(out=d0[:], in0=d1[:], in1=d0[:], op=Alu.subtract)
            v.tensor_scalar(d0[:], d0[:], 0.0, None, Alu.max)
            v.tensor_tensor(out=d1[:], in0=bxh[1][:], in1=mch[1][:], op=Alu.max)
            v.tensor_tensor(out=d2[:], in0=bxh[3][:], in1=mch[3][:], op=Alu.min)
            v.tensor_tensor(out=d1[:], in0=d2[:], in1=d1[:], op=Alu.subtract)
            v.tensor_scalar(d1[:], d1[:], 0.0, None, Alu.max)
            v.tensor_tensor(out=d0[:], in0=d0[:], in1=d1[:], op=Alu.mult)  # inter
            # matched area
            v.tensor_tensor(out=d1[:], in0=mch[2][:], in1=mch[0][:], op=Alu.subtract)
            v.tensor_tensor(out=d2[:], in0=mch[3][:], in1=mch[1][:], op=Alu.subtract)
            v.tensor_tensor(out=d1[:], in0=d1[:], in1=d2[:], op=Alu.mult)
            # union, iou
            v.tensor_tensor(out=d1[:], in0=d1[:], in1=areaPh[:], op=Alu.add)
            v.tensor_tensor(out=d1[:], in0=d1[:], in1=d0[:], op=Alu.subtract)
            _act_recip(nc, d1[:], d1[:])
            v.tensor_tensor(out=d0[:], in0=d0[:], in1=d1[:], op=Alu.mult)  # iou
            # enclosing c2
            v.tensor_tensor(out=d1[:], in0=bxh[0][:], in1=mch[0][:], op=Alu.min)
            v.tensor_tensor(out=d2[:], in0=bxh[2][:], in1=mch[2][:], op=Alu.max)
            v.tensor_tensor(out=d1[:], in0=d2[:], in1=d1[:], op=Alu.subtract)
            sc.activation(d1[:], d1[:], Act.Square)
            v.tensor_tensor(out=d2[:], in0=bxh[1][:], in1=mch[1][:], op=Alu.min)
            v.tensor_tensor(out=d3[:], in0=bxh[3][:], in1=mch[3][:], op=Alu.max)
            v.tensor_tensor(out=d2[:], in0=d3[:], in1=d2[:], op=Alu.subtract)
            sc.activation(d2[:], d2[:], Act.Square)
            v.tensor_tensor(out=d1[:], in0=d1[:], in1=d2[:], op=Alu.add)   # c2
            _act_recip(nc, d1[:], d1[:])
            # center dist d2 (quarter-scaled: absorbed by c2 ratio using same /2)
            v.tensor_tensor(out=d2[:], in0=bxh[0][:], in1=bxh[2][:], op=Alu.add)
            v.tensor_tensor(out=d3[:], in0=mch[0][:], in1=mch[2][:], op=Alu.add)
            v.tensor_tensor(out=d2[:], in0=d2[:], in1=d3[:], op=Alu.subtract)
            sc.activation(d2[:], d2[:], Act.Square)
            v.tensor_tensor(out=d3[:], in0=bxh[1][:], in1=bxh[3][:], op=Alu.add)
            v.tensor_tensor(out=d4[:], in0=mch[1][:], in1=mch[3][:], op=Alu.add)
            v.tensor_tensor(out=d3[:], in0=d3[:], in1=d4[:], op=Alu.subtract)
            sc.activation(d3[:], d3[:], Act.Square)
            v.tensor_tensor(out=d2[:], in0=d2[:], in1=d3[:], op=Alu.add)   # 4*d2
            v.tensor_tensor(out=d2[:], in0=d2[:], in1=d1[:], op=Alu.mult)
            v.tensor_scalar(d2[:], d2[:], 0.25, None, Alu.mult)            # d2/c2
            v.tensor_scalar(d0[:], d0[:], -1.0, 1.0, Alu.mult, Alu.add)    # 1-iou
            v.tensor_tensor(out=d2[:], in0=d2[:], in1=d0[:], op=Alu.add)
            v.tensor_scalar(d2[:], d2[:], 100.0, None, Alu.min)
            v.tensor_tensor(out=d2[:], in0=d2[:], in1=pos16[:], op=Alu.mult)
            v.tensor_scalar(sink16[:], d2[:], 1.0, 0.0, Alu.mult, Alu.add,
                            accum_out=locsum_pp[:])

            # ---------------- focal conf loss (f32, baseline) ----------------
            sc.activation(s0[:], conf_sb[:], Act.Sigmoid)
            sc.activation(s1[:], conf_sb[:], Act.Exp)
            sc.activation(s1[:], s1[:], Act.Ln, bias=1.0)
            v.tensor_tensor(out=s2[:], in0=conf_sb[:], in1=posf[:], op=Alu.mult)
            v.tensor_tensor(out=s2[:], in0=s1[:], in1=s2[:], op=Alu.subtract)
            v.tensor_scalar(s3[:], posf[:], -2.0, 1.0, Alu.mult, Alu.add)
            v.tensor_tensor(out=s3[:], in0=s0[:], in1=s3[:], op=Alu.mult)
            v.tensor_tensor(out=s3[:], in0=s3[:], in1=posf[:], op=Alu.add)
            sc.activation(s3[:], s3[:], Act.Square)
            v.tensor_tensor(out=cl[:], in0=s3[:], in1=s2[:], op=Alu.mult)
            v.tensor_scalar(s3[:], posf[:], -0.5, 0.75, Alu.mult, Alu.add)
            v.tensor_tensor(out=cl[:], in0=cl[:], in1=s3[:], op=Alu.mult)
            v.tensor_scalar(cl[:], cl[:], 100.0, None, Alu.min)
            v.tensor_tensor(out=s4[:], in0=cl[:], in1=posf[:], op=Alu.mult)
            v.tensor_scalar(s5[:], s4[:], 1.0, 0.0, Alu.mult, Alu.add,
                            accum_out=possum_pp[:])
            v.tensor_tensor(out=nv[:], in0=cl[:], in1=s4[:], op=Alu.subtract)
            v.tensor_copy(nv16[:], nv[:])

            # ---------------- hard negative mining (baseline) ----------------
            v.tensor_reduce(out=maxv_pp[:], in_=nv[:], axis=Ax.X, op=Alu.max)
            mx_ps = pspool.tile([1, P], F32, name="mx_ps", tag="pss")
            pe.transpose(mx_ps[:], maxv_pp[:], identf[:])
            v.tensor_copy(mx_row[:], mx_ps[:])
            v.tensor_reduce(out=maxv1[:], in_=mx_row[:], axis=Ax.X, op=Alu.max)

            np_ps = pspool.tile([1, 1], F32, name="np_ps", tag="pss")
            nc.tensor.matmul(np_ps[:], ones_col[:], npp[:])
            v.tensor_copy(npos1[:], np_ps[:])
            v.tensor_scalar(k1[:], npos1[:], NEG_POS_RATIO, None, Alu.mult)
            v.tensor_scalar(k2[:], npos1[:], -1.0, float(A), Alu.mult, Alu.add)
            v.tensor_tensor(out=kk[:], in0=k1[:], in1=k2[:], op=Alu.min)

            pbcast(maxvb[:], maxv1[:])
            v.tensor_scalar(w1c[:], maxvb[:], 1.0 / NBIN, None, Alu.mult)

            for lev in range(NLEV):
                if lev == 0:
                    v.tensor_copy(wl[0][:], w1c[:])
                    v.tensor_scalar(thr[:], iota_f[:], wl[0][:], None, Alu.mult)
                else:
                    v.tensor_scalar(wl[lev][:], wl[lev - 1][:], 1.0 / NBIN, None,
                                    Alu.mult)
                    v.tensor_scalar(thr[:], iota_f[:], wl[lev][:], lo_b[lev - 1][:],
                                    Alu.mult, Alu.add)
                v.tensor_scalar(nthr[:], thr[:], -1.0, None, Alu.mult)
                # img0 mining overlaps img1 grid (DVE-bound): all bins on Act.
                # img1 mining is the tail: split bins across Act and DVE.
                nact = NBIN if b == 0 else NBIN // 2
                for bn in range(nact):
                    sc.activation(sink16[:], nv16[:], Act.Sign,
                                  bias=nthr[:, bn:bn+1], accum_out=cge[:, bn:bn+1])
                for bn in range(nact, NBIN):
                    v.tensor_scalar(d4[:], nv16[:], thr[:, bn:bn+1], 0.0,
                                    Alu.is_gt, Alu.add, accum_out=cge[:, bn:bn+1])
                cg_ps = pspool.tile([1, NBIN], F32, name="cg_ps", tag="pss")
                nc.tensor.matmul(cg_ps[:], ones_col[:], cge[:])
                v.tensor_copy(cget[:], cg_ps[:])
                # Act bins hold sign-sums: cnt = (acc + A)/2; DVE bins exact
                v.tensor_scalar(cget[:, 0:nact], cget[:, 0:nact], 0.5,
                                float(A) * 0.5, Alu.mult, Alu.add)
                v.tensor_scalar(gek[:], cget[:], kk[:], None, Alu.is_ge)
                v.tensor_reduce(out=scnt[:], in_=gek[:], axis=Ax.X, op=Alu.add)
                v.tensor_scalar(lo_new[:], scnt[:], 1.0, wl[lev][0:1, :],
                                Alu.subtract, Alu.mult)
                v.tensor_scalar(tau[lev][:], scnt[:], wl[lev][0:1, :], None, Alu.mult)
                if lev > 0:
                    v.tensor_tensor(out=lo_new[:], in0=lo_new[:],
                                    in1=lo_b[lev - 1][0:1, :], op=Alu.add)
                    v.tensor_tensor(out=tau[lev][:], in0=tau[lev][:],
                                    in1=lo_b[lev - 1][0:1, :], op=Alu.add)
                pbcast(lo_b[lev][:], lo_new[:])

            pbcast(tau_b[:], tau[NLEV - 1][:])
            v.tensor_scalar(s4[:], nv[:], tau_b[:], 0.0, Alu.is_gt,
                            Alu.add, accum_out=cnt_pp[:])
            v.tensor_tensor(out=s5[:], in0=nv[:], in1=s4[:], op=Alu.mult)
            v.tensor_scalar(s5[:], s5[:], 1.0, 0.0, Alu.mult, Alu.add,
                            accum_out=sum_pp[:])

            # ---------------- gather scalars ----------------
            v.tensor_copy(stack[:, 0:1], npp[:])
            v.tensor_copy(stack[:, 1:2], locsum_pp[:])
            v.tensor_copy(stack[:, 2:3], possum_pp[:])
            v.tensor_copy(stack[:, 3:4], cnt_pp[:])
            st_ps = pspool.tile([1, 4], F32, name="st_ps", tag="pss")
            nc.tensor.matmul(st_ps[:], ones_col[:], stack[:])
            sm_ps = pspool.tile([1, 1], F32, name="sm_ps", tag="pss")
            nc.tensor.matmul(sm_ps[:], ones_col[:], sum_pp[:])

            v.tensor_copy(res_sb[:, 0:4], st_ps[:])
            v.tensor_copy(res_sb[:, 4:5], sm_ps[:])
            v.tensor_copy(res_sb[:, 5:6], tau[NLEV - 1][:])
            v.tensor_copy(res_sb[:, 6:7], maxv1[:])
            v.tensor_copy(res_sb[:, 7:8], kk[:])
            nc.sync.dma_start(res_d[b], res_sb[:])

    nc.compile()
    return nc


_NC_CACHE = None


def _get_nc():
    global _NC_CACHE
    if _NC_CACHE is None:
        _NC_CACHE = _build_nc()
    return _NC_CACHE


def _make_in_maps(inputs):
    bbox_pred = np.asarray(inputs["bbox_pred"])
    conf_pred = np.asarray(inputs["conf_pred"])
    anchors = np.asarray(inputs["anchors"])
    gt_boxes = np.asarray(inputs["gt_boxes"])
    anch_h = np.ascontiguousarray(anchors.reshape(P, COLS * 4), dtype=np.float32)
    in_maps = []
    for i in range(NCORE):
        bsl = slice(IMG * i, IMG * (i + 1))
        in_maps.append({
            "anch": anch_h,
            "bbox": np.ascontiguousarray(
                bbox_pred[bsl].reshape(IMG, P, COLS * 4), dtype=np.float32),
            "conf": np.ascontiguousarray(
                conf_pred[bsl].reshape(IMG, P, COLS), dtype=np.float32),
            "gtb": np.ascontiguousarray(
                gt_boxes[bsl].reshape(IMG, 1, G * 4), dtype=np.float32),
        })
    return in_maps


def kernel(bbox_pred, conf_pred, anchors, gt_boxes):
    nc = _get_nc()
    in_maps = _make_in_maps(dict(bbox_pred=bbox_pred, conf_pred=conf_pred,
                                 anchors=anchors, gt_boxes=gt_boxes))
    out = run_bass_kernel_spmd(nc, in_maps, core_ids=list(range(NCORE)))

    loc_total = np.float32(0.0)
    conf_total = np.float32(0.0)
    npos_total = np.float32(0.0)
    for i in range(NCORE):
        res = out.results[i]["res"]  # [IMG, 1, 8]
        for b in range(IMG):
            npos, locsum, possum, cnt_gt, sum_gt, tau_hi, maxv, kdev = \
                [np.float32(x) for x in res[b, 0, :8]]
            k = np.float32(min(NEG_POS_RATIO * npos, A - npos))
            wl_last = np.float32(maxv / NBIN ** NLEV)
            rem = max(np.float32(0.0), np.float32(k - cnt_gt))
            neg = np.float32(sum_gt + rem * (tau_hi - wl_last * np.float32(0.5)))
            loc_total = np.float32(loc_total + locsum)
            conf_total = np.float32(conf_total + possum + neg)
            npos_total = np.float32(npos_total + npos)
    num_pos = np.float32(max(1.0, npos_total))
    loc_loss = np.float32(loc_total / num_pos)
    conf_loss = np.float32(conf_total / num_pos)
    return (np.float32(loc_loss + conf_loss), conf_loss, loc_loss)


# revision 16
# speedup vs baseline: 1.6198x; 1.0054x over previous
"""Trainium2 Bass kernel v2 for nn_DetectionLoss — fp16 grid pipeline.

Data-parallel: 16 images over 8 cores (2 images/core). Per image, the
[A=65536, G=32] match grid is computed in fp16 (coords pre-scaled by 1/64 —
the loss is scale-invariant) in supertile-(g,u) layout so every DVE op is
packed-innermost and runs in 2x/4x mode. Division uses the Act engine's
table Reciprocal (~0.5% rel err — only feeds match *selection*, fine at the
2e-2 gate). Matched-gt coords go through fp16 PE transposes of the one-hot
grid + 16-wide block-diag matmuls. DIoU runs in fp16 on coord planes; focal
+ hard-negative mining keep the baseline f32/fp16 structure. Host combines
per-image scalars exactly like the reference."""
import sys

sys.path.insert(0, '/opt/trn_rl_repo')

import numpy as np
import concourse.bass as bass
import concourse.bacc as bacc
import concourse.mybir as mybir
from concourse.tile import TileContext
from concourse.bass_utils import run_bass_kernel_spmd
from concourse.masks import make_identity
from contextlib import ExitStack

Alu = mybir.AluOpType
Act = mybir.ActivationFunctionType
Ax = mybir.AxisListType
F32 = mybir.dt.float32
FP16 = mybir.dt.float16

P = 128
A = 65536
G = 32
IMG = 2
NCORE = 8
COLS = A // P       # 512
U = 32
W = G * U           # 1024 els per supertile block
NSUP = COLS // U    # 16
NQ = 4
QSUP = NSUP // NQ   # 4
QW = QSUP * W       # 4096
SC = 1.0 / 64.0
POS_THR = 1.0 / 3.0
NBIN = 16
NLEV = 2
NEG_POS_RATIO = 3.0


def _act_recip(nc, out, in_):
    """Raw Act-engine Reciprocal (table approx, ~0.5% rel err)."""
    sc = nc.scalar
    ins = [sc.lower_ap(in_)]
    for argv in (0.0, 1.0, 0.0):
        ins.append(mybir.ImmediateValue(dtype=mybir.dt.float32, value=argv))
    return sc.add_instruction(
        mybir.InstActivation(name=nc.get_next_instruction_name(),
                             func=Act.Reciprocal, ins=ins,
                             outs=[sc.lower_ap(out)]))


def _build_nc():
    nc = bacc.Bacc("TRN2", target_bir_lowering=False, debug=False)
    anch_d = nc.dram_tensor("anch", [P, COLS * 4], F32, kind="ExternalInput")
    bbox_d = nc.dram_tensor("bbox", [IMG, P, COLS * 4], F32, kind="ExternalInput")
    conf_d = nc.dram_tensor("conf", [IMG, P, COLS], F32, kind="ExternalInput")
    gt_d = nc.dram_tensor("gtb", [IMG, 1, G * 4], F32, kind="ExternalInput")
    res_d = nc.dram_tensor("res", [IMG, 1, 8], F32, kind="ExternalOutput")

    v = nc.vector
    sc = nc.scalar
    pe = nc.tensor

    with TileContext(nc) as tc, ExitStack() as ctx, \
            nc.allow_low_precision(reason="fp16 grid; host checks rel err"):
        pool = ctx.enter_context(tc.tile_pool(name="main", bufs=1))
        pspool = ctx.enter_context(tc.tile_pool(name="ps", bufs=1, space="PSUM"))

        def T(name, cols, parts=P, dt=F32):
            return pool.tile([parts, cols], dt, name=name)

        def T16(name, cols, parts=P):
            return pool.tile([parts, cols], FP16, name=name)

        # ---------------- per-core constants ----------------
        anch_sb = T("anch_sb", COLS * 4)
        nc.sync.dma_start(anch_sb[:, 0:COLS * 2], anch_d[:, 0:COLS * 2])
        nc.sync.dma_start(anch_sb[:, COLS * 2:], anch_d[:, COLS * 2:])
        anch3 = anch_sb[:].rearrange("p (n c) -> p n c", c=4)

        ax2h = T16("ax2h", COLS)
        ay2h = T16("ay2h", COLS)
        nax1h = T16("nax1h", COLS)
        nay1h = T16("nay1h", COLS)
        areaAh = T16("areaAh", COLS)
        f0 = T("f0", COLS)
        f1 = T("f1", COLS)
        sc.activation(ax2h[:], anch3[:, :, 2:3].squeeze(2), Act.Copy, scale=SC)
        sc.activation(ay2h[:], anch3[:, :, 3:4].squeeze(2), Act.Copy, scale=SC)
        sc.activation(nax1h[:], anch3[:, :, 0:1].squeeze(2), Act.Copy, scale=-SC)
        sc.activation(nay1h[:], anch3[:, :, 1:2].squeeze(2), Act.Copy, scale=-SC)
        v.tensor_tensor(out=f0[:], in0=anch3[:, :, 2:3].squeeze(2),
                        in1=anch3[:, :, 0:1].squeeze(2), op=Alu.subtract)
        v.tensor_tensor(out=f1[:], in0=anch3[:, :, 3:4].squeeze(2),
                        in1=anch3[:, :, 1:2].squeeze(2), op=Alu.subtract)
        v.tensor_tensor(out=f0[:], in0=f0[:], in1=f1[:], op=Alu.mult)
        sc.activation(areaAh[:], f0[:], Act.Copy, scale=SC * SC)

        ident = T16("ident", P)
        make_identity(nc, ident[:])
        identf = T("identf", P)
        make_identity(nc, identf[:])
        ones_col = T("ones_col", 1)
        ones_row = T("ones_row", P, parts=1)
        ones_row16 = T16("ones_row16", P, parts=1)
        v.memset(ones_col[:], 1.0)
        v.memset(ones_row[:], 1.0)
        v.memset(ones_row16[:], 1.0)
        iota_i = pool.tile([P, NBIN], mybir.dt.int32, name="iota_i")
        nc.gpsimd.iota(iota_i[:], pattern=[[1, NBIN]], base=0, channel_multiplier=0)
        iota_f = T("iota_f", NBIN)
        v.tensor_copy(iota_f[:], iota_i[:])

        def pbcast(dst, src_row):
            n = src_row.shape[-1]
            bc_ps = pspool.tile([P, G], F32, name="bc_ps", tag="pss")
            nc.tensor.matmul(bc_ps[:, 0:n], ones_row[:], src_row)
            v.tensor_copy(dst, bc_ps[:, 0:n])

        # ---------------- shared big tiles ----------------
        grid = T16("grid", NSUP * W)     # r values, (s, g, u) blocks
        tgrid = T16("tgrid", NSUP * W)   # one-hot, (s, u, g) blocks
        sA_l = [T16(f"sA{k}", QW) for k in range(2)]
        sB_l = [T16(f"sB{k}", QW) for k in range(2)]
        sCt_l = [T16("sCt0", QW)] * 2
        rm = T16("rm", COLS)
        colT = T16("colT", NSUP * G)
        cpa = T16("cpa", G)
        forced = T16("forced", COLS)
        pos16 = T16("pos16", COLS)
        posf = T("posf", COLS)
        cmb = T16("cmb", G)
        cmaxpl = T16("cmaxpl", W)

        gtrow_l = [T(f"gtrow{b}", G * 4, parts=1) for b in range(IMG)]
        gtall_l = [T(f"gtall{b}", G * 4) for b in range(IMG)]
        gsc = [T(f"gsc{c}", G) for c in range(4)]
        sGf = T("sGf", G)
        tG = T("tG", G)
        ngx1p = T16("ngx1p", W)
        ngy1p = T16("ngy1p", W)
        gx2p = T16("gx2p", W)
        gy2p = T16("gy2p", W)
        sGp = T16("sGp", W)
        gtmatf_l = [T(f"gtmatf{b}", 16) for b in range(IMG)]
        gtmath_l = [T16(f"gtmath{b}", 16) for b in range(IMG)]

        tsb_l = [T16(f"tsb{k}", W) for k in range(2)]
        matched = T16("matched", 4 * COLS)   # (s, u, c)
        mch = [T16(f"mch{c}", COLS) for c in range(4)]
        bxh = [T16(f"bxh{c}", COLS) for c in range(4)]
        areaPh = T16("areaPh", COLS)
        bbox_sb_l = [T(f"bbox_sb{b}", COLS * 4) for b in range(IMG)]
        conf_sb_l = [T(f"conf_sb{b}", COLS) for b in range(IMG)]

        d0 = T16("d0", COLS)
        d1 = T16("d1", COLS)
        d2 = T16("d2", COLS)
        d3 = T16("d3", COLS)
        d4 = T16("d4", COLS)

        s0 = T("s0", COLS)
        s1 = T("s1", COLS)
        s2 = T("s2", COLS)
        s3 = T("s3", COLS)
        s4 = T("s4", COLS)
        s5 = T("s5", COLS)
        cl = T("cl", COLS)
        nv = T("nv", COLS)
        nv16 = T16("nv16", COLS)
        sink16 = T16("sink16", COLS)

        npp = T("npp", 1)
        locsum_pp = T("locsum_pp", 1)
        possum_pp = T("possum_pp", 1)
        cnt_pp = T("cnt_pp", 1)
        sum_pp = T("sum_pp", 1)
        maxv_pp = T("maxv_pp", 1)
        maxvb = T("maxvb", 1)
        w1c = T("w1c", 1)
        tau_b = T("tau_b", 1)
        stack = T("stack", 4)
        thr = T("thr", NBIN)
        nthr = T("nthr", NBIN)
        cge = T("cge", NBIN)
        wl = [T(f"wl{l}", 1) for l in range(NLEV)]
        lo_b = [T(f"lo_b{l}", 1) for l in range(NLEV)]
        cget = T("cget", NBIN, parts=1)
        gek = T("gek", NBIN, parts=1)
        scnt = T("scnt", 1, parts=1)
        lo_new = T("lo_new", 1, parts=1)
        tau = [T(f"tau{l}", 1, parts=1) for l in range(NLEV)]
        maxv1 = T("maxv1", 1, parts=1)
        npos1 = T("npos1", 1, parts=1)
        k1 = T("k1", 1, parts=1)
        k2 = T("k2", 1, parts=1)
        kk = T("kk", 1, parts=1)
        mx_row = T("mx_row", P, parts=1)
        ctt = T16("ctt", P, parts=G)
        cmax_col = T16("cmax_col", 1, parts=G)
        cm_row = T16("cm_row", G, parts=1)
        res_sb = T("res_sb", 8, parts=1)

        # 4D view helpers
        def q4gu(t, q):  # (s, g, u) packed quarter
            return t[:, q*QW:(q+1)*QW].rearrange("p (s g u) -> p s g u", g=G, u=U)

        def aview(t, q):  # anchor [P, COLS] -> [p, s, g(b), u]
            return (t[:, q*QSUP*U:(q+1)*QSUP*U]
                    .rearrange("p (s u) -> p s u", u=U)
                    .unsqueeze(2).to_broadcast([P, QSUP, G, U]))

        def gview(t):     # gt plane [P, W] -> [p, s(b), g, u]
            return (t[:].rearrange("p (g u) -> p g u", u=U)
                    .unsqueeze(1).to_broadcast([P, QSUP, G, U]))

        for b in range(IMG):
            bbox_sb = bbox_sb_l[b]
            conf_sb = conf_sb_l[b]
            gtrow = gtrow_l[b]
            gtall = gtall_l[b]
            gtmatf = gtmatf_l[b]
            gtmath = gtmath_l[b]
            # ---------------- loads ----------------
            nc.sync.dma_start(bbox_sb[:, 0:COLS * 2], bbox_d[b][:, 0:COLS * 2])
            nc.sync.dma_start(bbox_sb[:, COLS * 2:], bbox_d[b][:, COLS * 2:])
            nc.sync.dma_start(conf_sb[:], conf_d[b])
            nc.scalar.dma_start(gtrow[:], gt_d[b])
            gt_ps = pspool.tile([P, G * 4], F32, name="gt_ps", tag="gtp")
            nc.tensor.matmul(gt_ps[:], ones_row[:], gtrow[:])
            v.tensor_copy(gtall[:], gt_ps[:])
            gt3 = gtall[:].rearrange("p (g c) -> p g c", c=4)
            gt2d = gt_d[b].rearrange("q (g c) -> (q g) c", c=4)
            v.memset(gtmatf[:], 0.0)
            for uu in range(4):
                nc.scalar.dma_start(gtmatf[uu*G:(uu+1)*G, uu*4:(uu+1)*4], gt2d)
            sc.activation(gtmath[:], gtmatf[:], Act.Copy, scale=SC)

            # ---------------- gt prep ----------------
            v.tensor_scalar(gsc[0][:], gt3[:, :, 0:1].squeeze(2), -SC, None, Alu.mult)
            v.tensor_scalar(gsc[1][:], gt3[:, :, 1:2].squeeze(2), -SC, None, Alu.mult)
            v.tensor_scalar(gsc[2][:], gt3[:, :, 2:3].squeeze(2), SC, None, Alu.mult)
            v.tensor_scalar(gsc[3][:], gt3[:, :, 3:4].squeeze(2), SC, None, Alu.mult)
            v.tensor_tensor(out=sGf[:], in0=gt3[:, :, 2:3].squeeze(2),
                            in1=gt3[:, :, 0:1].squeeze(2), op=Alu.subtract)
            v.tensor_tensor(out=tG[:], in0=gt3[:, :, 3:4].squeeze(2),
                            in1=gt3[:, :, 1:2].squeeze(2), op=Alu.subtract)
            v.tensor_tensor(out=sGf[:], in0=sGf[:], in1=tG[:], op=Alu.mult)
            v.tensor_scalar(sGf[:], sGf[:], SC * SC, None, Alu.mult)
            for pl, src in ((ngx1p, gsc[0]), (ngy1p, gsc[1]), (gx2p, gsc[2]),
                            (gy2p, gsc[3]), (sGp, sGf)):
                sc.activation(pl[:].rearrange("p (g u) -> p g u", u=U),
                              src[:].unsqueeze(2).to_broadcast([P, G, U]), Act.Copy)

            # ---------------- pass 1: grid ----------------
            for q in range(NQ):
                sA = sA_l[q % 2]
                sB = sB_l[q % 2]
                sCt = sCt_l[q % 2]
                gq = q4gu(grid, q)
                a4 = q4gu(sA, 0)
                b4 = q4gu(sB, 0)
                c4 = q4gu(sCt, 0)
                v.tensor_tensor(out=a4, in0=aview(nax1h, q), in1=gview(ngx1p), op=Alu.min)
                v.tensor_tensor(out=b4, in0=aview(ax2h, q), in1=gview(gx2p), op=Alu.min)
                v.tensor_tensor(out=a4, in0=b4, in1=a4, op=Alu.add)
                sc.activation(sA[:], sA[:], Act.Relu)
                v.tensor_tensor(out=b4, in0=aview(nay1h, q), in1=gview(ngy1p), op=Alu.min)
                v.tensor_tensor(out=c4, in0=aview(ay2h, q), in1=gview(gy2p), op=Alu.min)
                v.tensor_tensor(out=b4, in0=c4, in1=b4, op=Alu.add)
                v.tensor_scalar(sB[:], sB[:], 0.0, None, Alu.max)
                v.tensor_tensor(out=a4, in0=a4, in1=b4, op=Alu.mult)      # inter
                HW2 = QW // 2
                for hh in range(2):
                    s0q = q * QSUP + hh * (QSUP // 2)
                    hsl = slice(hh * HW2, (hh + 1) * HW2)
                    bh = (sB_l[q % 2][:, hsl]
                          .rearrange("p (s g u) -> p s g u", g=G, u=U))
                    ah = (sA_l[q % 2][:, hsl]
                          .rearrange("p (s g u) -> p s g u", g=G, u=U))
                    gh = (grid[:, q*QW + hh*HW2: q*QW + (hh+1)*HW2]
                          .rearrange("p (s g u) -> p s g u", g=G, u=U))
                    avh = (areaAh[:, s0q*U:(s0q + QSUP//2)*U]
                           .rearrange("p (s u) -> p s u", u=U)
                           .unsqueeze(2).to_broadcast([P, QSUP//2, G, U]))
                    gvh = (sGp[:].rearrange("p (g u) -> p g u", u=U)
                           .unsqueeze(1).to_broadcast([P, QSUP//2, G, U]))
                    v.tensor_tensor(out=bh, in0=avh, in1=gvh, op=Alu.add)
                    _act_recip(nc, sB_l[q % 2][:, hsl], sB_l[q % 2][:, hsl])
                    v.tensor_tensor(out=gh, in0=ah, in1=bh, op=Alu.mult)

                cur = gq
                width = G
                while width > 1:
                    h = width // 2
                    if h == 1:
                        dst = (rm[:, q*QSUP*U:(q+1)*QSUP*U]
                               .rearrange("p (s u) -> p s u", u=U).unsqueeze(2))
                    else:
                        dst = q4gu(sCt, 0)[:, :, 0:h, :]
                    v.tensor_tensor(out=dst, in0=cur[:, :, 0:h, :],
                                    in1=cur[:, :, h:width, :], op=Alu.max)
                    cur = dst
                    width = h
                curc = gq
                widc = U
                while widc > 1:
                    hc = widc // 2
                    if hc == 1:
                        dstc = (colT[:, q*QSUP*G:(q+1)*QSUP*G]
                                .rearrange("p (s g) -> p s g", g=G).unsqueeze(3))
                    else:
                        dstc = q4gu(sB, 0)[:, :, :, 0:hc]
                    v.tensor_tensor(out=dstc, in0=curc[:, :, :, 0:hc],
                                    in1=curc[:, :, :, hc:widc], op=Alu.max)
                    curc = dstc
                    widc = hc

            # ---------------- cmax finalize ----------------
            v.tensor_reduce(out=cpa[:], in_=colT[:].rearrange("p (s g) -> p g s", g=G),
                            axis=Ax.X, op=Alu.max)
            ct_ps = pspool.tile([G, P], FP16, name="ct_ps", tag="pst")
            pe.transpose(ct_ps[:], cpa[:], ident[:])
            v.tensor_copy(ctt[:], ct_ps[:])
            v.tensor_reduce(out=cmax_col[:], in_=ctt[:], axis=Ax.X, op=Alu.max)
            cm_ps = pspool.tile([1, G], FP16, name="cm_ps", tag="pst")
            pe.transpose(cm_ps[:], cmax_col[:], ident[:G, :G])
            v.tensor_copy(cm_row[:], cm_ps[:])
            bc_ps = pspool.tile([P, G], F32, name="bc_ps", tag="pss")
            nc.tensor.matmul(bc_ps[:], ones_row16[:], cm_row[:])
            v.tensor_copy(cmb[:], bc_ps[:])
            sc.activation(cmaxpl[:].rearrange("p (g u) -> p g u", u=U),
                          cmb[:].unsqueeze(2).to_broadcast([P, G, U]), Act.Copy)

            # ---------------- forced + pos ----------------
            for q in range(NQ):
                sA = sA_l[q % 2]
                gq = q4gu(grid, q)
                v.tensor_tensor(out=q4gu(sA, 0), in0=gq, in1=gview(cmaxpl),
                                op=Alu.is_equal)
                cur = q4gu(sA, 0)
                width = G
                while width > 1:
                    h = width // 2
                    if h == 1:
                        dst = (forced[:, q*QSUP*U:(q+1)*QSUP*U]
                               .rearrange("p (s u) -> p s u", u=U).unsqueeze(2))
                    else:
                        dst = cur[:, :, 0:h, :]
                    v.tensor_tensor(out=dst, in0=cur[:, :, 0:h, :],
                                    in1=cur[:, :, h:width, :], op=Alu.max)
                    cur = dst
                    width = h
            v.tensor_scalar(pos16[:], rm[:], POS_THR, None, Alu.is_gt)
            v.tensor_tensor(out=pos16[:], in0=pos16[:], in1=forced[:], op=Alu.max)
            v.tensor_scalar(sink16[:], pos16[:], 1.0, 0.0, Alu.mult, Alu.add,
                            accum_out=npp[:])
            sc.copy(posf[:], pos16[:])

            # ---------------- ismax -> tgrid (s, u, g) ----------------
            for q in range(NQ):
                gin = grid[:, q*QW:(q+1)*QW].rearrange(
                    "p (s g u) -> p s u g", g=G, u=U)
                rmv = (rm[:, q*QSUP*U:(q+1)*QSUP*U]
                       .rearrange("p (s u) -> p s u", u=U)
                       .unsqueeze(3).to_broadcast([P, QSUP, U, G]))
                tout = tgrid[:, q*QW:(q+1)*QW].rearrange(
                    "p (s u g) -> p s u g", u=U, g=G)
                v.tensor_tensor(out=tout, in0=gin, in1=rmv, op=Alu.is_equal)

            # ---------------- matched coords (PE) ----------------
            for s in range(NSUP):
                tsb = tsb_l[s % 2]
                mout = pspool.tile([P, P], F32, name=f"mo{s % 2}", tag=f"mo{s % 2}")
                for j in range(8):
                    tp = pspool.tile([P, P], FP16, name=f"tp{j % 2}", tag=f"tp{j % 2}")
                    pe.transpose(tp[:], tgrid[:, s*W + j*P: s*W + (j+1)*P], ident[:])
                    sc.copy(tsb[:, j*P:(j+1)*P], tp[:])
                    nc.tensor.matmul(mout[:, j*16:(j+1)*16], tsb[:, j*P:(j+1)*P],
                                     gtmath[:])
                sc.copy(matched[:, s*P:(s+1)*P], mout[:])

            m4 = matched[:].rearrange("p (n c) -> p n c", c=4)
            for c in range(4):
                sc.copy(mch[c][:], m4[:, :, c:c+1].squeeze(2))

            # ---------------- bbox planes + diou ----------------
            bb3 = bbox_sb[:].rearrange("p (n c) -> p n c", c=4)
            for c in range(4):
                sc.activation(bxh[c][:], bb3[:, :, c:c+1].squeeze(2), Act.Copy,
                              scale=SC)
            v.tensor_tensor(out=d0[:], in0=bxh[2][:], in1=bxh[0][:], op=Alu.subtract)
            v.tensor_tensor(out=d1[:], in0=bxh[3][:], in1=bxh[1][:], op=Alu.subtract)
            v.tensor_tensor(out=areaPh[:], in0=d0[:], in1=d1[:], op=Alu.mult)

            # inter
            v.tensor_tensor(out=d0[:], in0=bxh[0][:], in1=mch[0][:], op=Alu.max)
            v.tensor_tensor(out=d1[:], in0=bxh[2][:], in1=mch[2][:], op=Alu.min)
            v.tensor_tensor(out=d0[:], in0=d1[:], in1=d0[:], op=Alu.subtract)
            v.tensor_scalar(d0[:], d0[:], 0.0, None, Alu.max)
            v.tensor_tensor(out=d1[:], in0=bxh[1][:], in1=mch[1][:], op=Alu.max)
            v.tensor_tensor(out=d2[:], in0=bxh[3][:], in1=mch[3][:], op=Alu.min)
            v.tensor_tensor(out=d1[:], in0=d2[:], in1=d1[:], op=Alu.subtract)
            v.tensor_scalar(d1[:], d1[:], 0.0, None, Alu.max)
            v.tensor_tensor(out=d0[:], in0=d0[:], in1=d1[:], op=Alu.mult)  # inter
            # matched area
            v.tensor_tensor(out=d1[:], in0=mch[2][:], in1=mch[0][:], op=Alu.subtract)
            v.tensor_tensor(out=d2[:], in0=mch[3][:], in1=mch[1][:], op=Alu.subtract)
            v.tensor_tensor(out=d1[:], in0=d1[:], in1=d2[:], op=Alu.mult)
            # union, iou
            v.tensor_tensor(out=d1[:], in0=d1[:], in1=areaPh[:], op=Alu.add)
            v.tensor_tensor(out=d1[:], in0=d1[:], in1=d0[:], op=Alu.subtract)
            _act_recip(nc, d1[:], d1[:])
            v.tensor_tensor(out=d0[:], in0=d0[:], in1=d1[:], op=Alu.mult)  # iou
            # enclosing c2
            v.tensor_tensor(out=d1[:], in0=bxh[0][:], in1=mch[0][:], op=Alu.min)
            v.tensor_tensor(out=d2[:], in0=bxh[2][:], in1=mch[2][:], op=Alu.max)
            v.tensor_tensor(out=d1[:], in0=d2[:], in1=d1[:], op=Alu.subtract)
            sc.activation(d1[:], d1[:], Act.Square)
            v.tensor_tensor(out=d2[:], in0=bxh[1][:], in1=mch[1][:], op=Alu.min)
            v.tensor_tensor(out=d3[:], in0=bxh[3][:], in1=mch[3][:], op=Alu.max)
            v.tensor_tensor(out=d2[:], in0=d3[:], in1=d2[:], op=Alu.subtract)
            sc.activation(d2[:], d2[:], Act.Square)
            v.tensor_tensor(out=d1[:], in0=d1[:], in1=d2[:], op=Alu.add)   # c2
            _act_recip(nc, d1[:], d1[:])
            # center dist d2 (quarter-scaled: absorbed by c2 ratio using same /2)
            v.tensor_tensor(out=d2[:], in0=bxh[0][:], in1=bxh[2][:], op=Alu.add)
            v.tensor_tensor(out=d3[:], in0=mch[0][:], in1=mch[2][:], op=Alu.add)
            v.tensor_tensor(out=d2[:], in0=d2[:], in1=d3[:], op=Alu.subtract)
            sc.activation(d2[:], d2[:], Act.Square)
            v.tensor_tensor(out=d3[:], in0=bxh[1][:], in1=bxh[3][:], op=Alu.add)
            v.tensor_tensor(out=d4[:], in0=mch[1][:], in1=mch[3][:], op=Alu.add)
            v.tensor_tensor(out=d3[:], in0=d3[:], in1=d4[:], op=Alu.subtract)
            sc.activation(d3[:], d3[:], Act.Square)
            v.tensor_tensor(out=d2[:], in0=d2[:], in1=d3[:], op=Alu.add)   # 4*d2
            v.tensor_tensor(out=d2[:], in0=d2[:], in1=d1[:], op=Alu.mult)
            v.tensor_scalar(d2[:], d2[:], 0.25, None, Alu.mult)            # d2/c2
            v.tensor_scalar(d0[:], d0[:], -1.0, 1.0, Alu.mult, Alu.add)    # 1-iou
            v.tensor_tensor(out=d2[:], in0=d2[:], in1=d0[:], op=Alu.add)
            v.tensor_scalar(d2[:], d2[:], 100.0, None, Alu.min)
            v.tensor_tensor(out=d2[:], in0=d2[:], in1=pos16[:], op=Alu.mult)
            v.tensor_scalar(sink16[:], d2[:], 1.0, 0.0, Alu.mult, Alu.add,
                            accum_out=locsum_pp[:])

            # ---------------- focal conf loss (f32, baseline) ----------------
            sc.activation(s0[:], conf_sb[:], Act.Sigmoid)
            sc.activation(s1[:], conf_sb[:], Act.Exp)
            sc.activation(s1[:], s1[:], Act.Ln, bias=1.0)
            v.tensor_tensor(out=s2[:], in0=conf_sb[:], in1=posf[:], op=Alu.mult)
            v.tensor_tensor(out=s2[:], in0=s1[:], in1=s2[:], op=Alu.subtract)
            v.tensor_scalar(s3[:], posf[:], -2.0, 1.0, Alu.mult, Alu.add)
            v.tensor_tensor(out=s3[:], in0=s0[:], in1=s3[:], op=Alu.mult)
            v.tensor_tensor(out=s3[:], in0=s3[:], in1=posf[:], op=Alu.add)
            sc.activation(s3[:], s3[:], Act.Square)
            v.tensor_tensor(out=cl[:], in0=s3[:], in1=s2[:], op=Alu.mult)
            v.tensor_scalar(s3[:], posf[:], -0.5, 0.75, Alu.mult, Alu.add)
            v.tensor_tensor(out=cl[:], in0=cl[:], in1=s3[:], op=Alu.mult)
            v.tensor_scalar(cl[:], cl[:], 100.0, None, Alu.min)
            v.tensor_tensor(out=s4[:], in0=cl[:], in1=posf[:], op=Alu.mult)
            v.tensor_scalar(s5[:], s4[:], 1.0, 0.0, Alu.mult, Alu.add,
                            accum_out=possum_pp[:])
            v.tensor_tensor(out=nv[:], in0=cl[:], in1=s4[:], op=Alu.subtract)
            sc.copy(nv16[:], nv[:])

            # ---------------- hard negative mining (baseline) ----------------
            v.tensor_reduce(out=maxv_pp[:], in_=nv[:], axis=Ax.X, op=Alu.max)
            mx_ps = pspool.tile([1, P], F32, name="mx_ps", tag="pss")
            pe.transpose(mx_ps[:], maxv_pp[:], identf[:])
            v.tensor_copy(mx_row[:], mx_ps[:])
            v.tensor_reduce(out=maxv1[:], in_=mx_row[:], axis=Ax.X, op=Alu.max)

            np_ps = pspool.tile([1, 1], F32, name="np_ps", tag="pss")
            nc.tensor.matmul(np_ps[:], ones_col[:], npp[:])
            v.tensor_copy(npos1[:], np_ps[:])
            v.tensor_scalar(k1[:], npos1[:], NEG_POS_RATIO, None, Alu.mult)
            v.tensor_scalar(k2[:], npos1[:], -1.0, float(A), Alu.mult, Alu.add)
            v.tensor_tensor(out=kk[:], in0=k1[:], in1=k2[:], op=Alu.min)

            pbcast(maxvb[:], maxv1[:])
            v.tensor_scalar(w1c[:], maxvb[:], 1.0 / NBIN, None, Alu.mult)

            for lev in range(NLEV):
                if lev == 0:
                    v.tensor_copy(wl[0][:], w1c[:])
                    v.tensor_scalar(thr[:], iota_f[:], wl[0][:], None, Alu.mult)
                else:
                    v.tensor_scalar(wl[lev][:], wl[lev - 1][:], 1.0 / NBIN, None,
                                    Alu.mult)
                    v.tensor_scalar(thr[:], iota_f[:], wl[lev][:], lo_b[lev - 1][:],
                                    Alu.mult, Alu.add)
                v.tensor_scalar(nthr[:], thr[:], -1.0, None, Alu.mult)
                # img0 mining overlaps img1 grid (DVE-bound): all bins on Act.
                # img1 mining is the tail: split bins across Act and DVE.
                nact = NBIN if b == 0 else NBIN // 2
                for bn in range(nact):
                    sc.activation(sink16[:], nv16[:], Act.Sign,
                                  bias=nthr[:, bn:bn+1], accum_out=cge[:, bn:bn+1])
                for bn in range(nact, NBIN):
                    v.tensor_scalar(d4[:], nv16[:], thr[:, bn:bn+1], 0.0,
                                    Alu.is_gt, Alu.add, accum_out=cge[:, bn:bn+1])
                cg_ps = pspool.tile([1, NBIN], F32, name="cg_ps", tag="pss")
                nc.tensor.matmul(cg_ps[:], ones_col[:], cge[:])
                v.tensor_copy(cget[:], cg_ps[:])
                # Act bins hold sign-sums: cnt = (acc + A)/2; DVE bins exact
                v.tensor_scalar(cget[:, 0:nact], cget[:, 0:nact], 0.5,
                                float(A) * 0.5, Alu.mult, Alu.add)
                v.tensor_scalar(gek[:], cget[:], kk[:], None, Alu.is_ge)
                v.tensor_reduce(out=scnt[:], in_=gek[:], axis=Ax.X, op=Alu.add)
                v.tensor_scalar(lo_new[:], scnt[:], 1.0, wl[lev][0:1, :],
                                Alu.subtract, Alu.mult)
                v.tensor_scalar(tau[lev][:], scnt[:], wl[lev][0:1, :], None, Alu.mult)
                if lev > 0:
                    v.tensor_tensor(out=lo_new[:], in0=lo_new[:],
                                    in1=lo_b[lev - 1][0:1, :], op=Alu.add)
                    v.tensor_tensor(out=tau[lev][:], in0=tau[lev][:],
                                    in1=lo_b[lev - 1][0:1, :], op=Alu.add)
                pbcast(lo_b[lev][:], lo_new[:])

            pbcast(tau_b[:], tau[NLEV - 1][:])
            v.tensor_scalar(s4[:], nv[:], tau_b[:], 0.0, Alu.is_gt,
                            Alu.add, accum_out=cnt_pp[:])
            v.tensor_tensor(out=s5[:], in0=nv[:], in1=s4[:], op=Alu.mult)
            v.tensor_scalar(s5[:], s5[:], 1.0, 0.0, Alu.mult, Alu.add,
                            accum_out=sum_pp[:])

            # ---------------- gather scalars ----------------
            v.tensor_copy(stack[:, 0:1], npp[:])
            v.tensor_copy(stack[:, 1:2], locsum_pp[:])
            v.tensor_copy(stack[:, 2:3], possum_pp[:])
            v.tensor_copy(stack[:, 3:4], cnt_pp[:])
            st_ps = pspool.tile([1, 4], F32, name="st_ps", tag="pss")
            nc.tensor.matmul(st_ps[:], ones_col[:], stack[:])
            sm_ps = pspool.tile([1, 1], F32, name="sm_ps", tag="pss")
            nc.tensor.matmul(sm_ps[:], ones_col[:], sum_pp[:])

            v.tensor_copy(res_sb[:, 0:4], st_ps[:])
            v.tensor_copy(res_sb[:, 4:5], sm_ps[:])
            v.tensor_copy(res_sb[:, 5:6], tau[NLEV - 1][:])
            v.tensor_copy(res_sb[:, 6:7], maxv1[:])
            v.tensor_copy(res_sb[:, 7:8], kk[:])
            nc.sync.dma_start(res_d[b], res_sb[:])

    nc.compile()
    return nc


_NC_CACHE = None


def _get_nc():
    global _NC_CACHE
    if _NC_CACHE is None:
        _NC_CACHE = _build_nc()
    return _NC_CACHE


def _make_in_maps(inputs):
    bbox_pred = np.asarray(inputs["bbox_pred"])
    conf_pred = np.asarray(inputs["conf_pred"])
    anchors = np.asarray(inputs["anchors"])
    gt_boxes = np.asarray(inputs["gt_boxes"])
    anch_h = np.ascontiguousarray(anchors.reshape(P, COLS * 4), dtype=np.float32)
    in_maps = []
    for i in range(NCORE):
        bsl = slice(IMG * i, IMG * (i + 1))
        in_maps.append({
            "anch": anch_h,
            "bbox": np.ascontiguousarray(
                bbox_pred[bsl].reshape(IMG, P, COLS * 4), dtype=np.float32),
            "conf": np.ascontiguousarray(
                conf_pred[bsl].reshape(IMG, P, COLS), dtype=np.float32),
            "gtb": np.ascontiguousarray(
                gt_boxes[bsl].reshape(IMG, 1, G * 4), dtype=np.float32),
        })
    return in_maps


def kernel(bbox_pred, conf_pred, anchors, gt_boxes):
    nc = _get_nc()
    in_maps = _make_in_maps(dict(bbox_pred=bbox_pred, conf_pred=conf_pred,
                                 anchors=anchors, gt_boxes=gt_boxes))
    out = run_bass_kernel_spmd(nc, in_maps, core_ids=list(range(NCORE)))

    loc_total = np.float32(0.0)
    conf_total = np.float32(0.0)
    npos_total = np.float32(0.0)
    for i in range(NCORE):
        res = out.results[i]["res"]  # [IMG, 1, 8]
        for b in range(IMG):
            npos, locsum, possum, cnt_gt, sum_gt, tau_hi, maxv, kdev = \
                [np.float32(x) for x in res[b, 0, :8]]
            k = np.float32(min(NEG_POS_RATIO * npos, A - npos))
            wl_last = np.float32(maxv / NBIN ** NLEV)
            rem = max(np.float32(0.0), np.float32(k - cnt_gt))
            neg = np.float32(sum_gt + rem * (tau_hi - wl_last * np.float32(0.5)))
            loc_total = np.float32(loc_total + locsum)
            conf_total = np.float32(conf_total + possum + neg)
            npos_total = np.float32(npos_total + npos)
    num_pos = np.float32(max(1.0, npos_total))
    loc_loss = np.float32(loc_total / num_pos)
    conf_loss = np.float32(conf_total / num_pos)
    return (np.float32(loc_loss + conf_loss), conf_loss, loc_loss)
